# revision 1
# baseline (speedup 1.0000x reference)
"""Trainium2 Bass kernel for nn_Encoder_78889959293176 (Autoformer-style encoder layer).

Strategy: data-parallel over batch (16 batches -> 8 cores x 2).
All heavy compute on the TensorEngine in a d-major ([channel, time]) layout:
  - QKV projections as W-stationary matmuls
  - autocorrelation statistic mean_value via Q K^T tiles + a 2-copy diagonal
    "shear" DMA into DRAM + ones-matmul partition reduction (flipped-tau space)
  - AllReduce(8 cores) of the batch-summed statistic, on-device top-22 mask
    (iterated max8 + match_replace) and masked softmax -> sparse weight vector g
  - the rolls-weighted aggregation as a circulant matmul against a Toeplitz
    band buffer built from g with a single broadcast DMA (no data-dependent
    indexing anywhere)
  - series decomposition via tensor_tensor_scan cumsum, convs as bf16 matmuls,
    layernorm stats via ones-matmuls.

v3 pipeline layout (single pool scopes, no per-batch barriers):
  - the statistic AllReduce is split per batch: batch 0's collective is issued
    right after its phase 1 and hides under batch 1's phase-1 compute; only
    batch 1's collective sits near the critical path, shadowed by the V
    projections and topk.
  - phase-2 topk/softmax operates on [2, L] tiles (a partition per batch).
  - phases 3..7 interleave the two batches so the vector-engine chains
    (decompositions, layernorm) of one batch always overlap TensorEngine conv
    work of the other batch; layernorm stats borrow conv2's PSUM banks.
  - conv2 runs in two channel-half passes (partials parked in bf16 acc2) so h1
    only holds 8 of 16 hidden chunks.
  - all DRAM operands are host-staged partition-major: every DMA line is
    contiguous per partition.
"""

import numpy as np

import concourse.bass as bass
import concourse.bacc as bacc
import concourse.mybir as mybir
import concourse.tile as tile
from concourse import bass_utils
from concourse import library_config
from concourse.alu_op_type import AluOpType

try:
    import ml_dtypes

    BF16_NP = ml_dtypes.bfloat16
except Exception:  # pragma: no cover
    BF16_NP = np.float32

F32 = mybir.dt.float32
BF16 = mybir.dt.bfloat16
AF = mybir.ActivationFunctionType

B, L, D = 16, 2048, 512
CF = 2048  # conv hidden
TOPK = 22
KER = 25
EPS = 1e-5
SLOPE = 0.01
NCORES = 8
BPC = B // NCORES  # batches per core
DC = D // 128  # 4 d-chunks
CFC = CF // 128  # 16 conv-hidden chunks
CH = CFC // 2  # conv-hidden chunks per half
TW = L // 512  # 4 time windows of 512
TM = L // 128  # 16 time chunks of 128
NEG = -1.0e30


def build(nc: bass.Bass, n_group: int):
    x_dm = nc.dram_tensor("x_dm", [BPC, 128, DC * L], F32, kind="ExternalInput")
    wq_d = nc.dram_tensor("wq", [128, DC * D], BF16, kind="ExternalInput")
    wk_d = nc.dram_tensor("wk", [128, DC * D], BF16, kind="ExternalInput")
    wv_d = nc.dram_tensor("wv", [128, DC * D], BF16, kind="ExternalInput")
    wo_d = nc.dram_tensor("wo", [128, DC * D], BF16, kind="ExternalInput")
    bq_d = nc.dram_tensor("bq_t", [128, DC], F32, kind="ExternalInput")
    bk_d = nc.dram_tensor("bk_t", [128, DC], F32, kind="ExternalInput")
    bop_d = nc.dram_tensor("bop_t", [128, DC], F32, kind="ExternalInput")
    w1_d = nc.dram_tensor("w1h", [CFC, 128, DC * 3 * 128], BF16, kind="ExternalInput")
    w2_d = nc.dram_tensor("w2h", [DC, 128, CFC * 3 * 128], BF16, kind="ExternalInput")
    lng_d = nc.dram_tensor("lng_t", [128, DC], F32, kind="ExternalInput")
    lnb_d = nc.dram_tensor("lnb_t", [128, DC], F32, kind="ExternalInput")
    out_dm = nc.dram_tensor("out_dm", [BPC, 128, DC, L], F32, kind="ExternalOutput")

    with tile.TileContext(nc) as tc:
        _body(nc, tc, n_group, x_dm, wq_d, wk_d, wv_d, wo_d, bq_d, bk_d, bop_d,
              w1_d, w2_d, lng_d, lnb_d, out_dm)
    return nc


def _decompose_chunk(nc, scan_pool, src, dst, dci):
    """dst[:, dci, 1:L+1] = src[:, dci] - movavg_KER(src[:, dci])."""
    half = (KER - 1) // 2
    pad = scan_pool.tile([128, L + KER], F32, tag="scan_pad")
    cs = scan_pool.tile([128, L + KER], F32, tag="scan_cs")
    nc.vector.memset(pad[:, 0:1], 0.0)
    nc.vector.tensor_copy(
        out=pad[:, 1 : 1 + half],
        in_=src[:, dci, 0:1].to_broadcast([128, half]),
    )
    nc.scalar.activation(pad[:, 1 + half : 1 + half + L], src[:, dci, :], AF.Copy)
    nc.vector.tensor_copy(
        out=pad[:, 1 + half + L :],
        in_=src[:, dci, L - 1 : L].to_broadcast([128, half]),
    )
    nc.vector.tensor_tensor_scan(
        out=cs[:], data0=pad[:], data1=pad[:], initial=0.0,
        op0=AluOpType.add, op1=AluOpType.bypass,
    )
    d1 = pad[:, 0:L]  # cumsum done; reuse pad for the boxcar difference
    nc.vector.tensor_sub(out=d1, in0=cs[:, KER:], in1=cs[:, 0:L])
    nc.vector.scalar_tensor_tensor(
        out=dst[:, dci, 1 : L + 1], in0=d1, scalar=-1.0 / KER,
        in1=src[:, dci, :], op0=AluOpType.mult, op1=AluOpType.add,
    )
    nc.vector.tensor_copy(out=dst[:, dci, 0:1], in_=dst[:, dci, 1:2])
    nc.vector.tensor_copy(
        out=dst[:, dci, L + 1 : L + 2], in_=dst[:, dci, L : L + 1]
    )


def _decompose(nc, scan_pool, src, dst):
    for dci in range(DC):
        _decompose_chunk(nc, scan_pool, src, dst, dci)


def _body(nc, tc, n_group, x_dm, wq_d, wk_d, wv_d, wo_d, bq_d, bk_d, bop_d,
          w1_d, w2_d, lng_d, lnb_d, out_dm):
    with (
        tc.tile_pool(name="p0", bufs=1) as p0,
        tc.tile_pool(name="pp", bufs=1, space="PSUM") as pp,
        tc.tile_pool(name="dr", bufs=1, space="DRAM") as dr,
        tc.tile_pool(name="dr3", bufs=4, space="DRAM") as dr3,
    ):
        nc.gpsimd.load_library(library_config.attn)
        # ----- persistent constants -----
        ones_mv = p0.tile([128, 1], F32, tag="ones_mv")
        nc.vector.memset(ones_mv[:], 1.0 / D)
        ones_bf = p0.tile([128, 1], BF16, tag="ones_bf")
        nc.vector.memset(ones_bf[:], 1.0 / D)
        bq_c = p0.tile([128, DC], F32, tag="bq_c")
        bk_c = p0.tile([128, DC], F32, tag="bk_c")
        bop_c = p0.tile([128, DC], F32, tag="bop_c")
        lng_c = p0.tile([128, DC], F32, tag="lng_c")
        lnb_c = p0.tile([128, DC], F32, tag="lnb_c")
        nc.sync.dma_start(bq_c[:], bq_d[:, :])
        nc.sync.dma_start(bk_c[:], bk_d[:, :])
        nc.sync.dma_start(bop_c[:], bop_d[:, :])
        nc.sync.dma_start(lng_c[:], lng_d[:, :])
        nc.sync.dma_start(lnb_c[:], lnb_d[:, :])

        # 4 rotating PSUM accumulators shared by all phases
        ps4 = []
        for i in range(4):
            t = pp.tile([128, 512], F32, tag=f"ps{i}", name=f"ps_{i}")
            ps4.append(t)

        hb = {}
        seab = []
        cco = []

        with tc.tile_pool(name="psea", bufs=1) as psea:
            for b in range(BPC):
                t = psea.tile([128, DC, L + 2], BF16, tag=f"seab{b}",
                              name=f"seab_{b}")
                seab.append(t)

            with tc.tile_pool(name="pv", bufs=1) as pv:
                v_s = []
                for b in range(BPC):
                    t = pv.tile([128, TM, D], BF16, tag=f"v_s{b}", name=f"vs_{b}")
                    v_s.append(t)
                gfb = pv.tile([2, L], BF16, tag="gfb")

                def emit_hbuf(b):
                    # periodic replication: B[q] = g_f[q mod L]; a
                    # row-step-2047 read later yields
                    # Gbuf[i, p] = g_f[(127 + p - i) mod L].
                    # Split across the 3 DMA queues (each queue is one
                    # engine for a given transfer).
                    hbuf = dr.tile([1, 129 * L], BF16, tag=f"hb{b}")
                    _gs = gfb[b : b + 1, :]
                    _ga = [list(q) for q in _gs.ap]
                    hdst = hbuf[:].rearrange("a (r n) -> a r n", r=129)
                    bounds = (0, 43, 86, 129)
                    for j, eng in enumerate((nc.sync, nc.scalar, nc.gpsimd)):
                        r0, r1 = bounds[j], bounds[j + 1]
                        grep_ap = bass.AP(
                            _gs.tensor, _gs.offset,
                            [_ga[0], [0, r1 - r0], _ga[-1]],
                        )
                        eng.dma_start(hdst[0:1, r0:r1, :], grep_ap)
                    return hbuf

                with tc.tile_pool(name="pxv", bufs=1) as pxv:
                    xb = []
                    for b in range(BPC):
                        t = pxv.tile([128, DC, L], BF16, tag=f"xbt{b}",
                                     name=f"xb_{b}")
                        xb.append(t)
                    mvf = pxv.tile([1, BPC * L], F32, tag="mvf")
                    mvf2 = pxv.tile([2, L], F32, tag="mvf2")

                    # ========= phase 1: mean_value (flipped space) =========
                    with (
                        tc.tile_pool(name="ph1", bufs=1) as ph1,
                        tc.tile_pool(name="ph1b", bufs=4) as ph1b,
                        tc.tile_pool(name="ph1w", bufs=3) as ph1w,
                        tc.tile_pool(name="ppm1", bufs=1, space="PSUM") as ppm1,
                    ):
                        wq_s = ph1.tile([128, DC, D], BF16, tag="wqk")
                        wk_s = ph1.tile([128, DC, D], BF16, tag="wqk2")
                        nc.sync.dma_start(wq_s[:], wq_d.ap())
                        nc.sync.dma_start(wk_s[:], wk_d.ap())
                        # fine-grained x loads + bf16 converts, both batches
                        for b in range(BPC):
                            for dci in range(DC):
                                for tw in range(TW):
                                    xq = ph1b.tile([128, 512], F32, tag="xq")
                                    o = dci * L + 512 * tw
                                    nc.sync.dma_start(
                                        xq[:], x_dm.ap()[b, :, o : o + 512]
                                    )
                                    nc.scalar.activation(
                                        xb[b][:, dci, 512 * tw : 512 * tw + 512],
                                        xq[:], AF.Copy,
                                    )

                        mv_reg = []
                        for i in range(4):
                            t = ppm1.tile([1, 512], F32, tag=f"mv{i}",
                                          name=f"mv_{i}")
                            mv_reg.append(t)

                        for b in range(BPC):
                            q_s = ph1.tile([128, DC, L], BF16, tag="q_s")
                            k_s = ph1.tile([128, DC, L], BF16, tag="k_s")
                            for w_s, proj, bias in (
                                (wq_s, q_s, bq_c), (wk_s, k_s, bk_c)
                            ):
                                for dco in range(DC):
                                    for dci in range(DC):
                                        for twi in range(TW):
                                            nc.tensor.matmul(
                                                ps4[twi][:],
                                                lhsT=w_s[:, dci,
                                                         128 * dco : 128 * dco + 128],
                                                rhs=xb[b][:, dci,
                                                          512 * twi : 512 * twi + 512],
                                                start=(dci == 0),
                                                stop=(dci == DC - 1),
                                            )
                                    for twi in range(TW):
                                        nc.scalar.activation(
                                            proj[:, dco, 512 * twi : 512 * twi + 512],
                                            ps4[twi][:], AF.Identity,
                                            bias=bias[:, dco : dco + 1],
                                        )

                            def _emit_mv(A, wa):
                                for cc in range(4):
                                    w0 = (512 * cc + 128 * A) % L
                                    nc.tensor.matmul(
                                        mv_reg[cc][0:1, :],
                                        lhsT=ones_bf[:],
                                        rhs=wa[:, w0 : w0 + 512],
                                        start=(A == 0), stop=(A == TM - 1),
                                    )

                            pend = []
                            for A in range(TM):
                                bufA = dr3.tile([128, 4224], BF16, tag="bufA")
                                for dci in range(DC):
                                    for tB in range(TW):
                                        nc.tensor.matmul(
                                            ps4[tB][:],
                                            lhsT=q_s[:, dci,
                                                     128 * A : 128 * A + 128],
                                            rhs=k_s[:, dci,
                                                    512 * tB : 512 * tB + 512],
                                            start=(dci == 0), stop=(dci == DC - 1),
                                        )
                                for tB in range(TW):
                                    c_sb = ph1b.tile([128, 512], BF16, tag="c_sb")
                                    nc.scalar.activation(c_sb[:], ps4[tB][:],
                                                         AF.Copy)
                                    for cp, eng in ((0, nc.sync), (1, nc.scalar)):
                                        dst = bass.AP(
                                            bufA[:].tensor,
                                            127 + 512 * tB + 2048 * cp,
                                            [[4223, 128], [1, 512]],
                                        )
                                        eng.dma_start(dst, c_sb[:])
                                wa = ph1w.tile([128, 2560], BF16, tag="wa")
                                nc.sync.dma_start(
                                    wa[:],
                                    bass.AP(bufA[:].tensor, 128,
                                            [[4224, 128], [1, 2560]]),
                                )
                                pend.append((A, wa))
                                if len(pend) > 2:
                                    _emit_mv(*pend.pop(0))
                            for a_w in pend:
                                _emit_mv(*a_w)
                            for cc in range(4):
                                nc.scalar.activation(
                                    mvf[0:1,
                                        L * b + 512 * cc : L * b + 512 * cc + 512],
                                    mv_reg[cc][0:1, :], AF.Copy,
                                )
                            # issue this batch's AllReduce immediately: batch
                            # 0's collective hides under batch 1's phase 1.
                            cci = dr.tile([1, L], F32, tag=f"cci{b}")
                            cc_o = dr.tile([1, L], F32, tag=f"cco{b}")
                            nc.sync.dma_start(cci[:], mvf[0:1, L * b : L * b + L])
                            nc.gpsimd.collective_compute(
                                "AllReduce", AluOpType.add,
                                replica_groups=[list(range(n_group))],
                                ins=[cci[:].opt()], outs=[cc_o[:].opt()],
                            )
                            cco.append(cc_o)

                    # ========= phase 2: topk + softmax (both batches) ========
                    with (
                        tc.tile_pool(name="ph2", bufs=1) as ph2,
                        tc.tile_pool(name="ph2w", bufs=1) as ph2w,
                    ):
                        wv_s = ph2w.tile([128, DC, D], BF16, tag="wv_s")
                        nc.sync.dma_start(wv_s[:], wv_d.ap())

                        def _vproj(b):
                            for tm in range(TM):
                                pt = ps4[tm % 4]
                                for dci in range(DC):
                                    nc.tensor.matmul(
                                        pt[:],
                                        lhsT=xb[b][:, dci,
                                                   128 * tm : 128 * tm + 128],
                                        rhs=wv_s[:, dci, :],
                                        start=(dci == 0), stop=(dci == DC - 1),
                                    )
                                nc.scalar.activation(v_s[b][:, tm, :], pt[:],
                                                     AF.Copy)

                        _vproj(0)  # hides the second collective
                        nc.scalar.dma_start(mvf2[0:1, :], mvf[0:1, 0:L])
                        nc.scalar.dma_start(mvf2[1:2, :], mvf[0:1, L : 2 * L])
                        bs0 = ph2.tile([2, L], F32, tag="bs0")
                        bs1 = ph2.tile([2, L], F32, tag="bs1")
                        nc.sync.dma_start(
                            bs0[:], bass.AP(cco[0][:].tensor, 0, [[0, 2], [1, L]])
                        )
                        nc.sync.dma_start(
                            bs1[:], bass.AP(cco[1][:].tensor, 0, [[0, 2], [1, L]])
                        )
                        bsum = ph2.tile([2, L], F32, tag="bsum")
                        nc.vector.tensor_add(out=bsum[:], in0=bs0[:], in1=bs1[:])
                        # top-22 threshold via gpsimd kth_largest: the lerped
                        # quantile at rank 21.5 lies strictly between the 22nd
                        # and 23rd largest values -> tie-free >= mask.
                        bsA = ph2.tile([128, 16], F32, tag="bsA")
                        bsB = ph2.tile([128, 16], F32, tag="bsB")
                        nc.sync.dma_start(
                            bsA[:], bass.AP(cco[0][:].tensor, 0, [[16, 128], [1, 16]])
                        )
                        nc.sync.dma_start(
                            bsB[:], bass.AP(cco[1][:].tensor, 0, [[16, 128], [1, 16]])
                        )
                        bs128 = ph2.tile([128, 16], F32, tag="bs128")
                        nc.vector.tensor_add(out=bs128[:], in0=bsA[:], in1=bsB[:])
                        kth = ph2.tile([1, 2], F32, tag="kth")
                        nc.gpsimd.kth_largest(
                            kth[:], bs128[:], 16, 24,
                            quantile=1.0 - (TOPK - 0.5) / (L - 1),
                        )
                        thr2 = ph2.tile([2, 1], F32, tag="thr2")
                        nc.gpsimd.partition_broadcast(thr2[:], kth[0:1, 0:1],
                                                      channels=2)
                        mask = ph2.tile([2, L], F32, tag="mask")
                        nc.vector.tensor_scalar(
                            out=mask[:], in0=bsum[:], scalar1=thr2[:],
                            scalar2=None, op0=AluOpType.is_ge,
                        )
                        _vproj(1)  # hides topk/softmax
                        # masked softmax, both batches at once on [2, L]
                        gf = ph2.tile([2, L], F32, tag="gf")
                        neg9 = ph2.tile([2, L], F32, tag="neg9")
                        nc.vector.tensor_scalar(
                            out=neg9[:], in0=mask[:], scalar1=1.0, scalar2=1.0e9,
                            op0=AluOpType.subtract, op1=AluOpType.mult,
                        )
                        nc.vector.tensor_mul(out=gf[:], in0=mvf2[:], in1=mask[:])
                        nc.vector.tensor_add(out=gf[:], in0=gf[:], in1=neg9[:])
                        zz = ph2.tile([2, 1], F32, tag="sm_z")
                        nc.scalar.activation(gf[:], gf[:], AF.Exp,
                                             accum_out=zz[:])
                        nc.vector.reciprocal(out=zz[:], in_=zz[:])
                        nc.scalar.activation(gfb[:], gf[:], AF.Copy, scale=zz[:])
                        # periodic replication: B[q] = g_f[q mod L]; a
                        # row-step-2047 read later yields
                        # Gbuf[i, p] = g_f[(127 + p - i) mod L]
                        hb[1] = emit_hbuf(1)

                # == phases 3-4, batch order b1 then b0 (decomp overlaps PE) ==
                with (
                    tc.tile_pool(name="p3", bufs=1) as p3,
                    tc.tile_pool(name="p3r", bufs=2) as p3r,
                    tc.tile_pool(name="p3s", bufs=1) as p3s,
                ):
                    wo_s = p3.tile([128, DC, D], BF16, tag="wo_s")
                    nc.sync.dma_start(wo_s[:], wo_d.ap())
                    for b in (1, 0):
                        gbuf = p3.tile([128, 3968], BF16, tag="gbuf")
                        gsplit = ((0, 43), (43, 86), (86, 128))
                        for (i0, i1), eng in zip(
                            gsplit, (nc.sync, nc.scalar, nc.gpsimd)
                        ):
                            eng.dma_start(
                                gbuf[i0:i1, :],
                                bass.AP(hb[b][:].tensor, 127 + 2047 * i0,
                                        [[2047, i1 - i0], [1, 3968]]),
                            )
                        if b == 1:
                            # batch 0's broadcast write goes out after batch
                            # 1's reads and hides under agg(b1) PE work
                            hb[0] = emit_hbuf(0)
                        agg = p3.tile([128, DC, L], BF16, tag="agg")
                        for dm in range(DC):
                            for Bc in range(TM):
                                for nw in range(TW):
                                    gp = 512 * nw - 128 * Bc + 1920
                                    nc.tensor.matmul(
                                        ps4[nw][:],
                                        lhsT=v_s[b][:, Bc,
                                                    128 * dm : 128 * dm + 128],
                                        rhs=gbuf[:, gp : gp + 512],
                                        start=(Bc == 0), stop=(Bc == TM - 1),
                                    )
                            for nw in range(TW):
                                nc.scalar.activation(
                                    agg[:, dm, 512 * nw : 512 * nw + 512],
                                    ps4[nw][:], AF.Copy,
                                )
                        acx = p3.tile([128, DC, L], F32, tag="acx")
                        for dco in range(DC):
                            for dci in range(DC):
                                for twi in range(TW):
                                    nc.tensor.matmul(
                                        ps4[twi][:],
                                        lhsT=wo_s[:, dci,
                                                  128 * dco : 128 * dco + 128],
                                        rhs=agg[:, dci,
                                                512 * twi : 512 * twi + 512],
                                        start=(dci == 0), stop=(dci == DC - 1),
                                    )
                            for twi in range(TW):
                                xr = p3r.tile([128, 512], F32, tag="xr")
                                nc.sync.dma_start(
                                    xr[:],
                                    x_dm.ap()[b, :,
                                              dco * L + 512 * twi :
                                              dco * L + 512 * twi + 512],
                                )
                                nc.vector.scalar_tensor_tensor(
                                    out=acx[:, dco, 512 * twi : 512 * twi + 512],
                                    in0=ps4[twi][:],
                                    scalar=bop_c[:, dco : dco + 1],
                                    in1=xr[:],
                                    op0=AluOpType.add, op1=AluOpType.add,
                                )
                            # this channel chunk of acx is complete: its
                            # decomposition scan can overlap the next chunks'
                            # matmuls (and the following conv stages)
                            _decompose_chunk(nc, p3s, acx, seab[b], dco)

            # ======= phases 5-7, interleaved across the two batches =======
            with (
                tc.tile_pool(name="pcv", bufs=1) as pcv,
                tc.tile_pool(name="p5w1", bufs=2) as p5w1,
                tc.tile_pool(name="p5w2", bufs=2) as p5w2,
                tc.tile_pool(name="p6s", bufs=2) as p6s,
                tc.tile_pool(name="p7s", bufs=1) as p7s,
                tc.tile_pool(name="p7b", bufs=1) as p7b,
                tc.tile_pool(name="p7d", bufs=2, space="DRAM") as p7d,
                tc.tile_pool(name="ppc2", bufs=1, space="PSUM") as ppc2,
            ):
                pc2 = []
                for i in range(4):
                    t = ppc2.tile([128, 512], F32, tag=f"pc2_{i}", name=f"pc2_{i}")
                    pc2.append(t)
                h1 = pcv.tile([128, CH, L + 2], BF16, tag="h1")
                acc2 = pcv.tile([128, DC, L], BF16, tag="acc2")
                ysb = pcv.tile([128, DC, L], F32, tag="ysb")
                sea2 = pcv.tile([128, DC, L + 2], F32, tag="sea2")

                def conv1_half(b, half):
                    for co8 in range(CH):
                        co = CH * half + co8
                        w1t = p5w1.tile([128, DC * 3 * 128], BF16, tag="w1t")
                        nc.sync.dma_start(w1t[:], w1_d.ap()[co])
                        first = True
                        for dci in range(DC):
                            for tap in range(3):
                                ki = (3 * dci + tap) * 128
                                for nw in range(TW):
                                    nc.tensor.matmul(
                                        ps4[nw][:],
                                        lhsT=w1t[:, ki : ki + 128],
                                        rhs=seab[b][:, dci,
                                             512 * nw + tap : 512 * nw + tap + 512],
                                        start=first,
                                        stop=(dci == DC - 1 and tap == 2),
                                    )
                                first = False
                        for nw in range(TW):
                            nc.scalar.activation(
                                h1[:, co8, 1 + 512 * nw : 513 + 512 * nw],
                                ps4[nw][:], AF.Prelu, alpha=SLOPE,
                            )
                        nc.vector.tensor_copy(out=h1[:, co8, 0:1],
                                              in_=h1[:, co8, 1:2])
                        nc.vector.tensor_copy(out=h1[:, co8, L + 1 : L + 2],
                                              in_=h1[:, co8, L : L + 1])

                def conv2_pass(b, half):
                    for co in range(DC):
                        w2t = p5w2.tile([128, CH * 3 * 128], BF16, tag="w2t")
                        src = w2_d.ap()[co]
                        ofs = half * CH * 3 * 128
                        nc.sync.dma_start(
                            w2t[:],
                            bass.AP(src.tensor, src.offset + ofs,
                                    [[CFC * 3 * 128, 128], [1, CH * 3 * 128]]),
                        )
                        first = True
                        for ci8 in range(CH):
                            for tap in range(3):
                                ki = (3 * ci8 + tap) * 128
                                for nw in range(TW):
                                    nc.tensor.matmul(
                                        pc2[nw][:],
                                        lhsT=w2t[:, ki : ki + 128],
                                        rhs=h1[:, ci8,
                                             512 * nw + tap : 512 * nw + tap + 512],
                                        start=first,
                                        stop=(ci8 == CH - 1 and tap == 2),
                                    )
                                first = False
                        if half == 0:
                            for nw in range(TW):
                                nc.scalar.activation(
                                    acc2[:, co, 512 * nw : 512 * nw + 512],
                                    pc2[nw][:], AF.Copy,
                                )
                        else:
                            for nw in range(TW):
                                h2t = p6s.tile([128, 512], F32, tag="h2t")
                                nc.vector.tensor_add(
                                    out=h2t[:], in0=pc2[nw][:],
                                    in1=acc2[:, co, 512 * nw : 512 * nw + 512],
                                )
                                h2r = p6s.tile([128, 512], F32, tag="h2r")
                                nc.scalar.activation(h2r[:], h2t[:], AF.Prelu,
                                                     alpha=SLOPE)
                                nc.vector.tensor_add(
                                    out=ysb[:, co, 512 * nw : 512 * nw + 512],
                                    in0=h2r[:],
                                    in1=seab[b][:, co,
                                                1 + 512 * nw : 513 + 512 * nw],
                                )
                            # decompose2 of this channel chunk can start now
                            _decompose_chunk(nc, p7s, ysb, sea2, co)

                def layernorm(b):
                    # windowed pipeline; stats borrow conv2's PSUM banks
                    for twi in range(TW):
                        st_s = pc2[2 * (twi % 2)][0:1, :]
                        st_q = pc2[2 * (twi % 2) + 1][0:1, :]
                        for dci in range(DC):
                            sqt = p6s.tile([128, 512], F32, tag="sqt")
                            nc.scalar.activation(
                                sqt[:],
                                sea2[:, dci, 1 + 512 * twi : 513 + 512 * twi],
                                AF.Square,
                            )
                            nc.tensor.matmul(
                                st_s,
                                lhsT=ones_mv[:],
                                rhs=sea2[:, dci, 1 + 512 * twi : 513 + 512 * twi],
                                start=(dci == 0), stop=(dci == DC - 1),
                            )
                            nc.tensor.matmul(
                                st_q,
                                lhsT=ones_mv[:],
                                rhs=sqt[:],
                                start=(dci == 0), stop=(dci == DC - 1),
                            )
                        mu = p7b.tile([1, 512], F32, tag="mu")
                        rs = p7b.tile([1, 512], F32, tag="rs")
                        nc.scalar.activation(mu[:], st_s, AF.Copy)
                        nc.vector.tensor_mul(out=rs[:], in0=mu[:], in1=mu[:])
                        nc.vector.tensor_sub(out=rs[:], in0=st_q, in1=rs[:])
                        nc.vector.tensor_scalar_add(rs[:], rs[:], EPS)
                        nc.vector.reciprocal(out=rs[:], in_=rs[:])
                        nc.scalar.activation(rs[:], rs[:], AF.Sqrt)
                        mub = p7b.tile([128, 512], F32, tag="mub")
                        rsb = p7b.tile([128, 512], F32, tag="rsb")
                        nc.gpsimd.partition_broadcast(mub[:], mu[:])
                        nc.gpsimd.partition_broadcast(rsb[:], rs[:])
                        for dci in range(DC):
                            ve = nc.vector
                            og = p6s.tile([128, 512], F32, tag="og")
                            ve.tensor_sub(
                                out=og[:],
                                in0=sea2[:, dci, 1 + 512 * twi : 513 + 512 * twi],
                                in1=mub[:],
                            )
                            ve.scalar_tensor_tensor(
                                out=og[:], in0=og[:],
                                scalar=lng_c[:, dci : dci + 1], in1=rsb[:],
                                op0=AluOpType.mult, op1=AluOpType.mult,
                            )
                            nc.scalar.activation(
                                og[:], og[:], AF.Identity,
                                bias=lnb_c[:, dci : dci + 1],
                            )
                            nc.scalar.dma_start(
                                out_dm.ap()[b, :, dci,
                                            512 * twi : 512 * twi + 512],
                                og[:],
                            )

                # schedule: conv(b1) fully; LN(b1) sits between batch-0 conv
                # stages so its vector/DMA chains hide under PE work.
                conv1_half(1, 0)
                conv2_pass(1, 0)
                conv1_half(1, 1)
                conv2_pass(1, 1)      # finalize emits decompose2(b1) per chunk
                conv1_half(0, 0)
                layernorm(1)          # stats borrow pc2 (free here)
                conv2_pass(0, 0)
                conv1_half(0, 1)
                conv2_pass(0, 1)      # finalize emits decompose2(b0) per chunk
                layernorm(0)


# ---------------------------------------------------------------------------
# host side
# ---------------------------------------------------------------------------
_CACHE = {}


def _get_nc(n_group: int):
    key = n_group
    if key not in _CACHE:
        nc = bacc.Bacc("TRN2", target_bir_lowering=False, debug=False,
                       num_devices=n_group)
        build(nc, n_group)
        nc.compile()
        _CACHE[key] = nc
    return _CACHE[key]


def stage_inputs(inputs, ncores=NCORES):
    x = np.asarray(inputs["x"], np.float32)
    Wq = np.asarray(inputs["Wq"], np.float32)
    Wk = np.asarray(inputs["Wk"], np.float32)
    Wv = np.asarray(inputs["Wv"], np.float32)
    Wo = np.asarray(inputs["Wo"], np.float32)
    bq = np.asarray(inputs["bq"], np.float32)
    bk = np.asarray(inputs["bk"], np.float32)
    bv = np.asarray(inputs["bv"], np.float32)
    bo = np.asarray(inputs["bo"], np.float32)
    w1 = np.asarray(inputs["conv1_w"], np.float32)
    w2 = np.asarray(inputs["conv2_w"], np.float32)
    lng = np.asarray(inputs["ln_g"], np.float32)
    lnb = np.asarray(inputs["ln_b"], np.float32)

    bop = bo + bv @ Wo
    col = lambda v: np.ascontiguousarray(v.reshape(DC, 128).T)
    # projection weights partition-major: W[dci*128+p, n] -> [p, dci*D + n]
    wmaj = lambda W: np.ascontiguousarray(
        W.reshape(DC, 128, D).transpose(1, 0, 2).reshape(128, DC * D)
    ).astype(BF16_NP)
    # conv1 [3, D, CF] -> [CFC, 128(ci-part), DC*3*128(co)]
    w1h = np.ascontiguousarray(
        w1.reshape(3, DC, 128, CFC, 128).transpose(3, 2, 1, 0, 4)
    ).reshape(CFC, 128, DC * 3 * 128).astype(BF16_NP)
    # conv2 [3, CF, D] -> [DC, 128(ci-part), CFC*3*128(co)]
    w2h = np.ascontiguousarray(
        w2.reshape(3, CFC, 128, DC, 128).transpose(3, 2, 1, 0, 4)
    ).reshape(DC, 128, CFC * 3 * 128).astype(BF16_NP)

    shared = {
        "wq": wmaj(Wq), "wk": wmaj(Wk), "wv": wmaj(Wv), "wo": wmaj(Wo),
        "bq_t": col(bq), "bk_t": col(bk), "bop_t": col(bop),
        "w1h": w1h, "w2h": w2h, "lng_t": col(lng), "lnb_t": col(lnb),
    }
    bpc = B // ncores
    in_maps = []
    for c in range(ncores):
        m = dict(shared)
        # x [bpc, L, D] -> [bpc, 128, DC*L]  (d-major per partition)
        xc = x[bpc * c : bpc * (c + 1)]
        xc = np.ascontiguousarray(
            xc.reshape(bpc, L, DC, 128).transpose(0, 3, 2, 1)
        ).reshape(bpc, 128, DC * L)
        m["x_dm"] = xc
        in_maps.append(m)
    return in_maps


def unstage_output(res, ncores=NCORES):
    out = np.empty((B, L, D), np.float32)
    bpc = B // ncores
    for c in range(ncores):
        o = np.asarray(res.results[c]["out_dm"])  # [bpc, 128, DC, L]
        for i in range(bpc):
            # full[t, dci*128+p] = o[i][p, dci, t]
            out[bpc * c + i] = o[i].transpose(2, 1, 0).reshape(L, D)
    return out


def kernel(**inputs):
    nc = _get_nc(NCORES)
    in_maps = stage_inputs(inputs)
    res = bass_utils.run_bass_kernel_spmd(nc, in_maps, core_ids=list(range(NCORES)))
    return unstage_output(res)



# revision 19
# speedup vs baseline: 1.0545x; 1.0545x over previous
"""Trainium2 Bass kernel for nn_Encoder_78889959293176 (Autoformer-style encoder layer).

Strategy: data-parallel over batch (16 batches -> 8 cores x 2).
All heavy compute on the TensorEngine in a d-major ([channel, time]) layout:
  - QKV projections as W-stationary matmuls
  - autocorrelation statistic mean_value via Q K^T tiles + a 2-copy diagonal
    "shear" DMA into DRAM + ones-matmul partition reduction (flipped-tau space)
  - AllReduce(8 cores) of the batch-summed statistic, on-device top-22 mask
    (iterated max8 + match_replace) and masked softmax -> sparse weight vector g
  - the rolls-weighted aggregation as a circulant matmul against a Toeplitz
    band buffer built from g with a single broadcast DMA (no data-dependent
    indexing anywhere)
  - series decomposition via tensor_tensor_scan cumsum, convs as bf16 matmuls,
    layernorm stats via ones-matmuls.

v3 pipeline layout (single pool scopes, no per-batch barriers):
  - the statistic AllReduce is split per batch: batch 0's collective is issued
    right after its phase 1 and hides under batch 1's phase-1 compute; only
    batch 1's collective sits near the critical path, shadowed by the V
    projections and topk.
  - phase-2 topk/softmax operates on [2, L] tiles (a partition per batch).
  - phases 3..7 interleave the two batches so the vector-engine chains
    (decompositions, layernorm) of one batch always overlap TensorEngine conv
    work of the other batch; layernorm stats borrow conv2's PSUM banks.
  - conv2 runs in two channel-half passes (partials parked in bf16 acc2) so h1
    only holds 8 of 16 hidden chunks.
  - all DRAM operands are host-staged partition-major: every DMA line is
    contiguous per partition.
"""

import numpy as np

import concourse.bass as bass
import concourse.bacc as bacc
import concourse.mybir as mybir
import concourse.tile as tile
from concourse import bass_utils
from concourse import library_config
from concourse.alu_op_type import AluOpType

try:
    import ml_dtypes

    BF16_NP = ml_dtypes.bfloat16
except Exception:  # pragma: no cover
    BF16_NP = np.float32

F32 = mybir.dt.float32
BF16 = mybir.dt.bfloat16
AF = mybir.ActivationFunctionType

B, L, D = 16, 2048, 512
CF = 2048  # conv hidden
TOPK = 22
KER = 25
EPS = 1e-5
SLOPE = 0.01
NCORES = 8
BPC = B // NCORES  # batches per core
DC = D // 128  # 4 d-chunks
CFC = CF // 128  # 16 conv-hidden chunks
CH = CFC // 2  # conv-hidden chunks per half
TW = L // 512  # 4 time windows of 512
TM = L // 128  # 16 time chunks of 128
NEG = -1.0e30


def build(nc: bass.Bass, n_group: int):
    x_dm = nc.dram_tensor("x_dm", [BPC, 128, DC * L], F32, kind="ExternalInput")
    wm_d = nc.dram_tensor("wm", [128, DC * D], BF16, kind="ExternalInput")
    wv_d = nc.dram_tensor("wv", [128, DC * D], BF16, kind="ExternalInput")
    wo_d = nc.dram_tensor("wo", [128, DC * D], BF16, kind="ExternalInput")
    bop_d = nc.dram_tensor("bop_t", [128, DC], F32, kind="ExternalInput")
    w1_d = nc.dram_tensor("w1h", [CFC, 128, DC * 3 * 128], BF16, kind="ExternalInput")
    w2_d = nc.dram_tensor("w2h", [DC, 128, CFC * 3 * 128], BF16, kind="ExternalInput")
    lng_d = nc.dram_tensor("lng_t", [128, DC], F32, kind="ExternalInput")
    lnb_d = nc.dram_tensor("lnb_t", [128, DC], F32, kind="ExternalInput")
    out_dm = nc.dram_tensor("out_dm", [BPC, 128, DC, L], F32, kind="ExternalOutput")

    with tile.TileContext(nc) as tc:
        _body(nc, tc, n_group, x_dm, wm_d, wv_d, wo_d, bop_d,
              w1_d, w2_d, lng_d, lnb_d, out_dm)
    return nc


def _decompose_chunk(nc, scan_pool, src, dst, dci):
    """dst[:, dci, 1:L+1] = src[:, dci] - movavg_KER(src[:, dci])."""
    half = (KER - 1) // 2
    pad = scan_pool.tile([128, L + KER], F32, tag="scan_pad")
    cs = scan_pool.tile([128, L + KER], F32, tag="scan_cs")
    nc.vector.memset(pad[:, 0:1], 0.0)
    nc.vector.tensor_copy(
        out=pad[:, 1 : 1 + half],
        in_=src[:, dci, 0:1].to_broadcast([128, half]),
    )
    nc.scalar.activation(pad[:, 1 + half : 1 + half + L], src[:, dci, :], AF.Copy)
    nc.vector.tensor_copy(
        out=pad[:, 1 + half + L :],
        in_=src[:, dci, L - 1 : L].to_broadcast([128, half]),
    )
    nc.vector.tensor_tensor_scan(
        out=cs[:], data0=pad[:], data1=pad[:], initial=0.0,
        op0=AluOpType.add, op1=AluOpType.bypass,
    )
    d1 = pad[:, 0:L]  # cumsum done; reuse pad for the boxcar difference
    nc.vector.tensor_sub(out=d1, in0=cs[:, KER:], in1=cs[:, 0:L])
    nc.vector.scalar_tensor_tensor(
        out=dst[:, dci, 1 : L + 1], in0=d1, scalar=-1.0 / KER,
        in1=src[:, dci, :], op0=AluOpType.mult, op1=AluOpType.add,
    )
    nc.vector.tensor_copy(out=dst[:, dci, 0:1], in_=dst[:, dci, 1:2])
    nc.vector.tensor_copy(
        out=dst[:, dci, L + 1 : L + 2], in_=dst[:, dci, L : L + 1]
    )


def _decompose(nc, scan_pool, src, dst):
    for dci in range(DC):
        _decompose_chunk(nc, scan_pool, src, dst, dci)


def _body(nc, tc, n_group, x_dm, wm_d, wv_d, wo_d, bop_d,
          w1_d, w2_d, lng_d, lnb_d, out_dm):
    with (
        tc.tile_pool(name="p0", bufs=1) as p0,
        tc.tile_pool(name="pp", bufs=1, space="PSUM") as pp,
        tc.tile_pool(name="dr", bufs=1, space="DRAM") as dr,
        tc.tile_pool(name="dr3", bufs=4, space="DRAM") as dr3,
    ):
        nc.gpsimd.load_library(library_config.attn)
        # ----- persistent constants -----
        ones_mv = p0.tile([128, 1], F32, tag="ones_mv")
        nc.vector.memset(ones_mv[:], 1.0 / D)
        ones_bf = p0.tile([128, 1], BF16, tag="ones_bf")
        nc.vector.memset(ones_bf[:], 1.0 / D)
        ones1 = p0.tile([128, 1], BF16, tag="ones1")
        nc.vector.memset(ones1[:], 1.0)
        bop_c = p0.tile([128, DC], F32, tag="bop_c")
        lng_c = p0.tile([128, DC], F32, tag="lng_c")
        lnb_c = p0.tile([128, DC], F32, tag="lnb_c")
        nc.sync.dma_start(bop_c[:], bop_d[:, :])
        nc.sync.dma_start(lng_c[:], lng_d[:, :])
        nc.sync.dma_start(lnb_c[:], lnb_d[:, :])

        # 4 rotating PSUM accumulators shared by all phases
        ps4 = []
        for i in range(4):
            t = pp.tile([128, 512], F32, tag=f"ps{i}", name=f"ps_{i}")
            ps4.append(t)

        hb = {}
        seab = []
        cco = []

        with tc.tile_pool(name="psea", bufs=1) as psea:
            for b in range(BPC):
                t = psea.tile([128, DC, L + 2], BF16, tag=f"seab{b}",
                              name=f"seab_{b}")
                seab.append(t)

            with tc.tile_pool(name="pv", bufs=1) as pv:
                v_s = []
                for b in range(BPC):
                    t = pv.tile([128, TM, D], BF16, tag=f"v_s{b}", name=f"vs_{b}")
                    v_s.append(t)
                # unnormalized masked-exp weights, [128,16] per batch
                # (partition-major time order); softmax 1/Z folded into the
                # agg PSUM drain via zib.
                g8 = pv.tile([128, 2 * 16], BF16, tag="g8")
                zib = pv.tile([128, 2], F32, tag="zib")
                mvloc = pv.tile([128, 2 * 16], F32, tag="mvloc")

                with tc.tile_pool(name="pxv", bufs=1) as pxv:
                    xb = []
                    for b in range(BPC):
                        t = pxv.tile([128, DC, L], BF16, tag=f"xbt{b}",
                                     name=f"xb_{b}")
                        xb.append(t)
                    mvf = pxv.tile([1, BPC * L], F32, tag="mvf")

                    # ========= phase 1: mean_value (flipped space) =========
                    with (
                        tc.tile_pool(name="ph1", bufs=1) as ph1,
                        tc.tile_pool(name="ph1b", bufs=4) as ph1b,
                        tc.tile_pool(name="ph1w", bufs=3) as ph1w,
                        tc.tile_pool(name="ppm1", bufs=1, space="PSUM") as ppm1,
                    ):
                        wm_s = ph1.tile([128, DC, D], BF16, tag="wqk")
                        nc.sync.dma_start(wm_s[:], wm_d.ap())
                        # fine-grained x loads + bf16 converts, both batches
                        for b in range(BPC):
                            for dci in range(DC):
                                for tw in range(TW):
                                    xq = ph1b.tile([128, 512], F32, tag="xq")
                                    o = dci * L + 512 * tw
                                    nc.sync.dma_start(
                                        xq[:], x_dm.ap()[b, :, o : o + 512]
                                    )
                                    nc.scalar.activation(
                                        xb[b][:, dci, 512 * tw : 512 * tw + 512],
                                        xq[:], AF.Copy,
                                    )

                        mv_reg = []
                        for i in range(4):
                            t = ppm1.tile([1, 512], F32, tag=f"mv{i}",
                                          name=f"mv_{i}")
                            mv_reg.append(t)

                        for b in range(BPC):
                            # q' = (Wq Wk^T)^T x (biases provably drop out of
                            # the statistic: rank-1 terms are constant over
                            # tau, and topk+softmax are shift-invariant)
                            q_s = ph1.tile([128, DC, L], BF16, tag="q_s")
                            for dco in range(DC):
                                for dci in range(DC):
                                    for twi in range(TW):
                                        nc.tensor.matmul(
                                            ps4[twi][:],
                                            lhsT=wm_s[:, dci,
                                                      128 * dco : 128 * dco + 128],
                                            rhs=xb[b][:, dci,
                                                      512 * twi : 512 * twi + 512],
                                            start=(dci == 0),
                                            stop=(dci == DC - 1),
                                        )
                                for twi in range(TW):
                                    nc.scalar.activation(
                                        q_s[:, dco, 512 * twi : 512 * twi + 512],
                                        ps4[twi][:], AF.Copy,
                                    )

                            def _emit_mv(A, wa):
                                for cc in range(4):
                                    w0 = (512 * cc + 128 * A) % L
                                    nc.tensor.matmul(
                                        mv_reg[cc][0:1, :],
                                        lhsT=ones_bf[:],
                                        rhs=wa[:, w0 : w0 + 512],
                                        start=(A == 0), stop=(A == TM - 1),
                                    )

                            pend = []
                            for A in range(TM):
                                bufA = dr3.tile([128, 4224], BF16, tag="bufA")
                                for dci in range(DC):
                                    for tB in range(TW):
                                        nc.tensor.matmul(
                                            ps4[tB][:],
                                            lhsT=q_s[:, dci,
                                                     128 * A : 128 * A + 128],
                                            rhs=xb[b][:, dci,
                                                     512 * tB : 512 * tB + 512],
                                            start=(dci == 0), stop=(dci == DC - 1),
                                        )
                                for tB in range(TW):
                                    c_sb = ph1b.tile([128, 512], BF16, tag="c_sb")
                                    nc.scalar.activation(c_sb[:], ps4[tB][:],
                                                         AF.Copy)
                                    for cp, eng in ((0, nc.sync), (1, nc.scalar)):
                                        dst = bass.AP(
                                            bufA[:].tensor,
                                            127 + 512 * tB + 2048 * cp,
                                            [[4223, 128], [1, 512]],
                                        )
                                        eng.dma_start(dst, c_sb[:])
                                wa = ph1w.tile([128, 2560], BF16, tag="wa")
                                nc.sync.dma_start(
                                    wa[:],
                                    bass.AP(bufA[:].tensor, 128,
                                            [[4224, 128], [1, 2560]]),
                                )
                                pend.append((A, wa))
                                if len(pend) > 2:
                                    _emit_mv(*pend.pop(0))
                            for a_w in pend:
                                _emit_mv(*a_w)
                            for cc in range(4):
                                nc.scalar.activation(
                                    mvf[0:1,
                                        L * b + 512 * cc : L * b + 512 * cc + 512],
                                    mv_reg[cc][0:1, :], AF.Copy,
                                )
                            # issue this batch's AllReduce immediately: batch
                            # 0's collective hides under batch 1's phase 1.
                            cci = dr.tile([1, L], F32, tag=f"cci{b}")
                            cc_o = dr.tile([1, L], F32, tag=f"cco{b}")
                            nc.sync.dma_start(cci[:], mvf[0:1, L * b : L * b + L])
                            nc.gpsimd.collective_compute(
                                "AllReduce", AluOpType.add,
                                replica_groups=[list(range(n_group))],
                                ins=[cci[:].opt()], outs=[cc_o[:].opt()],
                            )
                            cco.append(cc_o)
                            # local statistic back in [128,16] layout for the
                            # masked-exp path (off critical path)
                            nc.gpsimd.dma_start(
                                mvloc[:, 16 * b : 16 * b + 16],
                                bass.AP(cci[:].tensor, cci[:].offset,
                                        [[16, 128], [1, 16]]),
                            )

                    # ========= phase 2: topk + softmax (both batches) ========
                    with (
                        tc.tile_pool(name="ph2", bufs=1) as ph2,
                        tc.tile_pool(name="ph2w", bufs=1) as ph2w,
                    ):
                        wv_s = ph2w.tile([128, DC, D], BF16, tag="wv_s")
                        nc.sync.dma_start(wv_s[:], wv_d.ap())

                        def _vproj(b):
                            for tm in range(TM):
                                pt = ps4[tm % 4]
                                for dci in range(DC):
                                    nc.tensor.matmul(
                                        pt[:],
                                        lhsT=xb[b][:, dci,
                                                   128 * tm : 128 * tm + 128],
                                        rhs=wv_s[:, dci, :],
                                        start=(dci == 0), stop=(dci == DC - 1),
                                    )
                                nc.scalar.activation(v_s[b][:, tm, :], pt[:],
                                                     AF.Copy)

                        _vproj(0)  # hides the second collective
                        # top-22 threshold via gpsimd kth_largest: the lerped
                        # quantile at rank 21.5 lies strictly between the 22nd
                        # and 23rd largest values -> tie-free >= mask.
                        bsA = ph2.tile([128, 16], F32, tag="bsA")
                        bsB = ph2.tile([128, 16], F32, tag="bsB")
                        nc.sync.dma_start(
                            bsA[:], bass.AP(cco[0][:].tensor, 0, [[16, 128], [1, 16]])
                        )
                        nc.sync.dma_start(
                            bsB[:], bass.AP(cco[1][:].tensor, 0, [[16, 128], [1, 16]])
                        )
                        bs128 = ph2.tile([128, 16], F32, tag="bs128")
                        nc.vector.tensor_add(out=bs128[:], in0=bsA[:], in1=bsB[:])
                        kth = ph2.tile([1, 2], F32, tag="kth")
                        nc.gpsimd.kth_largest(
                            kth[:], bs128[:], 16, 24,
                            quantile=1.0 - (TOPK - 0.5) / (L - 1),
                        )
                        _vproj(1)  # hides kth_largest
                        # masked exp (unnormalized) on [128,16] per batch;
                        # normalization deferred to the agg PSUM drain.
                        thrb = ph2.tile([128, 1], F32, tag="thrb")
                        nc.gpsimd.partition_broadcast(thrb[:], kth[0:1, 0:1])
                        mask = ph2.tile([128, 16], F32, tag="mask")
                        nc.vector.tensor_scalar(
                            out=mask[:], in0=bs128[:], scalar1=thrb[:, 0:1],
                            scalar2=None, op0=AluOpType.is_ge,
                        )
                        # stt = stat*mask + (mask-1)*1e9: selected entries keep
                        # the exact statistic (no 1e9 roundtrip -- f32 at 1e9
                        # has quantum 64, which would wipe out the values).
                        neg9 = ph2.tile([128, 16], F32, tag="neg9")
                        nc.vector.tensor_scalar(
                            out=neg9[:], in0=mask[:], scalar1=1.0, scalar2=1.0e9,
                            op0=AluOpType.subtract, op1=AluOpType.mult,
                        )
                        stt = ph2.tile([128, 2 * 16], F32, tag="stt")
                        for b in range(BPC):
                            sl = stt[:, 16 * b : 16 * b + 16]
                            nc.vector.tensor_mul(
                                out=sl, in0=mvloc[:, 16 * b : 16 * b + 16],
                                in1=mask[:],
                            )
                            nc.vector.tensor_add(out=sl, in0=sl, in1=neg9[:])
                        nc.scalar.activation(g8[:], stt[:], AF.Exp)
                        # periodic replication B[q] = g_f[q mod L] built by a
                        # [128,16]->[1,2048] gather write + one step-0-source
                        # DRAM->DRAM replication blast per batch (the
                        # row-step-2047 gbuf read needs 129 copies).
                        for b, eng in ((1, nc.sync), (0, nc.scalar)):
                            hb0 = dr.tile([1, L], BF16, tag=f"hb0{b}")
                            eng.dma_start(
                                bass.AP(hb0[:].tensor, hb0[:].offset,
                                        [[16, 128], [1, 16]]),
                                g8[:, 16 * b : 16 * b + 16],
                            )
                            hbb = dr.tile([1, 129 * L], BF16, tag=f"hb{b}")
                            hb[b] = hbb
                            for (r0, r1), e2 in zip(
                                ((0, 65), (65, 129)), (nc.sync, nc.scalar)
                            ):
                                e2.dma_start(
                                    bass.AP(hbb[:].tensor,
                                            hbb[:].offset + L * r0,
                                            [[L, r1 - r0], [1, L]]),
                                    bass.AP(hb0[:].tensor, hb0[:].offset,
                                            [[0, r1 - r0], [1, L]]),
                                )
                        # softmax normalizers 1/Z per batch (off critical path)
                        nc.tensor.matmul(ps4[0][0:1, 0:32], lhsT=ones1[:],
                                         rhs=g8[:], start=True, stop=True)
                        zrow = ph2.tile([1, 32], F32, tag="zrow")
                        nc.scalar.activation(zrow[:], ps4[0][0:1, 0:32], AF.Copy)
                        z2 = ph2.tile([1, 2], F32, tag="z2")
                        ztmp = ph2.tile([1, 32], F32, tag="ztmp")
                        for b in range(BPC):
                            nc.scalar.activation(
                                ztmp[0:1, 16 * b : 16 * b + 16],
                                zrow[0:1, 16 * b : 16 * b + 16], AF.Copy,
                                accum_out=z2[0:1, b : b + 1],
                            )
                        nc.vector.reciprocal(out=z2[:], in_=z2[:])
                        nc.gpsimd.partition_broadcast(zib[:], z2[0:1, :])

                # == phases 3-4, batch order b1 then b0 (decomp overlaps PE) ==
                with (
                    tc.tile_pool(name="p3", bufs=1) as p3,
                    tc.tile_pool(name="p3r", bufs=2) as p3r,
                    tc.tile_pool(name="p3s", bufs=1) as p3s,
                ):
                    wo_s = p3.tile([128, DC, D], BF16, tag="wo_s")
                    nc.sync.dma_start(wo_s[:], wo_d.ap())
                    gbufs = {}
                    gsplit = ((0, 43), (43, 86), (86, 128))
                    for b in (1, 0):
                        gbuf = p3.tile([128, 3968], BF16, tag=f"gbuf{b}")
                        gbufs[b] = gbuf
                        for (i0, i1), eng in zip(
                            gsplit, (nc.sync, nc.scalar, nc.gpsimd)
                        ):
                            eng.dma_start(
                                gbuf[i0:i1, :],
                                bass.AP(hb[b][:].tensor,
                                        hb[b][:].offset + 127 + 2047 * i0,
                                        [[2047, i1 - i0], [1, 3968]]),
                            )
                    for b in (1, 0):
                        gbuf = gbufs[b]
                        agg = p3.tile([128, DC, L], BF16, tag=f"agg{b}")
                        for dm in range(DC):
                            for Bc in range(TM):
                                for nw in range(TW):
                                    gp = 512 * nw - 128 * Bc + 1920
                                    nc.tensor.matmul(
                                        ps4[nw][:],
                                        lhsT=v_s[b][:, Bc,
                                                    128 * dm : 128 * dm + 128],
                                        rhs=gbuf[:, gp : gp + 512],
                                        start=(Bc == 0), stop=(Bc == TM - 1),
                                    )
                            for nw in range(TW):
                                nc.scalar.activation(
                                    agg[:, dm, 512 * nw : 512 * nw + 512],
                                    ps4[nw][:], AF.Copy, scale=zib[:, b : b + 1],
                                )
                        acx = p3.tile([128, DC, L], F32, tag="acx")
                        for dco in range(DC):
                            for dci in range(DC):
                                for twi in range(TW):
                                    nc.tensor.matmul(
                                        ps4[twi][:],
                                        lhsT=wo_s[:, dci,
                                                  128 * dco : 128 * dco + 128],
                                        rhs=agg[:, dci,
                                                512 * twi : 512 * twi + 512],
                                        start=(dci == 0), stop=(dci == DC - 1),
                                    )
                            for twi in range(TW):
                                xr = p3r.tile([128, 512], F32, tag="xr")
                                nc.sync.dma_start(
                                    xr[:],
                                    x_dm.ap()[b, :,
                                              dco * L + 512 * twi :
                                              dco * L + 512 * twi + 512],
                                )
                                nc.vector.scalar_tensor_tensor(
                                    out=acx[:, dco, 512 * twi : 512 * twi + 512],
                                    in0=ps4[twi][:],
                                    scalar=bop_c[:, dco : dco + 1],
                                    in1=xr[:],
                                    op0=AluOpType.add, op1=AluOpType.add,
                                )
                            # this channel chunk of acx is complete: its
                            # decomposition scan can overlap the next chunks'
                            # matmuls (and the following conv stages)
                            _decompose_chunk(nc, p3s, acx, seab[b], dco)

            # ======= phases 5-7, interleaved across the two batches =======
            with (
                tc.tile_pool(name="pcv", bufs=1) as pcv,
                tc.tile_pool(name="p5w1", bufs=2) as p5w1,
                tc.tile_pool(name="p5w2", bufs=2) as p5w2,
                tc.tile_pool(name="p6s", bufs=2) as p6s,
                tc.tile_pool(name="p7s", bufs=1) as p7s,
                tc.tile_pool(name="p7b", bufs=1) as p7b,
                tc.tile_pool(name="p7d", bufs=2, space="DRAM") as p7d,
                tc.tile_pool(name="ppc2", bufs=1, space="PSUM") as ppc2,
            ):
                pc2 = []
                for i in range(4):
                    t = ppc2.tile([128, 512], F32, tag=f"pc2_{i}", name=f"pc2_{i}")
                    pc2.append(t)
                h1 = pcv.tile([128, CH, L + 2], BF16, tag="h1")
                acc2 = pcv.tile([128, DC, L], BF16, tag="acc2")
                ysb = pcv.tile([128, DC, L], F32, tag="ysb")
                sea2 = pcv.tile([128, DC, L + 2], F32, tag="sea2")

                def conv1_half(b, half):
                    for co8 in range(CH):
                        co = CH * half + co8
                        w1t = p5w1.tile([128, DC * 3 * 128], BF16, tag="w1t")
                        nc.sync.dma_start(w1t[:], w1_d.ap()[co])
                        first = True
                        for dci in range(DC):
                            for tap in range(3):
                                ki = (3 * dci + tap) * 128
                                for nw in range(TW):
                                    nc.tensor.matmul(
                                        ps4[nw][:],
                                        lhsT=w1t[:, ki : ki + 128],
                                        rhs=seab[b][:, dci,
                                             512 * nw + tap : 512 * nw + tap + 512],
                                        start=first,
                                        stop=(dci == DC - 1 and tap == 2),
                                    )
                                first = False
                        for nw in range(TW):
                            nc.scalar.activation(
                                h1[:, co8, 1 + 512 * nw : 513 + 512 * nw],
                                ps4[nw][:], AF.Prelu, alpha=SLOPE,
                            )
                        nc.vector.tensor_copy(out=h1[:, co8, 0:1],
                                              in_=h1[:, co8, 1:2])
                        nc.vector.tensor_copy(out=h1[:, co8, L + 1 : L + 2],
                                              in_=h1[:, co8, L : L + 1])

                def conv2_pass(b, half):
                    for co in range(DC):
                        w2t = p5w2.tile([128, CH * 3 * 128], BF16, tag="w2t")
                        src = w2_d.ap()[co]
                        ofs = half * CH * 3 * 128
                        nc.sync.dma_start(
                            w2t[:],
                            bass.AP(src.tensor, src.offset + ofs,
                                    [[CFC * 3 * 128, 128], [1, CH * 3 * 128]]),
                        )
                        first = True
                        for ci8 in range(CH):
                            for tap in range(3):
                                ki = (3 * ci8 + tap) * 128
                                for nw in range(TW):
                                    nc.tensor.matmul(
                                        pc2[nw][:],
                                        lhsT=w2t[:, ki : ki + 128],
                                        rhs=h1[:, ci8,
                                             512 * nw + tap : 512 * nw + tap + 512],
                                        start=first,
                                        stop=(ci8 == CH - 1 and tap == 2),
                                    )
                                first = False
                        if half == 0:
                            for nw in range(TW):
                                nc.scalar.activation(
                                    acc2[:, co, 512 * nw : 512 * nw + 512],
                                    pc2[nw][:], AF.Copy,
                                )
                        else:
                            for nw in range(TW):
                                h2t = p6s.tile([128, 512], F32, tag="h2t")
                                nc.vector.tensor_add(
                                    out=h2t[:], in0=pc2[nw][:],
                                    in1=acc2[:, co, 512 * nw : 512 * nw + 512],
                                )
                                h2r = p6s.tile([128, 512], F32, tag="h2r")
                                nc.scalar.activation(h2r[:], h2t[:], AF.Prelu,
                                                     alpha=SLOPE)
                                nc.vector.tensor_add(
                                    out=ysb[:, co, 512 * nw : 512 * nw + 512],
                                    in0=h2r[:],
                                    in1=seab[b][:, co,
                                                1 + 512 * nw : 513 + 512 * nw],
                                )
                            # decompose2 of this channel chunk can start now
                            _decompose_chunk(nc, p7s, ysb, sea2, co)

                def layernorm(b):
                    # windowed pipeline; stats borrow conv2's PSUM banks
                    for twi in range(TW):
                        st_s = pc2[2 * (twi % 2)][0:1, :]
                        st_q = pc2[2 * (twi % 2) + 1][0:1, :]
                        for dci in range(DC):
                            sqt = p6s.tile([128, 512], F32, tag="sqt")
                            nc.scalar.activation(
                                sqt[:],
                                sea2[:, dci, 1 + 512 * twi : 513 + 512 * twi],
                                AF.Square,
                            )
                            nc.tensor.matmul(
                                st_s,
                                lhsT=ones_mv[:],
                                rhs=sea2[:, dci, 1 + 512 * twi : 513 + 512 * twi],
                                start=(dci == 0), stop=(dci == DC - 1),
                            )
                            nc.tensor.matmul(
                                st_q,
                                lhsT=ones_mv[:],
                                rhs=sqt[:],
                                start=(dci == 0), stop=(dci == DC - 1),
                            )
                        mu = p7b.tile([1, 512], F32, tag="mu")
                        rs = p7b.tile([1, 512], F32, tag="rs")
                        nc.scalar.activation(mu[:], st_s, AF.Copy)
                        nc.vector.tensor_mul(out=rs[:], in0=mu[:], in1=mu[:])
                        nc.vector.tensor_sub(out=rs[:], in0=st_q, in1=rs[:])
                        nc.vector.tensor_scalar_add(rs[:], rs[:], EPS)
                        nc.vector.reciprocal(out=rs[:], in_=rs[:])
                        nc.scalar.activation(rs[:], rs[:], AF.Sqrt)
                        mub = p7b.tile([128, 512], F32, tag="mub")
                        rsb = p7b.tile([128, 512], F32, tag="rsb")
                        nc.gpsimd.partition_broadcast(mub[:], mu[:])
                        nc.gpsimd.partition_broadcast(rsb[:], rs[:])
                        for dci in range(DC):
                            ve = nc.vector
                            og = p6s.tile([128, 512], F32, tag="og")
                            ve.tensor_sub(
                                out=og[:],
                                in0=sea2[:, dci, 1 + 512 * twi : 513 + 512 * twi],
                                in1=mub[:],
                            )
                            ve.scalar_tensor_tensor(
                                out=og[:], in0=og[:],
                                scalar=lng_c[:, dci : dci + 1], in1=rsb[:],
                                op0=AluOpType.mult, op1=AluOpType.mult,
                            )
                            nc.scalar.activation(
                                og[:], og[:], AF.Identity,
                                bias=lnb_c[:, dci : dci + 1],
                            )
                            nc.scalar.dma_start(
                                out_dm.ap()[b, :, dci,
                                            512 * twi : 512 * twi + 512],
                                og[:],
                            )

                # schedule: conv(b1) fully; LN(b1) sits between batch-0 conv
                # stages so its vector/DMA chains hide under PE work.
                conv1_half(1, 0)
                conv2_pass(1, 0)
                conv1_half(1, 1)
                conv2_pass(1, 1)      # finalize emits decompose2(b1) per chunk
                conv1_half(0, 0)
                layernorm(1)          # stats borrow pc2 (free here)
                conv2_pass(0, 0)
                conv1_half(0, 1)
                conv2_pass(0, 1)      # finalize emits decompose2(b0) per chunk
                layernorm(0)


# ---------------------------------------------------------------------------
# host side
# ---------------------------------------------------------------------------
_CACHE = {}


def _get_nc(n_group: int):
    key = n_group
    if key not in _CACHE:
        nc = bacc.Bacc("TRN2", target_bir_lowering=False, debug=False,
                       num_devices=n_group)
        build(nc, n_group)
        nc.compile()
        _CACHE[key] = nc
    return _CACHE[key]


def stage_inputs(inputs, ncores=NCORES):
    x = np.asarray(inputs["x"], np.float32)
    Wq = np.asarray(inputs["Wq"], np.float32)
    Wk = np.asarray(inputs["Wk"], np.float32)
    Wv = np.asarray(inputs["Wv"], np.float32)
    Wo = np.asarray(inputs["Wo"], np.float32)
    bq = np.asarray(inputs["bq"], np.float32)
    bk = np.asarray(inputs["bk"], np.float32)
    bv = np.asarray(inputs["bv"], np.float32)
    bo = np.asarray(inputs["bo"], np.float32)
    w1 = np.asarray(inputs["conv1_w"], np.float32)
    w2 = np.asarray(inputs["conv2_w"], np.float32)
    lng = np.asarray(inputs["ln_g"], np.float32)
    lnb = np.asarray(inputs["ln_b"], np.float32)

    bop = bo + bv @ Wo
    # merged QK projection: the statistic only needs x^T (Wq Wk^T) x
    # (biases drop: they shift the statistic by a constant, and both topk
    # and softmax are shift-invariant)
    Wm = Wq @ Wk.T
    col = lambda v: np.ascontiguousarray(v.reshape(DC, 128).T)
    # projection weights partition-major: W[dci*128+p, n] -> [p, dci*D + n]
    wmaj = lambda W: np.ascontiguousarray(
        W.reshape(DC, 128, D).transpose(1, 0, 2).reshape(128, DC * D)
    ).astype(BF16_NP)
    # conv1 [3, D, CF] -> [CFC, 128(ci-part), DC*3*128(co)]
    w1h = np.ascontiguousarray(
        w1.reshape(3, DC, 128, CFC, 128).transpose(3, 2, 1, 0, 4)
    ).reshape(CFC, 128, DC * 3 * 128).astype(BF16_NP)
    # conv2 [3, CF, D] -> [DC, 128(ci-part), CFC*3*128(co)]
    w2h = np.ascontiguousarray(
        w2.reshape(3, CFC, 128, DC, 128).transpose(3, 2, 1, 0, 4)
    ).reshape(DC, 128, CFC * 3 * 128).astype(BF16_NP)

    shared = {
        "wm": wmaj(Wm), "wv": wmaj(Wv), "wo": wmaj(Wo),
        "bop_t": col(bop),
        "w1h": w1h, "w2h": w2h, "lng_t": col(lng), "lnb_t": col(lnb),
    }
    bpc = B // ncores
    in_maps = []
    for c in range(ncores):
        m = dict(shared)
        # x [bpc, L, D] -> [bpc, 128, DC*L]  (d-major per partition)
        xc = x[bpc * c : bpc * (c + 1)]
        xc = np.ascontiguousarray(
            xc.reshape(bpc, L, DC, 128).transpose(0, 3, 2, 1)
        ).reshape(bpc, 128, DC * L)
        m["x_dm"] = xc
        in_maps.append(m)
    return in_maps


def unstage_output(res, ncores=NCORES):
    out = np.empty((B, L, D), np.float32)
    bpc = B // ncores
    for c in range(ncores):
        o = np.asarray(res.results[c]["out_dm"])  # [bpc, 128, DC, L]
        for i in range(bpc):
            # full[t, dci*128+p] = o[i][p, dci, t]
            out[bpc * c + i] = o[i].transpose(2, 1, 0).reshape(L, D)
    return out


def kernel(**inputs):
    nc = _get_nc(NCORES)
    in_maps = stage_inputs(inputs)
    res = bass_utils.run_bass_kernel_spmd(nc, in_maps, core_ids=list(range(NCORES)))
    return unstage_output(res)



# revision 46
# speedup vs baseline: 1.0983x; 1.0415x over previous
"""Trainium2 Bass kernel for nn_Encoder_78889959293176 (Autoformer-style encoder layer).

Strategy: data-parallel over batch (16 batches -> 8 cores x 2).
All heavy compute on the TensorEngine in a d-major ([channel, time]) layout:
  - QKV projections as W-stationary matmuls
  - autocorrelation statistic mean_value via Q K^T tiles + a 2-copy diagonal
    "shear" DMA into DRAM + ones-matmul partition reduction (flipped-tau space)
  - AllReduce(8 cores) of the batch-summed statistic, on-device top-22 mask
    (iterated max8 + match_replace) and masked softmax -> sparse weight vector g
  - the rolls-weighted aggregation as a circulant matmul against a Toeplitz
    band buffer built from g with a single broadcast DMA (no data-dependent
    indexing anywhere)
  - series decomposition via tensor_tensor_scan cumsum, convs as bf16 matmuls,
    layernorm stats via ones-matmuls.

v3 pipeline layout (single pool scopes, no per-batch barriers):
  - the statistic AllReduce is split per batch: batch 0's collective is issued
    right after its phase 1 and hides under batch 1's phase-1 compute; only
    batch 1's collective sits near the critical path, shadowed by the V
    projections and topk.
  - phase-2 topk/softmax operates on [2, L] tiles (a partition per batch).
  - phases 3..7 interleave the two batches so the vector-engine chains
    (decompositions, layernorm) of one batch always overlap TensorEngine conv
    work of the other batch; layernorm stats borrow conv2's PSUM banks.
  - conv2 runs in two channel-half passes (partials parked in bf16 acc2) so h1
    only holds 8 of 16 hidden chunks.
  - all DRAM operands are host-staged partition-major: every DMA line is
    contiguous per partition.
"""

import numpy as np

import concourse.bass as bass
import concourse.bacc as bacc
import concourse.mybir as mybir
import concourse.tile as tile
from concourse import bass_utils
from concourse import library_config
from concourse.alu_op_type import AluOpType

try:
    import ml_dtypes

    BF16_NP = ml_dtypes.bfloat16
    FP8_NP = ml_dtypes.float8_e4m3
except Exception:  # pragma: no cover
    BF16_NP = np.float32
    FP8_NP = np.float32

F32 = mybir.dt.float32
BF16 = mybir.dt.bfloat16
FP8 = mybir.dt.float8e4
DR = mybir.MatmulPerfMode.DoubleRow
AF = mybir.ActivationFunctionType

# fp8 e4m3 scale factors (folded back out via matmul-drain scales / the
# mean-value reduction constant)
SX = 4.0    # x
SW = 64.0   # wm = Wq Wk^T and wvo = Wv Wo weight matrices
SQ = 16.0   # q' = x (Wq Wk^T)
SV = 16.0   # v' = x (Wv Wo)

B, L, D = 16, 2048, 512
CF = 2048  # conv hidden
TOPK = 22
KER = 25
EPS = 1e-5
SLOPE = 0.01
NCORES = 8
BPC = B // NCORES  # batches per core
DC = D // 128  # 4 d-chunks
CFC = CF // 128  # 16 conv-hidden chunks
CH = CFC // 2  # conv-hidden chunks per half
TW = L // 512  # 4 time windows of 512
TM = L // 128  # 16 time chunks of 128
NEG = -1.0e30


def build(nc: bass.Bass, n_group: int):
    x_dm = nc.dram_tensor("x_dm", [BPC, 128, DC * L], F32, kind="ExternalInput")
    wm_d = nc.dram_tensor("wm", [128, DC * D], BF16, kind="ExternalInput")
    wv_d = nc.dram_tensor("wv", [128, DC * D], FP8, kind="ExternalInput")
    bop_d = nc.dram_tensor("bop_t", [128, DC], F32, kind="ExternalInput")
    w1_d = nc.dram_tensor("w1h", [CFC, 128, DC * 3 * 128], BF16, kind="ExternalInput")
    w2_d = nc.dram_tensor("w2h", [DC, 128, CFC * 3 * 128], BF16, kind="ExternalInput")
    lng_d = nc.dram_tensor("lng_t", [128, DC], F32, kind="ExternalInput")
    lnb_d = nc.dram_tensor("lnb_t", [128, DC], F32, kind="ExternalInput")
    out_dm = nc.dram_tensor("out_dm", [BPC, 128, DC, L], F32, kind="ExternalOutput")

    with tile.TileContext(nc) as tc:
        _body(nc, tc, n_group, x_dm, wm_d, wv_d, bop_d,
              w1_d, w2_d, lng_d, lnb_d, out_dm)
    return nc


def _decompose_chunk(nc, scan_pool, src, dst, dci):
    """dst[:, dci, 1:L+1] = src[:, dci] - movavg_KER(src[:, dci])."""
    half = (KER - 1) // 2
    pad = scan_pool.tile([128, L + KER], F32, tag="scan_pad")
    cs = scan_pool.tile([128, L + KER], F32, tag="scan_cs")
    nc.vector.memset(pad[:, 0:1], 0.0)
    nc.vector.tensor_copy(
        out=pad[:, 1 : 1 + half],
        in_=src[:, dci, 0:1].to_broadcast([128, half]),
    )
    nc.scalar.activation(pad[:, 1 + half : 1 + half + L], src[:, dci, :], AF.Copy)
    nc.vector.tensor_copy(
        out=pad[:, 1 + half + L :],
        in_=src[:, dci, L - 1 : L].to_broadcast([128, half]),
    )
    nc.vector.tensor_tensor_scan(
        out=cs[:], data0=pad[:], data1=pad[:], initial=0.0,
        op0=AluOpType.add, op1=AluOpType.bypass,
    )
    d1 = pad[:, 0:L]  # cumsum done; reuse pad for the boxcar difference
    nc.vector.tensor_sub(out=d1, in0=cs[:, KER:], in1=cs[:, 0:L])
    nc.vector.scalar_tensor_tensor(
        out=dst[:, dci, 1 : L + 1], in0=d1, scalar=-1.0 / KER,
        in1=src[:, dci, :], op0=AluOpType.mult, op1=AluOpType.add,
    )
    nc.vector.tensor_copy(out=dst[:, dci, 0:1], in_=dst[:, dci, 1:2])
    nc.vector.tensor_copy(
        out=dst[:, dci, L + 1 : L + 2], in_=dst[:, dci, L : L + 1]
    )


def _decompose(nc, scan_pool, src, dst):
    for dci in range(DC):
        _decompose_chunk(nc, scan_pool, src, dst, dci)


def _body(nc, tc, n_group, x_dm, wm_d, wv_d, bop_d,
          w1_d, w2_d, lng_d, lnb_d, out_dm):
    with (
        tc.tile_pool(name="p0", bufs=1) as p0,
        tc.tile_pool(name="pp", bufs=1, space="PSUM") as pp,
        tc.tile_pool(name="dr", bufs=1, space="DRAM") as dr,
        tc.tile_pool(name="dr3", bufs=4, space="DRAM") as dr3,
    ):
        nc.gpsimd.load_library(library_config.attn)
        # ----- persistent constants -----
        ones_mv = p0.tile([128, 1], F32, tag="ones_mv")
        nc.vector.memset(ones_mv[:], 1.0 / D)
        ones_bf = p0.tile([128, 1], BF16, tag="ones_bf")
        nc.vector.memset(ones_bf[:], 1.0 / D)
        ones1 = p0.tile([128, 1], BF16, tag="ones1")
        nc.vector.memset(ones1[:], 1.0)
        bop_c = p0.tile([128, DC], F32, tag="bop_c")
        lng_c = p0.tile([128, DC], F32, tag="lng_c")
        lnb_c = p0.tile([128, DC], F32, tag="lnb_c")
        nc.sync.dma_start(bop_c[:], bop_d[:, :])
        nc.sync.dma_start(lng_c[:], lng_d[:, :])
        nc.sync.dma_start(lnb_c[:], lnb_d[:, :])

        # 4 rotating PSUM accumulators shared by all phases
        ps4 = []
        for i in range(4):
            t = pp.tile([128, 512], F32, tag=f"ps{i}", name=f"ps_{i}")
            ps4.append(t)

        hb = {}
        seab = []
        cco = []

        with tc.tile_pool(name="psea", bufs=1) as psea:
            for b in range(BPC):
                t = psea.tile([128, DC, L + 2], BF16, tag=f"seab{b}",
                              name=f"seab_{b}")
                seab.append(t)

            with tc.tile_pool(name="pv", bufs=1) as pv:
                # v' = x (Wv Wo) in e4m3, time chunks stored REVERSED
                # (chunk j = true time chunk TM-1-j) so the DoubleRow agg
                # pairs read gbuf with a positive +128 k-substride.
                v8 = []
                for b in range(BPC):
                    t = pv.tile([128, TM, D], FP8, tag=f"v8_{b}", name=f"v8_{b}")
                    v8.append(t)
                # unnormalized masked-exp weights, [128,16] per batch
                # (partition-major time order); softmax 1/Z folded into the
                # agg PSUM drain via zib.
                g8 = pv.tile([128, 2 * 16], FP8, tag="g8")
                zib = pv.tile([128, 2], F32, tag="zib")
                mvloc = pv.tile([128, 2 * 16], F32, tag="mvloc")

                with tc.tile_pool(name="pxv", bufs=1) as pxv:
                    # xb bf16 feeds the statistic path (fp8 there perturbs the
                    # top-k selection); x8 e4m3 feeds the V projection.
                    xb = []
                    x8 = []
                    for b in range(BPC):
                        t = pxv.tile([128, DC, L], BF16, tag=f"xbt{b}",
                                     name=f"xb_{b}")
                        xb.append(t)
                        t8 = pxv.tile([128, DC, L], FP8, tag=f"x8t{b}",
                                      name=f"x8_{b}")
                        x8.append(t8)
                    mvf = pxv.tile([1, BPC * L], F32, tag="mvf")

                    # ========= phase 1: mean_value (flipped space) =========
                    with (
                        tc.tile_pool(name="ph1", bufs=1) as ph1,
                        tc.tile_pool(name="ph1b", bufs=4) as ph1b,
                        tc.tile_pool(name="ph1w", bufs=3) as ph1w,
                        tc.tile_pool(name="ppm1", bufs=1, space="PSUM") as ppm1,
                    ):
                        wm_s = ph1.tile([128, DC, D], BF16, tag="wqk")
                        nc.sync.dma_start(wm_s[:], wm_d.ap())
                        # fine-grained x loads; bf16 convert (scalar) + scaled
                        # e4m3 convert (vector), both batches
                        for b in range(BPC):
                            for dci in range(DC):
                                for tw in range(TW):
                                    xq = ph1b.tile([128, 512], F32, tag="xq")
                                    o = dci * L + 512 * tw
                                    nc.sync.dma_start(
                                        xq[:], x_dm.ap()[b, :, o : o + 512]
                                    )
                                    nc.scalar.activation(
                                        xb[b][:, dci, 512 * tw : 512 * tw + 512],
                                        xq[:], AF.Copy,
                                    )
                                    nc.vector.tensor_scalar(
                                        out=x8[b][:, dci,
                                                  512 * tw : 512 * tw + 512],
                                        in0=xq[:], scalar1=SX, scalar2=None,
                                        op0=AluOpType.mult,
                                    )

                        mv_reg = []
                        for i in range(4):
                            t = ppm1.tile([1, 512], F32, tag=f"mv{i}",
                                          name=f"mv_{i}")
                            mv_reg.append(t)

                        for b in range(BPC):
                            # q' = (Wq Wk^T)^T x (biases provably drop out of
                            # the statistic: rank-1 terms are constant over
                            # tau, and topk+softmax are shift-invariant)
                            q_s = ph1.tile([128, DC, L], BF16, tag="q_s")
                            for dco in range(DC):
                                for dci in range(DC):
                                    for twi in range(TW):
                                        nc.tensor.matmul(
                                            ps4[twi][:],
                                            lhsT=wm_s[:, dci,
                                                      128 * dco : 128 * dco + 128],
                                            rhs=xb[b][:, dci,
                                                      512 * twi : 512 * twi + 512],
                                            start=(dci == 0),
                                            stop=(dci == DC - 1),
                                        )
                                for twi in range(TW):
                                    nc.scalar.activation(
                                        q_s[:, dco, 512 * twi : 512 * twi + 512],
                                        ps4[twi][:], AF.Copy,
                                    )

                            def _emit_mv(A, wa):
                                for cc in range(4):
                                    w0 = (512 * cc + 128 * A) % L
                                    nc.tensor.matmul(
                                        mv_reg[cc][0:1, :],
                                        lhsT=ones_bf[:],
                                        rhs=wa[:, w0 : w0 + 512],
                                        start=(A == 0), stop=(A == TM - 1),
                                    )

                            pend = []
                            for A in range(TM):
                                bufA = dr3.tile([128, 4224], BF16, tag="bufA")
                                for dci in range(DC):
                                    for tB in range(TW):
                                        nc.tensor.matmul(
                                            ps4[tB][:],
                                            lhsT=q_s[:, dci,
                                                     128 * A : 128 * A + 128],
                                            rhs=xb[b][:, dci,
                                                     512 * tB : 512 * tB + 512],
                                            start=(dci == 0),
                                            stop=(dci == DC - 1),
                                        )
                                for tB in range(TW):
                                    c_sb = ph1b.tile([128, 512], BF16, tag="c_sb")
                                    nc.scalar.activation(c_sb[:], ps4[tB][:],
                                                         AF.Copy)
                                    for cp, eng in ((0, nc.sync), (1, nc.scalar)):
                                        dst = bass.AP(
                                            bufA[:].tensor,
                                            127 + 512 * tB + 2048 * cp,
                                            [[4223, 128], [1, 512]],
                                        )
                                        eng.dma_start(dst, c_sb[:])
                                wa = ph1w.tile([128, 2560], BF16, tag="wa")
                                nc.sync.dma_start(
                                    wa[:],
                                    bass.AP(bufA[:].tensor, 128,
                                            [[4224, 128], [1, 2560]]),
                                )
                                pend.append((A, wa))
                                if len(pend) > 2:
                                    _emit_mv(*pend.pop(0))
                            for a_w in pend:
                                _emit_mv(*a_w)
                            for cc in range(4):
                                nc.scalar.activation(
                                    mvf[0:1,
                                        L * b + 512 * cc : L * b + 512 * cc + 512],
                                    mv_reg[cc][0:1, :], AF.Copy,
                                )
                            # issue this batch's AllReduce immediately: batch
                            # 0's collective hides under batch 1's phase 1.
                            cci = dr.tile([1, L], F32, tag=f"cci{b}")
                            cc_o = dr.tile([1, L], F32, tag=f"cco{b}")
                            nc.sync.dma_start(cci[:], mvf[0:1, L * b : L * b + L])
                            nc.gpsimd.collective_compute(
                                "AllReduce", AluOpType.add,
                                replica_groups=[list(range(n_group))],
                                ins=[cci[:].opt()], outs=[cc_o[:].opt()],
                            )
                            cco.append(cc_o)
                            # local statistic back in [128,16] layout for the
                            # masked-exp path (off critical path)
                            nc.gpsimd.dma_start(
                                mvloc[:, 16 * b : 16 * b + 16],
                                bass.AP(cci[:].tensor, cci[:].offset,
                                        [[16, 128], [1, 16]]),
                            )

                    # ========= phase 2: topk + softmax (both batches) ========
                    with (
                        tc.tile_pool(name="ph2", bufs=1) as ph2,
                        tc.tile_pool(name="ph2w", bufs=1) as ph2w,
                    ):
                        wv_s = ph2w.tile([128, DC, D], FP8, tag="wv_s")
                        nc.sync.dma_start(wv_s[:], wv_d.ap())

                        def _vproj(b):
                            for tm in range(TM):
                                pt = ps4[tm % 4]
                                for dp in range(DC // 2):
                                    nc.tensor.matmul(
                                        pt[:],
                                        lhsT=x8[b][:, 2 * dp : 2 * dp + 2,
                                                   128 * tm : 128 * tm + 128],
                                        rhs=wv_s[:, 2 * dp : 2 * dp + 2, :],
                                        start=(dp == 0),
                                        stop=(dp == DC // 2 - 1),
                                        perf_mode=DR,
                                    )
                                # reversed chunk order for the DoubleRow agg
                                nc.scalar.activation(
                                    v8[b][:, TM - 1 - tm, :], pt[:], AF.Copy,
                                    scale=SV / (SX * SW),
                                )

                        _vproj(0)  # hides the second collective
                        # top-22 threshold via gpsimd kth_largest: the lerped
                        # quantile at rank 21.5 lies strictly between the 22nd
                        # and 23rd largest values -> tie-free >= mask.
                        bsA = ph2.tile([128, 16], F32, tag="bsA")
                        bsB = ph2.tile([128, 16], F32, tag="bsB")
                        nc.sync.dma_start(
                            bsA[:], bass.AP(cco[0][:].tensor, 0, [[16, 128], [1, 16]])
                        )
                        nc.sync.dma_start(
                            bsB[:], bass.AP(cco[1][:].tensor, 0, [[16, 128], [1, 16]])
                        )
                        bs128 = ph2.tile([128, 16], F32, tag="bs128")
                        nc.vector.tensor_add(out=bs128[:], in0=bsA[:], in1=bsB[:])
                        kth = ph2.tile([1, 2], F32, tag="kth")
                        nc.gpsimd.kth_largest(
                            kth[:], bs128[:], 16, 24,
                            quantile=1.0 - (TOPK - 0.5) / (L - 1),
                        )
                        _vproj(1)  # hides kth_largest
                        # masked exp (unnormalized) on [128,16] per batch;
                        # normalization deferred to the agg PSUM drain.
                        thrb = ph2.tile([128, 1], F32, tag="thrb")
                        nc.gpsimd.partition_broadcast(thrb[:], kth[0:1, 0:1])
                        mask = ph2.tile([128, 16], F32, tag="mask")
                        nc.vector.tensor_scalar(
                            out=mask[:], in0=bs128[:], scalar1=thrb[:, 0:1],
                            scalar2=None, op0=AluOpType.is_ge,
                        )
                        # stt = stat*mask + (mask-1)*1e9: selected entries keep
                        # the exact statistic (no 1e9 roundtrip -- f32 at 1e9
                        # has quantum 64, which would wipe out the values).
                        neg9 = ph2.tile([128, 16], F32, tag="neg9")
                        nc.vector.tensor_scalar(
                            out=neg9[:], in0=mask[:], scalar1=1.0, scalar2=1.0e9,
                            op0=AluOpType.subtract, op1=AluOpType.mult,
                        )
                        stt = ph2.tile([128, 2 * 16], F32, tag="stt")
                        for b in range(BPC):
                            sl = stt[:, 16 * b : 16 * b + 16]
                            nc.vector.tensor_mul(
                                out=sl, in0=mvloc[:, 16 * b : 16 * b + 16],
                                in1=mask[:],
                            )
                            nc.vector.tensor_add(out=sl, in0=sl, in1=neg9[:])
                        nc.scalar.activation(g8[:], stt[:], AF.Exp)
                        # periodic replication B[q] = g_f[q mod L] built by a
                        # [128,16]->[1,2048] gather write + one step-0-source
                        # DRAM->DRAM replication blast per batch (the
                        # row-step-2047 gbuf read needs 129 copies).
                        for b, eng in ((1, nc.sync), (0, nc.scalar)):
                            hb0 = dr.tile([1, L], FP8, tag=f"hb0{b}")
                            eng.dma_start(
                                bass.AP(hb0[:].tensor, hb0[:].offset,
                                        [[16, 128], [1, 16]]),
                                g8[:, 16 * b : 16 * b + 16],
                            )
                            hbb = dr.tile([1, 129 * L], FP8, tag=f"hb{b}")
                            hb[b] = hbb
                            for (r0, r1), e2 in zip(
                                ((0, 65), (65, 129)), (nc.sync, nc.scalar)
                            ):
                                e2.dma_start(
                                    bass.AP(hbb[:].tensor,
                                            hbb[:].offset + L * r0,
                                            [[L, r1 - r0], [1, L]]),
                                    bass.AP(hb0[:].tensor, hb0[:].offset,
                                            [[0, r1 - r0], [1, L]]),
                                )
                        # softmax normalizers 1/Z per batch (off critical path)
                        nc.tensor.matmul(ps4[0][0:1, 0:32], lhsT=ones1[:],
                                         rhs=g8[:], start=True, stop=True)
                        zrow = ph2.tile([1, 32], F32, tag="zrow")
                        nc.scalar.activation(zrow[:], ps4[0][0:1, 0:32], AF.Copy)
                        z2 = ph2.tile([1, 2], F32, tag="z2")
                        ztmp = ph2.tile([1, 32], F32, tag="ztmp")
                        for b in range(BPC):
                            nc.scalar.activation(
                                ztmp[0:1, 16 * b : 16 * b + 16],
                                zrow[0:1, 16 * b : 16 * b + 16], AF.Copy,
                                accum_out=z2[0:1, b : b + 1],
                            )
                        nc.vector.reciprocal(out=z2[:], in_=z2[:])
                        # agg psum carries sum g~ * (SV v'); fold 1/SV here
                        nc.vector.tensor_scalar(
                            out=z2[:], in0=z2[:], scalar1=1.0 / SV,
                            scalar2=None, op0=AluOpType.mult,
                        )
                        nc.gpsimd.partition_broadcast(zib[:], z2[0:1, :])

                # == phases 3-4, batch order b1 then b0 (decomp overlaps PE) ==
                with (
                    tc.tile_pool(name="p3", bufs=1) as p3,
                    tc.tile_pool(name="p3r", bufs=2) as p3r,
                    tc.tile_pool(name="p3s", bufs=1) as p3s,
                ):
                    gbufs = {}
                    gsplit = ((0, 43), (43, 86), (86, 128))
                    for b in (1, 0):
                        gbuf = p3.tile([128, 3968], FP8, tag=f"gbuf{b}")
                        gbufs[b] = gbuf
                        for (i0, i1), eng in zip(
                            gsplit, (nc.sync, nc.scalar, nc.gpsimd)
                        ):
                            eng.dma_start(
                                gbuf[i0:i1, :],
                                bass.AP(hb[b][:].tensor,
                                        hb[b][:].offset + 127 + 2047 * i0,
                                        [[2047, i1 - i0], [1, 3968]]),
                            )
                    for b in (1, 0):
                        gbuf = gbufs[b]

                        def gpair(Bp, nw):
                            # [128, 2, 512] gbuf k-pair: v8 chunk j=2Bp is
                            # true time chunk TM-1-2Bp -> base column
                            # 512nw + 256Bp, +128 for the second k-subtile
                            gs = gbuf[:]
                            return bass.AP(
                                gs.tensor, gs.offset + 512 * nw + 256 * Bp,
                                [list(gs.ap[0]), [128, 2], [1, 512]],
                            )

                        # ac = sum_k g_k roll(v') + bop + x, drained straight
                        # to acx (Wo is folded into v' = x Wv Wo on the host)
                        acx = p3.tile([128, DC, L], F32, tag="acx")
                        for dm in range(DC):
                            for Bp in range(TM // 2):
                                for nw in range(TW):
                                    nc.tensor.matmul(
                                        ps4[nw][:],
                                        lhsT=v8[b][:, 2 * Bp : 2 * Bp + 2,
                                                   128 * dm : 128 * dm + 128],
                                        rhs=gpair(Bp, nw),
                                        start=(Bp == 0),
                                        stop=(Bp == TM // 2 - 1),
                                        perf_mode=DR,
                                    )
                            for nw in range(TW):
                                xr = p3r.tile([128, 512], F32, tag="xr")
                                nc.sync.dma_start(
                                    xr[:],
                                    x_dm.ap()[b, :,
                                              dm * L + 512 * nw :
                                              dm * L + 512 * nw + 512],
                                )
                                act = p3r.tile([128, 512], F32, tag="act")
                                nc.scalar.activation(
                                    act[:], ps4[nw][:], AF.Identity,
                                    scale=zib[:, b : b + 1],
                                    bias=bop_c[:, dm : dm + 1],
                                )
                                nc.vector.tensor_add(
                                    out=acx[:, dm, 512 * nw : 512 * nw + 512],
                                    in0=act[:], in1=xr[:],
                                )
                            # this channel chunk of acx is complete: its
                            # decomposition scan can overlap the next chunks'
                            # matmuls (and the following conv stages)
                            _decompose_chunk(nc, p3s, acx, seab[b], dm)

            # ======= phases 5-7, interleaved across the two batches =======
            with (
                tc.tile_pool(name="pcv", bufs=1) as pcv,
                tc.tile_pool(name="p5w1", bufs=2) as p5w1,
                tc.tile_pool(name="p5w2", bufs=2) as p5w2,
                tc.tile_pool(name="p6s", bufs=2) as p6s,
                tc.tile_pool(name="p7s", bufs=1) as p7s,
                tc.tile_pool(name="p7b", bufs=1) as p7b,
                tc.tile_pool(name="p7d", bufs=2, space="DRAM") as p7d,
                tc.tile_pool(name="ppc2", bufs=1, space="PSUM") as ppc2,
            ):
                pc2 = []
                for i in range(4):
                    t = ppc2.tile([128, 512], F32, tag=f"pc2_{i}", name=f"pc2_{i}")
                    pc2.append(t)
                h1 = pcv.tile([128, CH, L + 2], BF16, tag="h1")
                acc2 = pcv.tile([128, DC, L], BF16, tag="acc2")
                ysb = pcv.tile([128, DC, L], F32, tag="ysb")
                sea2 = pcv.tile([128, DC, L + 2], F32, tag="sea2")

                def conv1_half(b, half):
                    for co8 in range(CH):
                        co = CH * half + co8
                        w1t = p5w1.tile([128, DC * 3 * 128], BF16, tag="w1t")
                        nc.sync.dma_start(w1t[:], w1_d.ap()[co])
                        first = True
                        for dci in range(DC):
                            for tap in range(3):
                                ki = (3 * dci + tap) * 128
                                for nw in range(TW):
                                    nc.tensor.matmul(
                                        ps4[nw][:],
                                        lhsT=w1t[:, ki : ki + 128],
                                        rhs=seab[b][:, dci,
                                             512 * nw + tap : 512 * nw + tap + 512],
                                        start=first,
                                        stop=(dci == DC - 1 and tap == 2),
                                    )
                                first = False
                        for nw in range(TW):
                            nc.scalar.activation(
                                h1[:, co8, 1 + 512 * nw : 513 + 512 * nw],
                                ps4[nw][:], AF.Prelu, alpha=SLOPE,
                            )
                        nc.vector.tensor_copy(out=h1[:, co8, 0:1],
                                              in_=h1[:, co8, 1:2])
                        nc.vector.tensor_copy(out=h1[:, co8, L + 1 : L + 2],
                                              in_=h1[:, co8, L : L + 1])

                def conv2_pass(b, half):
                    for co in range(DC):
                        w2t = p5w2.tile([128, CH * 3 * 128], BF16, tag="w2t")
                        src = w2_d.ap()[co]
                        ofs = half * CH * 3 * 128
                        nc.sync.dma_start(
                            w2t[:],
                            bass.AP(src.tensor, src.offset + ofs,
                                    [[CFC * 3 * 128, 128], [1, CH * 3 * 128]]),
                        )
                        first = True
                        for ci8 in range(CH):
                            for tap in range(3):
                                ki = (3 * ci8 + tap) * 128
                                for nw in range(TW):
                                    nc.tensor.matmul(
                                        pc2[nw][:],
                                        lhsT=w2t[:, ki : ki + 128],
                                        rhs=h1[:, ci8,
                                             512 * nw + tap : 512 * nw + tap + 512],
                                        start=first,
                                        stop=(ci8 == CH - 1 and tap == 2),
                                    )
                                first = False
                        if half == 0:
                            for nw in range(TW):
                                nc.scalar.activation(
                                    acc2[:, co, 512 * nw : 512 * nw + 512],
                                    pc2[nw][:], AF.Copy,
                                )
                        else:
                            for nw in range(TW):
                                h2t = p6s.tile([128, 512], F32, tag="h2t")
                                nc.vector.tensor_add(
                                    out=h2t[:], in0=pc2[nw][:],
                                    in1=acc2[:, co, 512 * nw : 512 * nw + 512],
                                )
                                h2r = p6s.tile([128, 512], F32, tag="h2r")
                                nc.scalar.activation(h2r[:], h2t[:], AF.Prelu,
                                                     alpha=SLOPE)
                                nc.vector.tensor_add(
                                    out=ysb[:, co, 512 * nw : 512 * nw + 512],
                                    in0=h2r[:],
                                    in1=seab[b][:, co,
                                                1 + 512 * nw : 513 + 512 * nw],
                                )
                            # decompose2 of this channel chunk can start now
                            _decompose_chunk(nc, p7s, ysb, sea2, co)

                def layernorm(b):
                    # windowed pipeline; stats borrow conv2's PSUM banks
                    for twi in range(TW):
                        st_s = pc2[2 * (twi % 2)][0:1, :]
                        st_q = pc2[2 * (twi % 2) + 1][0:1, :]
                        for dci in range(DC):
                            sqt = p6s.tile([128, 512], F32, tag="sqt")
                            nc.scalar.activation(
                                sqt[:],
                                sea2[:, dci, 1 + 512 * twi : 513 + 512 * twi],
                                AF.Square,
                            )
                            nc.tensor.matmul(
                                st_s,
                                lhsT=ones_mv[:],
                                rhs=sea2[:, dci, 1 + 512 * twi : 513 + 512 * twi],
                                start=(dci == 0), stop=(dci == DC - 1),
                            )
                            nc.tensor.matmul(
                                st_q,
                                lhsT=ones_mv[:],
                                rhs=sqt[:],
                                start=(dci == 0), stop=(dci == DC - 1),
                            )
                        mu = p7b.tile([1, 512], F32, tag="mu")
                        rs = p7b.tile([1, 512], F32, tag="rs")
                        nc.scalar.activation(mu[:], st_s, AF.Copy)
                        nc.vector.tensor_mul(out=rs[:], in0=mu[:], in1=mu[:])
                        nc.vector.tensor_sub(out=rs[:], in0=st_q, in1=rs[:])
                        nc.vector.tensor_scalar_add(rs[:], rs[:], EPS)
                        nc.vector.reciprocal(out=rs[:], in_=rs[:])
                        nc.scalar.activation(rs[:], rs[:], AF.Sqrt)
                        mub = p7b.tile([128, 512], F32, tag="mub")
                        rsb = p7b.tile([128, 512], F32, tag="rsb")
                        nc.gpsimd.partition_broadcast(mub[:], mu[:])
                        nc.gpsimd.partition_broadcast(rsb[:], rs[:])
                        for dci in range(DC):
                            ve = nc.vector
                            og = p6s.tile([128, 512], F32, tag="og")
                            ve.tensor_sub(
                                out=og[:],
                                in0=sea2[:, dci, 1 + 512 * twi : 513 + 512 * twi],
                                in1=mub[:],
                            )
                            ve.scalar_tensor_tensor(
                                out=og[:], in0=og[:],
                                scalar=lng_c[:, dci : dci + 1], in1=rsb[:],
                                op0=AluOpType.mult, op1=AluOpType.mult,
                            )
                            nc.scalar.activation(
                                og[:], og[:], AF.Identity,
                                bias=lnb_c[:, dci : dci + 1],
                            )
                            nc.scalar.dma_start(
                                out_dm.ap()[b, :, dci,
                                            512 * twi : 512 * twi + 512],
                                og[:],
                            )

                # schedule: conv(b1) fully; LN(b1) sits between batch-0 conv
                # stages so its vector/DMA chains hide under PE work.
                conv1_half(1, 0)
                conv2_pass(1, 0)
                conv1_half(1, 1)
                conv2_pass(1, 1)      # finalize emits decompose2(b1) per chunk
                conv1_half(0, 0)
                layernorm(1)          # stats borrow pc2 (free here)
                conv2_pass(0, 0)
                conv1_half(0, 1)
                conv2_pass(0, 1)      # finalize emits decompose2(b0) per chunk
                layernorm(0)


# ---------------------------------------------------------------------------
# host side
# ---------------------------------------------------------------------------
_CACHE = {}


def _get_nc(n_group: int):
    key = n_group
    if key not in _CACHE:
        nc = bacc.Bacc("TRN2", target_bir_lowering=False, debug=False,
                       num_devices=n_group)
        build(nc, n_group)
        nc.compile()
        _CACHE[key] = nc
    return _CACHE[key]


def stage_inputs(inputs, ncores=NCORES):
    x = np.asarray(inputs["x"], np.float32)
    Wq = np.asarray(inputs["Wq"], np.float32)
    Wk = np.asarray(inputs["Wk"], np.float32)
    Wv = np.asarray(inputs["Wv"], np.float32)
    Wo = np.asarray(inputs["Wo"], np.float32)
    bq = np.asarray(inputs["bq"], np.float32)
    bk = np.asarray(inputs["bk"], np.float32)
    bv = np.asarray(inputs["bv"], np.float32)
    bo = np.asarray(inputs["bo"], np.float32)
    w1 = np.asarray(inputs["conv1_w"], np.float32)
    w2 = np.asarray(inputs["conv2_w"], np.float32)
    lng = np.asarray(inputs["ln_g"], np.float32)
    lnb = np.asarray(inputs["ln_b"], np.float32)

    bop = bo + bv @ Wo
    # merged QK projection: the statistic only needs x^T (Wq Wk^T) x
    # (biases drop: they shift the statistic by a constant, and both topk
    # and softmax are shift-invariant). Wo folds into the V projection.
    Wm = Wq @ Wk.T
    Wvo = Wv @ Wo
    col = lambda v: np.ascontiguousarray(v.reshape(DC, 128).T)
    # projection weights partition-major: W[dci*128+p, n] -> [p, dci*D + n]
    wmaj = lambda W: np.ascontiguousarray(
        W.reshape(DC, 128, D).transpose(1, 0, 2).reshape(128, DC * D)
    ).astype(BF16_NP)
    # same, scaled into the e4m3 normal range
    wmaj8 = lambda W: np.ascontiguousarray(
        np.clip(W * SW, -240, 240)
        .reshape(DC, 128, D).transpose(1, 0, 2).reshape(128, DC * D)
    ).astype(FP8_NP)
    # conv1 [3, D, CF] -> [CFC, 128(ci-part), DC*3*128(co)]
    w1h = np.ascontiguousarray(
        w1.reshape(3, DC, 128, CFC, 128).transpose(3, 2, 1, 0, 4)
    ).reshape(CFC, 128, DC * 3 * 128).astype(BF16_NP)
    # conv2 [3, CF, D] -> [DC, 128(ci-part), CFC*3*128(co)]
    w2h = np.ascontiguousarray(
        w2.reshape(3, CFC, 128, DC, 128).transpose(3, 2, 1, 0, 4)
    ).reshape(DC, 128, CFC * 3 * 128).astype(BF16_NP)

    shared = {
        "wm": wmaj(Wm), "wv": wmaj8(Wvo),
        "bop_t": col(bop),
        "w1h": w1h, "w2h": w2h, "lng_t": col(lng), "lnb_t": col(lnb),
    }
    bpc = B // ncores
    in_maps = []
    for c in range(ncores):
        m = dict(shared)
        # x [bpc, L, D] -> [bpc, 128, DC*L]  (d-major per partition)
        xc = x[bpc * c : bpc * (c + 1)]
        xc = np.ascontiguousarray(
            xc.reshape(bpc, L, DC, 128).transpose(0, 3, 2, 1)
        ).reshape(bpc, 128, DC * L)
        m["x_dm"] = xc
        in_maps.append(m)
    return in_maps


def unstage_output(res, ncores=NCORES):
    out = np.empty((B, L, D), np.float32)
    bpc = B // ncores
    for c in range(ncores):
        o = np.asarray(res.results[c]["out_dm"])  # [bpc, 128, DC, L]
        for i in range(bpc):
            # full[t, dci*128+p] = o[i][p, dci, t]
            out[bpc * c + i] = o[i].transpose(2, 1, 0).reshape(L, D)
    return out


def kernel(**inputs):
    nc = _get_nc(NCORES)
    in_maps = stage_inputs(inputs)
    res = bass_utils.run_bass_kernel_spmd(nc, in_maps, core_ids=list(range(NCORES)))
    return unstage_output(res)



# revision 55
# speedup vs baseline: 1.1525x; 1.0493x over previous
"""Trainium2 Bass kernel for nn_Encoder_78889959293176 (Autoformer-style encoder layer).

Strategy: data-parallel over batch (16 batches -> 8 cores x 2).
All heavy compute on the TensorEngine in a d-major ([channel, time]) layout:
  - the Q/K projections are merged on the host into M = Wq Wk^T (the
    statistic only needs x^T M x; biases shift it by a constant, and both
    topk and softmax are shift-invariant), Wo is folded into the V
    projection (v' = x Wv Wo), so only two projections remain
  - autocorrelation statistic mean_value via q'^T x tiles + a 2-copy
    diagonal "shear" DMA into DRAM + ones-matmul partition reduction
    (flipped-tau space)
  - AllReduce(8 cores) of the batch-summed statistic, on-device top-22
    threshold via gpsimd kth_largest, masked exp in a [16,128] layout
    (unnormalized; 1/Z folds into the agg PSUM drain scale)
  - the rolls-weighted aggregation as an fp8e4 DoubleRow circulant matmul
    of v' (e4m3, time chunks reversed) against a Toeplitz band buffer
    read row-step-2047 from a 129-copy periodic DRAM buffer built by one
    16-packet row write + a 3-way step-0-source DRAM->DRAM blast
  - series decomposition via tensor_tensor_scan cumsum, convs as bf16
    matmuls (fp8 convs/statistic provably break the rel-err budget /
    top-k selection), layernorm stats via ones-matmuls.

Pipeline layout (single pool scopes, no per-batch barriers):
  - per-batch statistic AllReduces issue right after each batch's phase 1;
    batch 1's collective + kth_largest hide under the two V projections.
  - phases 3..7 interleave the two batches so the vector-engine chains
    (decompositions, layernorm) of one batch always overlap TensorEngine
    conv work of the other batch; layernorm stats borrow conv2's PSUM banks.
  - conv2 runs in two channel-half passes (partials parked in bf16 acc2) so
    h1 only holds 8 of 16 hidden chunks.
  - all DRAM operands are host-staged partition-major: every DMA line is
    contiguous per partition.
"""

import numpy as np

import concourse.bass as bass
import concourse.bacc as bacc
import concourse.mybir as mybir
import concourse.tile as tile
from concourse import bass_utils
from concourse import library_config
from concourse.alu_op_type import AluOpType

try:
    import ml_dtypes

    BF16_NP = ml_dtypes.bfloat16
    FP8_NP = ml_dtypes.float8_e4m3
except Exception:  # pragma: no cover
    BF16_NP = np.float32
    FP8_NP = np.float32

F32 = mybir.dt.float32
BF16 = mybir.dt.bfloat16
FP8 = mybir.dt.float8e4
DR = mybir.MatmulPerfMode.DoubleRow
AF = mybir.ActivationFunctionType

# fp8 e4m3 scale factors (folded back out via matmul-drain scales / the
# mean-value reduction constant)
SX = 4.0    # x
SW = 64.0   # wm = Wq Wk^T and wvo = Wv Wo weight matrices
SQ = 16.0   # q' = x (Wq Wk^T)
SV = 16.0   # v' = x (Wv Wo)

B, L, D = 16, 2048, 512
CF = 2048  # conv hidden
TOPK = 22
KER = 25
EPS = 1e-5
SLOPE = 0.01
NCORES = 8
BPC = B // NCORES  # batches per core
DC = D // 128  # 4 d-chunks
CFC = CF // 128  # 16 conv-hidden chunks
CH = CFC // 2  # conv-hidden chunks per half
TW = L // 512  # 4 time windows of 512
TM = L // 128  # 16 time chunks of 128
NEG = -1.0e30


def build(nc: bass.Bass, n_group: int):
    x_dm = nc.dram_tensor("x_dm", [BPC, 128, DC * L], F32, kind="ExternalInput")
    wm_d = nc.dram_tensor("wm", [128, DC * D], BF16, kind="ExternalInput")
    wv_d = nc.dram_tensor("wv", [128, DC * D], FP8, kind="ExternalInput")
    bop_d = nc.dram_tensor("bop_t", [128, DC], F32, kind="ExternalInput")
    w1_d = nc.dram_tensor("w1h", [CFC, 128, DC * 3 * 128], BF16, kind="ExternalInput")
    w2_d = nc.dram_tensor("w2h", [DC, 128, CFC * 3 * 128], BF16, kind="ExternalInput")
    lng_d = nc.dram_tensor("lng_t", [128, DC], F32, kind="ExternalInput")
    lnb_d = nc.dram_tensor("lnb_t", [128, DC], F32, kind="ExternalInput")
    out_dm = nc.dram_tensor("out_dm", [BPC, 128, DC, L], F32, kind="ExternalOutput")

    with tile.TileContext(nc) as tc:
        _body(nc, tc, n_group, x_dm, wm_d, wv_d, bop_d,
              w1_d, w2_d, lng_d, lnb_d, out_dm)
    return nc


def _decompose_chunk(nc, scan_pool, src, dst, dci):
    """dst[:, dci, 1:L+1] = src[:, dci] - movavg_KER(src[:, dci])."""
    half = (KER - 1) // 2
    pad = scan_pool.tile([128, L + KER], F32, tag="scan_pad")
    cs = scan_pool.tile([128, L + KER], F32, tag="scan_cs")
    nc.vector.memset(pad[:, 0:1], 0.0)
    nc.vector.tensor_copy(
        out=pad[:, 1 : 1 + half],
        in_=src[:, dci, 0:1].to_broadcast([128, half]),
    )
    nc.scalar.activation(pad[:, 1 + half : 1 + half + L], src[:, dci, :], AF.Copy)
    nc.vector.tensor_copy(
        out=pad[:, 1 + half + L :],
        in_=src[:, dci, L - 1 : L].to_broadcast([128, half]),
    )
    nc.vector.tensor_tensor_scan(
        out=cs[:], data0=pad[:], data1=pad[:], initial=0.0,
        op0=AluOpType.add, op1=AluOpType.bypass,
    )
    d1 = pad[:, 0:L]  # cumsum done; reuse pad for the boxcar difference
    nc.vector.tensor_sub(out=d1, in0=cs[:, KER:], in1=cs[:, 0:L])
    nc.vector.scalar_tensor_tensor(
        out=dst[:, dci, 1 : L + 1], in0=d1, scalar=-1.0 / KER,
        in1=src[:, dci, :], op0=AluOpType.mult, op1=AluOpType.add,
    )
    nc.vector.tensor_copy(out=dst[:, dci, 0:1], in_=dst[:, dci, 1:2])
    nc.vector.tensor_copy(
        out=dst[:, dci, L + 1 : L + 2], in_=dst[:, dci, L : L + 1]
    )


def _decompose(nc, scan_pool, src, dst):
    for dci in range(DC):
        _decompose_chunk(nc, scan_pool, src, dst, dci)


def _body(nc, tc, n_group, x_dm, wm_d, wv_d, bop_d,
          w1_d, w2_d, lng_d, lnb_d, out_dm):
    with (
        tc.tile_pool(name="p0", bufs=1) as p0,
        tc.tile_pool(name="pp", bufs=1, space="PSUM") as pp,
        tc.tile_pool(name="dr", bufs=1, space="DRAM") as dr,
        tc.tile_pool(name="dr3", bufs=4, space="DRAM") as dr3,
    ):
        nc.gpsimd.load_library(library_config.attn)
        # ----- persistent constants -----
        ones_mv = p0.tile([128, 1], F32, tag="ones_mv")
        nc.vector.memset(ones_mv[:], 1.0 / D)
        ones_bf = p0.tile([128, 1], BF16, tag="ones_bf")
        nc.vector.memset(ones_bf[:], 1.0 / D)
        ones16 = p0.tile([16, 1], BF16, tag="ones16")
        nc.vector.memset(ones16[:], 1.0)
        bop_c = p0.tile([128, DC], F32, tag="bop_c")
        lng_c = p0.tile([128, DC], F32, tag="lng_c")
        lnb_c = p0.tile([128, DC], F32, tag="lnb_c")
        nc.sync.dma_start(bop_c[:], bop_d[:, :])
        nc.sync.dma_start(lng_c[:], lng_d[:, :])
        nc.sync.dma_start(lnb_c[:], lnb_d[:, :])

        # 4 rotating PSUM accumulators shared by all phases
        ps4 = []
        for i in range(4):
            t = pp.tile([128, 512], F32, tag=f"ps{i}", name=f"ps_{i}")
            ps4.append(t)

        hb = {}
        seab = []
        cco = []

        with tc.tile_pool(name="psea", bufs=1) as psea:
            for b in range(BPC):
                t = psea.tile([128, DC, L + 2], BF16, tag=f"seab{b}",
                              name=f"seab_{b}")
                seab.append(t)

            with tc.tile_pool(name="pv", bufs=1) as pv:
                # v' = x (Wv Wo) in e4m3, time chunks stored REVERSED
                # (chunk j = true time chunk TM-1-j) so the DoubleRow agg
                # pairs read gbuf with a positive +128 k-substride.
                v8 = []
                for b in range(BPC):
                    t = pv.tile([128, TM, D], FP8, tag=f"v8_{b}", name=f"v8_{b}")
                    v8.append(t)
                # unnormalized masked-exp weights in [16,128] layout
                # (row r holds stat indices 128r..128r+127, so the hb0 write
                # is 16 contiguous 128B packets); softmax 1/Z folded into
                # the agg PSUM drain via zib.
                g16 = pv.tile([16, 2 * 128], FP8, tag="g16")
                zib = pv.tile([128, 2], F32, tag="zib")
                mvloc = pv.tile([16, 2, 128], F32, tag="mvloc")
                bs16 = pv.tile([16, 2, 128], F32, tag="bs16")

                with tc.tile_pool(name="pxv", bufs=1) as pxv:
                    xb = []
                    for b in range(BPC):
                        t = pxv.tile([128, DC, L], BF16, tag=f"xbt{b}",
                                     name=f"xb_{b}")
                        xb.append(t)
                    mvf = pxv.tile([1, BPC * L], F32, tag="mvf")

                    # ========= phase 1: mean_value (flipped space) =========
                    with (
                        tc.tile_pool(name="ph1", bufs=1) as ph1,
                        tc.tile_pool(name="ph1b", bufs=4) as ph1b,
                        tc.tile_pool(name="ph1w", bufs=3) as ph1w,
                        tc.tile_pool(name="ppm1", bufs=1, space="PSUM") as ppm1,
                    ):
                        wm_s = ph1.tile([128, DC, D], BF16, tag="wqk")
                        nc.sync.dma_start(wm_s[:], wm_d.ap())
                        # fine-grained x loads; bf16 convert (scalar) + scaled
                        # e4m3 convert (vector), both batches
                        for b in range(BPC):
                            for dci in range(DC):
                                for tw in range(TW):
                                    xq = ph1b.tile([128, 512], F32, tag="xq")
                                    o = dci * L + 512 * tw
                                    nc.sync.dma_start(
                                        xq[:], x_dm.ap()[b, :, o : o + 512]
                                    )
                                    nc.scalar.activation(
                                        xb[b][:, dci, 512 * tw : 512 * tw + 512],
                                        xq[:], AF.Copy,
                                    )

                        mv_reg = []
                        for i in range(4):
                            t = ppm1.tile([1, 512], F32, tag=f"mv{i}",
                                          name=f"mv_{i}")
                            mv_reg.append(t)

                        for b in range(BPC):
                            # q' = (Wq Wk^T)^T x (biases provably drop out of
                            # the statistic: rank-1 terms are constant over
                            # tau, and topk+softmax are shift-invariant)
                            q_s = ph1.tile([128, DC, L], BF16, tag="q_s")
                            for dco in range(DC):
                                for dci in range(DC):
                                    for twi in range(TW):
                                        nc.tensor.matmul(
                                            ps4[twi][:],
                                            lhsT=wm_s[:, dci,
                                                      128 * dco : 128 * dco + 128],
                                            rhs=xb[b][:, dci,
                                                      512 * twi : 512 * twi + 512],
                                            start=(dci == 0),
                                            stop=(dci == DC - 1),
                                        )
                                for twi in range(TW):
                                    nc.scalar.activation(
                                        q_s[:, dco, 512 * twi : 512 * twi + 512],
                                        ps4[twi][:], AF.Copy,
                                    )

                            def _emit_mv(A, wa):
                                for cc in range(4):
                                    w0 = (512 * cc + 128 * A) % L
                                    nc.tensor.matmul(
                                        mv_reg[cc][0:1, :],
                                        lhsT=ones_bf[:],
                                        rhs=wa[:, w0 : w0 + 512],
                                        start=(A == 0), stop=(A == TM - 1),
                                    )

                            pend = []
                            for A in range(TM):
                                bufA = dr3.tile([128, 4224], BF16, tag="bufA")
                                for dci in range(DC):
                                    for tB in range(TW):
                                        nc.tensor.matmul(
                                            ps4[tB][:],
                                            lhsT=q_s[:, dci,
                                                     128 * A : 128 * A + 128],
                                            rhs=xb[b][:, dci,
                                                     512 * tB : 512 * tB + 512],
                                            start=(dci == 0),
                                            stop=(dci == DC - 1),
                                        )
                                for tB in range(TW):
                                    c_sb = ph1b.tile([128, 512], BF16, tag="c_sb")
                                    nc.scalar.activation(c_sb[:], ps4[tB][:],
                                                         AF.Copy)
                                    for cp, eng in ((0, nc.sync), (1, nc.scalar)):
                                        dst = bass.AP(
                                            bufA[:].tensor,
                                            127 + 512 * tB + 2048 * cp,
                                            [[4223, 128], [1, 512]],
                                        )
                                        eng.dma_start(dst, c_sb[:])
                                wa = ph1w.tile([128, 2560], BF16, tag="wa")
                                nc.sync.dma_start(
                                    wa[:],
                                    bass.AP(bufA[:].tensor, 128,
                                            [[4224, 128], [1, 2560]]),
                                )
                                pend.append((A, wa))
                                if len(pend) > 2:
                                    _emit_mv(*pend.pop(0))
                            for a_w in pend:
                                _emit_mv(*a_w)
                            for cc in range(4):
                                nc.scalar.activation(
                                    mvf[0:1,
                                        L * b + 512 * cc : L * b + 512 * cc + 512],
                                    mv_reg[cc][0:1, :], AF.Copy,
                                )
                            # issue this batch's AllReduce immediately: batch
                            # 0's collective hides under batch 1's phase 1.
                            cci = dr.tile([1, L], F32, tag=f"cci{b}")
                            cc_o = dr.tile([1, L], F32, tag=f"cco{b}")
                            nc.sync.dma_start(cci[:], mvf[0:1, L * b : L * b + L])
                            nc.gpsimd.collective_compute(
                                "AllReduce", AluOpType.add,
                                replica_groups=[list(range(n_group))],
                                ins=[cci[:].opt()], outs=[cc_o[:].opt()],
                            )
                            cco.append(cc_o)
                            # local + summed statistic back in [16,128]
                            # layout for the masked-exp path (off critical
                            # path; 16 contiguous packets each)
                            nc.gpsimd.dma_start(
                                mvloc[:, b, :],
                                bass.AP(cci[:].tensor, cci[:].offset,
                                        [[128, 16], [1, 128]]),
                            )
                            nc.gpsimd.dma_start(
                                bs16[:, b, :],
                                bass.AP(cc_o[:].tensor, cc_o[:].offset,
                                        [[128, 16], [1, 128]]),
                            )

                    # ========= phase 2: topk + softmax (both batches) ========
                    with (
                        tc.tile_pool(name="ph2", bufs=1) as ph2,
                        tc.tile_pool(name="ph2w", bufs=1) as ph2w,
                    ):
                        wv_s = ph2w.tile([128, DC, D], FP8, tag="wv_s")
                        nc.sync.dma_start(wv_s[:], wv_d.ap())

                        def _vproj(b):
                            # bf16 x against fp8 Wvo; psum = SW * v'
                            for tm in range(TM):
                                pt = ps4[tm % 4]
                                for dci in range(DC):
                                    nc.tensor.matmul(
                                        pt[:],
                                        lhsT=xb[b][:, dci,
                                                   128 * tm : 128 * tm + 128],
                                        rhs=wv_s[:, dci, :],
                                        start=(dci == 0),
                                        stop=(dci == DC - 1),
                                    )
                                # reversed chunk order for the DoubleRow agg
                                nc.scalar.activation(
                                    v8[b][:, TM - 1 - tm, :], pt[:], AF.Copy,
                                    scale=SV / SW,
                                )

                        _vproj(0)  # hides the second collective
                        # top-22 threshold via gpsimd kth_largest: the lerped
                        # quantile at rank 21.5 lies strictly between the 22nd
                        # and 23rd largest values -> tie-free >= mask.
                        bsA = ph2.tile([128, 16], F32, tag="bsA")
                        bsB = ph2.tile([128, 16], F32, tag="bsB")
                        nc.sync.dma_start(
                            bsA[:], bass.AP(cco[0][:].tensor, 0, [[16, 128], [1, 16]])
                        )
                        nc.sync.dma_start(
                            bsB[:], bass.AP(cco[1][:].tensor, 0, [[16, 128], [1, 16]])
                        )
                        bs128 = ph2.tile([128, 16], F32, tag="bs128")
                        nc.vector.tensor_add(out=bs128[:], in0=bsA[:], in1=bsB[:])
                        kth = ph2.tile([1, 2], F32, tag="kth")
                        nc.gpsimd.kth_largest(
                            kth[:], bs128[:], 16, 24,
                            quantile=1.0 - (TOPK - 0.5) / (L - 1),
                        )
                        _vproj(1)  # hides kth_largest
                        # masked exp (unnormalized) on [16,128] per batch;
                        # normalization deferred to the agg PSUM drain.
                        thr16 = ph2.tile([16, 1], F32, tag="thr16")
                        nc.gpsimd.partition_broadcast(thr16[:], kth[0:1, 0:1],
                                                      channels=16)
                        bsum16 = ph2.tile([16, 128], F32, tag="bsum16")
                        nc.vector.tensor_add(out=bsum16[:], in0=bs16[:, 0, :],
                                             in1=bs16[:, 1, :])
                        mask16 = ph2.tile([16, 128], F32, tag="mask16")
                        nc.vector.tensor_scalar(
                            out=mask16[:], in0=bsum16[:], scalar1=thr16[:, 0:1],
                            scalar2=None, op0=AluOpType.is_ge,
                        )
                        # stt = stat*mask + (mask-1)*1e9: selected entries keep
                        # the exact statistic (no 1e9 roundtrip -- f32 at 1e9
                        # has quantum 64, which would wipe out the values).
                        neg916 = ph2.tile([16, 128], F32, tag="neg916")
                        nc.vector.tensor_scalar(
                            out=neg916[:], in0=mask16[:], scalar1=1.0,
                            scalar2=1.0e9,
                            op0=AluOpType.subtract, op1=AluOpType.mult,
                        )
                        stt = ph2.tile([16, 2 * 128], F32, tag="stt")
                        for b in range(BPC):
                            sl = stt[:, 128 * b : 128 * b + 128]
                            nc.vector.tensor_mul(
                                out=sl, in0=mvloc[:, b, :], in1=mask16[:],
                            )
                            nc.vector.tensor_add(out=sl, in0=sl, in1=neg916[:])
                        nc.scalar.activation(g16[:], stt[:], AF.Exp)
                        # periodic replication B[q] = g_f[q mod L]: 16-packet
                        # row write + 3-way step-0-source DRAM->DRAM blast
                        # (the row-step-2047 gbuf read needs 129 copies).
                        for b, eng in ((1, nc.sync), (0, nc.scalar)):
                            hb0 = dr.tile([1, L], FP8, tag=f"hb0{b}")
                            eng.dma_start(
                                bass.AP(hb0[:].tensor, hb0[:].offset,
                                        [[128, 16], [1, 128]]),
                                g16[:, 128 * b : 128 * b + 128],
                            )
                            hbb = dr.tile([1, 129 * L], FP8, tag=f"hb{b}")
                            hb[b] = hbb
                            for (r0, r1), e2 in zip(
                                ((0, 43), (43, 86), (86, 129)),
                                (nc.sync, nc.scalar, nc.gpsimd),
                            ):
                                e2.dma_start(
                                    bass.AP(hbb[:].tensor,
                                            hbb[:].offset + L * r0,
                                            [[L, r1 - r0], [1, L]]),
                                    bass.AP(hb0[:].tensor, hb0[:].offset,
                                            [[0, r1 - r0], [1, L]]),
                                )
                        # softmax normalizers 1/Z per batch (off critical path)
                        nc.tensor.matmul(ps4[0][0:1, 0:256], lhsT=ones16[:],
                                         rhs=g16[:], start=True, stop=True)
                        zrow = ph2.tile([1, 256], F32, tag="zrow")
                        nc.scalar.activation(zrow[:], ps4[0][0:1, 0:256],
                                             AF.Copy)
                        z2 = ph2.tile([1, 2], F32, tag="z2")
                        ztmp = ph2.tile([1, 256], F32, tag="ztmp")
                        for b in range(BPC):
                            nc.scalar.activation(
                                ztmp[0:1, 128 * b : 128 * b + 128],
                                zrow[0:1, 128 * b : 128 * b + 128], AF.Copy,
                                accum_out=z2[0:1, b : b + 1],
                            )
                        nc.vector.reciprocal(out=z2[:], in_=z2[:])
                        # agg psum carries sum g~ * (SV v'); fold 1/SV here
                        nc.vector.tensor_scalar(
                            out=z2[:], in0=z2[:], scalar1=1.0 / SV,
                            scalar2=None, op0=AluOpType.mult,
                        )
                        nc.gpsimd.partition_broadcast(zib[:], z2[0:1, :])

                # == phases 3-4, batch order b1 then b0 (decomp overlaps PE) ==
                with (
                    tc.tile_pool(name="p3", bufs=1) as p3,
                    tc.tile_pool(name="p3r", bufs=2) as p3r,
                    tc.tile_pool(name="p3s", bufs=1) as p3s,
                ):
                    gbufs = {}
                    # b1 (needed first) on all 3 queues; b0 on scalar/gpsimd
                    # only so the sync queue stays free for the xr loads that
                    # pace the b1 acx drains.
                    for b, split in (
                        (1, (((0, 43), nc.sync), ((43, 86), nc.scalar),
                             ((86, 128), nc.gpsimd))),
                        (0, (((0, 64), nc.scalar), ((64, 128), nc.gpsimd))),
                    ):
                        gbuf = p3.tile([128, 3968], FP8, tag=f"gbuf{b}")
                        gbufs[b] = gbuf
                        for (i0, i1), eng in split:
                            eng.dma_start(
                                gbuf[i0:i1, :],
                                bass.AP(hb[b][:].tensor,
                                        hb[b][:].offset + 127 + 2047 * i0,
                                        [[2047, i1 - i0], [1, 3968]]),
                            )
                    for b in (1, 0):
                        gbuf = gbufs[b]

                        def gpair(Bp, nw):
                            # [128, 2, 512] gbuf k-pair: v8 chunk j=2Bp is
                            # true time chunk TM-1-2Bp -> base column
                            # 512nw + 256Bp, +128 for the second k-subtile
                            gs = gbuf[:]
                            return bass.AP(
                                gs.tensor, gs.offset + 512 * nw + 256 * Bp,
                                [list(gs.ap[0]), [128, 2], [1, 512]],
                            )

                        # ac = sum_k g_k roll(v') + bop + x, drained straight
                        # to acx (Wo is folded into v' = x Wv Wo on the host)
                        acx = p3.tile([128, DC, L], F32, tag="acx")
                        for dm in range(DC):
                            for Bp in range(TM // 2):
                                for nw in range(TW):
                                    nc.tensor.matmul(
                                        ps4[nw][:],
                                        lhsT=v8[b][:, 2 * Bp : 2 * Bp + 2,
                                                   128 * dm : 128 * dm + 128],
                                        rhs=gpair(Bp, nw),
                                        start=(Bp == 0),
                                        stop=(Bp == TM // 2 - 1),
                                        perf_mode=DR,
                                    )
                            for nw in range(TW):
                                xr = p3r.tile([128, 512], F32, tag="xr")
                                nc.sync.dma_start(
                                    xr[:],
                                    x_dm.ap()[b, :,
                                              dm * L + 512 * nw :
                                              dm * L + 512 * nw + 512],
                                )
                                act = p3r.tile([128, 512], F32, tag="act")
                                nc.scalar.activation(
                                    act[:], ps4[nw][:], AF.Identity,
                                    scale=zib[:, b : b + 1],
                                    bias=bop_c[:, dm : dm + 1],
                                )
                                nc.vector.tensor_add(
                                    out=acx[:, dm, 512 * nw : 512 * nw + 512],
                                    in0=act[:], in1=xr[:],
                                )
                            # this channel chunk of acx is complete: its
                            # decomposition scan can overlap the next chunks'
                            # matmuls (and the following conv stages)
                            _decompose_chunk(nc, p3s, acx, seab[b], dm)

            # ======= phases 5-7, interleaved across the two batches =======
            with (
                tc.tile_pool(name="pcv", bufs=1) as pcv,
                tc.tile_pool(name="p5w1", bufs=2) as p5w1,
                tc.tile_pool(name="p5w2", bufs=2) as p5w2,
                tc.tile_pool(name="p6s", bufs=2) as p6s,
                tc.tile_pool(name="p7s", bufs=1) as p7s,
                tc.tile_pool(name="p7b", bufs=1) as p7b,
                tc.tile_pool(name="p7d", bufs=2, space="DRAM") as p7d,
                tc.tile_pool(name="ppc2", bufs=1, space="PSUM") as ppc2,
            ):
                pc2 = []
                for i in range(4):
                    t = ppc2.tile([128, 512], F32, tag=f"pc2_{i}", name=f"pc2_{i}")
                    pc2.append(t)
                h1 = pcv.tile([128, CH, L + 2], BF16, tag="h1")
                acc2 = pcv.tile([128, DC, L], BF16, tag="acc2")
                ysb = pcv.tile([128, DC, L], F32, tag="ysb")
                sea2 = pcv.tile([128, DC, L + 2], F32, tag="sea2")

                def conv1_half(b, half):
                    for co8 in range(CH):
                        co = CH * half + co8
                        w1t = p5w1.tile([128, DC * 3 * 128], BF16, tag="w1t")
                        nc.sync.dma_start(w1t[:], w1_d.ap()[co])
                        first = True
                        for dci in range(DC):
                            for tap in range(3):
                                ki = (3 * dci + tap) * 128
                                for nw in range(TW):
                                    nc.tensor.matmul(
                                        ps4[nw][:],
                                        lhsT=w1t[:, ki : ki + 128],
                                        rhs=seab[b][:, dci,
                                             512 * nw + tap : 512 * nw + tap + 512],
                                        start=first,
                                        stop=(dci == DC - 1 and tap == 2),
                                    )
                                first = False
                        for nw in range(TW):
                            nc.scalar.activation(
                                h1[:, co8, 1 + 512 * nw : 513 + 512 * nw],
                                ps4[nw][:], AF.Prelu, alpha=SLOPE,
                            )
                        nc.vector.tensor_copy(out=h1[:, co8, 0:1],
                                              in_=h1[:, co8, 1:2])
                        nc.vector.tensor_copy(out=h1[:, co8, L + 1 : L + 2],
                                              in_=h1[:, co8, L : L + 1])

                def conv2_pass(b, half):
                    for co in range(DC):
                        w2t = p5w2.tile([128, CH * 3 * 128], BF16, tag="w2t")
                        src = w2_d.ap()[co]
                        ofs = half * CH * 3 * 128
                        nc.sync.dma_start(
                            w2t[:],
                            bass.AP(src.tensor, src.offset + ofs,
                                    [[CFC * 3 * 128, 128], [1, CH * 3 * 128]]),
                        )
                        first = True
                        for ci8 in range(CH):
                            for tap in range(3):
                                ki = (3 * ci8 + tap) * 128
                                for nw in range(TW):
                                    nc.tensor.matmul(
                                        pc2[nw][:],
                                        lhsT=w2t[:, ki : ki + 128],
                                        rhs=h1[:, ci8,
                                             512 * nw + tap : 512 * nw + tap + 512],
                                        start=first,
                                        stop=(ci8 == CH - 1 and tap == 2),
                                    )
                                first = False
                        if half == 0:
                            for nw in range(TW):
                                nc.scalar.activation(
                                    acc2[:, co, 512 * nw : 512 * nw + 512],
                                    pc2[nw][:], AF.Copy,
                                )
                        else:
                            for nw in range(TW):
                                h2t = p6s.tile([128, 512], F32, tag="h2t")
                                nc.vector.tensor_add(
                                    out=h2t[:], in0=pc2[nw][:],
                                    in1=acc2[:, co, 512 * nw : 512 * nw + 512],
                                )
                                h2r = p6s.tile([128, 512], F32, tag="h2r")
                                nc.scalar.activation(h2r[:], h2t[:], AF.Prelu,
                                                     alpha=SLOPE)
                                nc.vector.tensor_add(
                                    out=ysb[:, co, 512 * nw : 512 * nw + 512],
                                    in0=h2r[:],
                                    in1=seab[b][:, co,
                                                1 + 512 * nw : 513 + 512 * nw],
                                )
                            # decompose2 of this channel chunk can start now
                            _decompose_chunk(nc, p7s, ysb, sea2, co)

                def layernorm(b):
                    # windowed pipeline; stats borrow conv2's PSUM banks
                    for twi in range(TW):
                        st_s = pc2[2 * (twi % 2)][0:1, :]
                        st_q = pc2[2 * (twi % 2) + 1][0:1, :]
                        for dci in range(DC):
                            sqt = p6s.tile([128, 512], F32, tag="sqt")
                            nc.scalar.activation(
                                sqt[:],
                                sea2[:, dci, 1 + 512 * twi : 513 + 512 * twi],
                                AF.Square,
                            )
                            nc.tensor.matmul(
                                st_s,
                                lhsT=ones_mv[:],
                                rhs=sea2[:, dci, 1 + 512 * twi : 513 + 512 * twi],
                                start=(dci == 0), stop=(dci == DC - 1),
                            )
                            nc.tensor.matmul(
                                st_q,
                                lhsT=ones_mv[:],
                                rhs=sqt[:],
                                start=(dci == 0), stop=(dci == DC - 1),
                            )
                        mu = p7b.tile([1, 512], F32, tag="mu")
                        rs = p7b.tile([1, 512], F32, tag="rs")
                        nc.scalar.activation(mu[:], st_s, AF.Copy)
                        nc.vector.tensor_mul(out=rs[:], in0=mu[:], in1=mu[:])
                        nc.vector.tensor_sub(out=rs[:], in0=st_q, in1=rs[:])
                        nc.vector.tensor_scalar_add(rs[:], rs[:], EPS)
                        nc.vector.reciprocal(out=rs[:], in_=rs[:])
                        nc.scalar.activation(rs[:], rs[:], AF.Sqrt)
                        mub = p7b.tile([128, 512], F32, tag="mub")
                        rsb = p7b.tile([128, 512], F32, tag="rsb")
                        nc.gpsimd.partition_broadcast(mub[:], mu[:])
                        nc.gpsimd.partition_broadcast(rsb[:], rs[:])
                        for dci in range(DC):
                            ve = nc.vector
                            og = p6s.tile([128, 512], F32, tag="og")
                            ve.tensor_sub(
                                out=og[:],
                                in0=sea2[:, dci, 1 + 512 * twi : 513 + 512 * twi],
                                in1=mub[:],
                            )
                            ve.scalar_tensor_tensor(
                                out=og[:], in0=og[:],
                                scalar=lng_c[:, dci : dci + 1], in1=rsb[:],
                                op0=AluOpType.mult, op1=AluOpType.mult,
                            )
                            nc.scalar.activation(
                                og[:], og[:], AF.Identity,
                                bias=lnb_c[:, dci : dci + 1],
                            )
                            nc.scalar.dma_start(
                                out_dm.ap()[b, :, dci,
                                            512 * twi : 512 * twi + 512],
                                og[:],
                            )

                # schedule: conv(b1) fully; LN(b1) sits between batch-0 conv
                # stages so its vector/DMA chains hide under PE work.
                conv1_half(1, 0)
                conv2_pass(1, 0)
                conv1_half(1, 1)
                conv2_pass(1, 1)      # finalize emits decompose2(b1) per chunk
                conv1_half(0, 0)
                layernorm(1)          # stats borrow pc2 (free here)
                conv2_pass(0, 0)
                conv1_half(0, 1)
                conv2_pass(0, 1)      # finalize emits decompose2(b0) per chunk
                layernorm(0)


# ---------------------------------------------------------------------------
# host side
# ---------------------------------------------------------------------------
_CACHE = {}


def _get_nc(n_group: int):
    key = n_group
    if key not in _CACHE:
        nc = bacc.Bacc("TRN2", target_bir_lowering=False, debug=False,
                       num_devices=n_group)
        build(nc, n_group)
        nc.compile()
        _CACHE[key] = nc
    return _CACHE[key]


def stage_inputs(inputs, ncores=NCORES):
    x = np.asarray(inputs["x"], np.float32)
    Wq = np.asarray(inputs["Wq"], np.float32)
    Wk = np.asarray(inputs["Wk"], np.float32)
    Wv = np.asarray(inputs["Wv"], np.float32)
    Wo = np.asarray(inputs["Wo"], np.float32)
    bq = np.asarray(inputs["bq"], np.float32)
    bk = np.asarray(inputs["bk"], np.float32)
    bv = np.asarray(inputs["bv"], np.float32)
    bo = np.asarray(inputs["bo"], np.float32)
    w1 = np.asarray(inputs["conv1_w"], np.float32)
    w2 = np.asarray(inputs["conv2_w"], np.float32)
    lng = np.asarray(inputs["ln_g"], np.float32)
    lnb = np.asarray(inputs["ln_b"], np.float32)

    bop = bo + bv @ Wo
    # merged QK projection: the statistic only needs x^T (Wq Wk^T) x
    # (biases drop: they shift the statistic by a constant, and both topk
    # and softmax are shift-invariant). Wo folds into the V projection.
    Wm = Wq @ Wk.T
    Wvo = Wv @ Wo
    col = lambda v: np.ascontiguousarray(v.reshape(DC, 128).T)
    # projection weights partition-major: W[dci*128+p, n] -> [p, dci*D + n]
    wmaj = lambda W: np.ascontiguousarray(
        W.reshape(DC, 128, D).transpose(1, 0, 2).reshape(128, DC * D)
    ).astype(BF16_NP)
    # same, scaled into the e4m3 normal range
    wmaj8 = lambda W: np.ascontiguousarray(
        np.clip(W * SW, -240, 240)
        .reshape(DC, 128, D).transpose(1, 0, 2).reshape(128, DC * D)
    ).astype(FP8_NP)
    # conv1 [3, D, CF] -> [CFC, 128(ci-part), DC*3*128(co)]
    w1h = np.ascontiguousarray(
        w1.reshape(3, DC, 128, CFC, 128).transpose(3, 2, 1, 0, 4)
    ).reshape(CFC, 128, DC * 3 * 128).astype(BF16_NP)
    # conv2 [3, CF, D] -> [DC, 128(ci-part), CFC*3*128(co)]
    w2h = np.ascontiguousarray(
        w2.reshape(3, CFC, 128, DC, 128).transpose(3, 2, 1, 0, 4)
    ).reshape(DC, 128, CFC * 3 * 128).astype(BF16_NP)

    shared = {
        "wm": wmaj(Wm), "wv": wmaj8(Wvo),
        "bop_t": col(bop),
        "w1h": w1h, "w2h": w2h, "lng_t": col(lng), "lnb_t": col(lnb),
    }
    bpc = B // ncores
    in_maps = []
    for c in range(ncores):
        m = dict(shared)
        # x [bpc, L, D] -> [bpc, 128, DC*L]  (d-major per partition)
        xc = x[bpc * c : bpc * (c + 1)]
        xc = np.ascontiguousarray(
            xc.reshape(bpc, L, DC, 128).transpose(0, 3, 2, 1)
        ).reshape(bpc, 128, DC * L)
        m["x_dm"] = xc
        in_maps.append(m)
    return in_maps


def unstage_output(res, ncores=NCORES):
    out = np.empty((B, L, D), np.float32)
    bpc = B // ncores
    for c in range(ncores):
        o = np.asarray(res.results[c]["out_dm"])  # [bpc, 128, DC, L]
        for i in range(bpc):
            # full[t, dci*128+p] = o[i][p, dci, t]
            out[bpc * c + i] = o[i].transpose(2, 1, 0).reshape(L, D)
    return out


def kernel(**inputs):
    nc = _get_nc(NCORES)
    in_maps = stage_inputs(inputs)
    res = bass_utils.run_bass_kernel_spmd(nc, in_maps, core_ids=list(range(NCORES)))
    return unstage_output(res)



# revision 57
# speedup vs baseline: 1.1792x; 1.0232x over previous
"""Trainium2 Bass kernel for nn_Encoder_78889959293176 (Autoformer-style encoder layer).

Strategy: data-parallel over batch (16 batches -> 8 cores x 2).
All heavy compute on the TensorEngine in a d-major ([channel, time]) layout:
  - the Q/K projections are merged on the host into M = Wq Wk^T (the
    statistic only needs x^T M x; biases shift it by a constant, and both
    topk and softmax are shift-invariant), Wo is folded into the V
    projection (v' = x Wv Wo), so only two projections remain
  - autocorrelation statistic mean_value via q'^T x tiles + a 2-copy
    diagonal "shear" DMA into DRAM + ones-matmul partition reduction
    (flipped-tau space)
  - AllReduce(8 cores) of the batch-summed statistic, on-device top-22
    threshold via gpsimd kth_largest, masked exp in a [16,128] layout
    (unnormalized; 1/Z folds into the agg PSUM drain scale)
  - the rolls-weighted aggregation as an fp8e4 DoubleRow circulant matmul
    of v' (e4m3, time chunks reversed) against a Toeplitz band buffer
    read row-step-2047 from a 129-copy periodic DRAM buffer built by one
    16-packet row write + a 3-way step-0-source DRAM->DRAM blast
  - series decomposition via tensor_tensor_scan cumsum, convs as bf16
    matmuls (fp8 convs/statistic provably break the rel-err budget /
    top-k selection), layernorm stats via ones-matmuls.

Pipeline layout (single pool scopes, no per-batch barriers):
  - per-batch statistic AllReduces issue right after each batch's phase 1;
    batch 1's collective + kth_largest hide under the two V projections.
  - phases 3..7 interleave the two batches so the vector-engine chains
    (decompositions, layernorm) of one batch always overlap TensorEngine
    conv work of the other batch; layernorm stats borrow conv2's PSUM banks.
  - conv2 runs in two channel-half passes (partials parked in bf16 acc2) so
    h1 only holds 8 of 16 hidden chunks.
  - all DRAM operands are host-staged partition-major: every DMA line is
    contiguous per partition.
"""

import numpy as np

import concourse.bass as bass
import concourse.bacc as bacc
import concourse.mybir as mybir
import concourse.tile as tile
from concourse import bass_utils
from concourse import library_config
from concourse.alu_op_type import AluOpType

try:
    import ml_dtypes

    BF16_NP = ml_dtypes.bfloat16
    FP8_NP = ml_dtypes.float8_e4m3
except Exception:  # pragma: no cover
    BF16_NP = np.float32
    FP8_NP = np.float32

F32 = mybir.dt.float32
BF16 = mybir.dt.bfloat16
FP8 = mybir.dt.float8e4
DR = mybir.MatmulPerfMode.DoubleRow
AF = mybir.ActivationFunctionType

# fp8 e4m3 scale factors (folded back out via matmul-drain scales / the
# mean-value reduction constant)
SX = 4.0    # x
SW = 64.0   # wm = Wq Wk^T and wvo = Wv Wo weight matrices
SQ = 16.0   # q' = x (Wq Wk^T)
SV = 16.0   # v' = x (Wv Wo)

B, L, D = 16, 2048, 512
CF = 2048  # conv hidden
TOPK = 22
KER = 25
EPS = 1e-5
SLOPE = 0.01
NCORES = 8
BPC = B // NCORES  # batches per core
DC = D // 128  # 4 d-chunks
CFC = CF // 128  # 16 conv-hidden chunks
CH = CFC // 2  # conv-hidden chunks per half
TW = L // 512  # 4 time windows of 512
TM = L // 128  # 16 time chunks of 128
NEG = -1.0e30


def build(nc: bass.Bass, n_group: int):
    x_dm = nc.dram_tensor("x_dm", [BPC, 128, DC * L], F32, kind="ExternalInput")
    wm_d = nc.dram_tensor("wm", [128, DC * D], BF16, kind="ExternalInput")
    wv_d = nc.dram_tensor("wv", [128, DC * D], FP8, kind="ExternalInput")
    bop_d = nc.dram_tensor("bop_t", [128, DC], F32, kind="ExternalInput")
    w1_d = nc.dram_tensor("w1h", [CFC, 128, DC * 3 * 128], BF16, kind="ExternalInput")
    w2_d = nc.dram_tensor("w2h", [DC, 128, CFC * 3 * 128], BF16, kind="ExternalInput")
    lng_d = nc.dram_tensor("lng_t", [128, DC], F32, kind="ExternalInput")
    lnb_d = nc.dram_tensor("lnb_t", [128, DC], F32, kind="ExternalInput")
    out_dm = nc.dram_tensor("out_dm", [BPC, 128, DC, L], F32, kind="ExternalOutput")

    with tile.TileContext(nc) as tc:
        _body(nc, tc, n_group, x_dm, wm_d, wv_d, bop_d,
              w1_d, w2_d, lng_d, lnb_d, out_dm)
    return nc


def _decompose_chunk(nc, scan_pool, src, dst, dci):
    """dst[:, dci, 1:L+1] = src[:, dci] - movavg_KER(src[:, dci])."""
    half = (KER - 1) // 2
    pad = scan_pool.tile([128, L + KER], F32, tag="scan_pad")
    cs = scan_pool.tile([128, L + KER], F32, tag="scan_cs")
    nc.vector.memset(pad[:, 0:1], 0.0)
    nc.vector.tensor_copy(
        out=pad[:, 1 : 1 + half],
        in_=src[:, dci, 0:1].to_broadcast([128, half]),
    )
    nc.scalar.activation(pad[:, 1 + half : 1 + half + L], src[:, dci, :], AF.Copy)
    nc.vector.tensor_copy(
        out=pad[:, 1 + half + L :],
        in_=src[:, dci, L - 1 : L].to_broadcast([128, half]),
    )
    nc.vector.tensor_tensor_scan(
        out=cs[:], data0=pad[:], data1=pad[:], initial=0.0,
        op0=AluOpType.add, op1=AluOpType.bypass,
    )
    d1 = pad[:, 0:L]  # cumsum done; reuse pad for the boxcar difference
    nc.vector.tensor_sub(out=d1, in0=cs[:, KER:], in1=cs[:, 0:L])
    nc.vector.scalar_tensor_tensor(
        out=dst[:, dci, 1 : L + 1], in0=d1, scalar=-1.0 / KER,
        in1=src[:, dci, :], op0=AluOpType.mult, op1=AluOpType.add,
    )
    nc.vector.tensor_copy(out=dst[:, dci, 0:1], in_=dst[:, dci, 1:2])
    nc.vector.tensor_copy(
        out=dst[:, dci, L + 1 : L + 2], in_=dst[:, dci, L : L + 1]
    )


def _decompose(nc, scan_pool, src, dst):
    for dci in range(DC):
        _decompose_chunk(nc, scan_pool, src, dst, dci)


def _body(nc, tc, n_group, x_dm, wm_d, wv_d, bop_d,
          w1_d, w2_d, lng_d, lnb_d, out_dm):
    with (
        tc.tile_pool(name="p0", bufs=1) as p0,
        tc.tile_pool(name="pp", bufs=1, space="PSUM") as pp,
        tc.tile_pool(name="dr", bufs=1, space="DRAM") as dr,
        tc.tile_pool(name="dr3", bufs=4, space="DRAM") as dr3,
    ):
        nc.gpsimd.load_library(library_config.attn)
        # ----- persistent constants -----
        ones_mv = p0.tile([128, 1], F32, tag="ones_mv")
        nc.vector.memset(ones_mv[:], 1.0 / D)
        ones_bf = p0.tile([128, 1], BF16, tag="ones_bf")
        nc.vector.memset(ones_bf[:], 1.0 / D)
        ones16 = p0.tile([16, 1], BF16, tag="ones16")
        nc.vector.memset(ones16[:], 1.0)
        bop_c = p0.tile([128, DC], F32, tag="bop_c")
        lng_c = p0.tile([128, DC], F32, tag="lng_c")
        lnb_c = p0.tile([128, DC], F32, tag="lnb_c")
        nc.sync.dma_start(bop_c[:], bop_d[:, :])
        nc.sync.dma_start(lng_c[:], lng_d[:, :])
        nc.sync.dma_start(lnb_c[:], lnb_d[:, :])

        # 4 rotating PSUM accumulators shared by all phases
        ps4 = []
        for i in range(4):
            t = pp.tile([128, 512], F32, tag=f"ps{i}", name=f"ps_{i}")
            ps4.append(t)

        hb = {}
        seab = []
        cco = []

        with tc.tile_pool(name="psea", bufs=1) as psea:
            for b in range(BPC):
                t = psea.tile([128, DC, L + 2], BF16, tag=f"seab{b}",
                              name=f"seab_{b}")
                seab.append(t)

            with tc.tile_pool(name="pv", bufs=1) as pv:
                # v' = x (Wv Wo) in e4m3, time chunks stored REVERSED
                # (chunk j = true time chunk TM-1-j) so the DoubleRow agg
                # pairs read gbuf with a positive +128 k-substride.
                v8 = []
                for b in range(BPC):
                    t = pv.tile([128, TM, D], FP8, tag=f"v8_{b}", name=f"v8_{b}")
                    v8.append(t)
                # unnormalized masked-exp weights in [16,128] layout
                # (row r holds stat indices 128r..128r+127, so the hb0 write
                # is 16 contiguous 128B packets); softmax 1/Z folded into
                # the agg PSUM drain via zib.
                g16 = pv.tile([16, 2 * 128], FP8, tag="g16")
                zib = pv.tile([128, 2], F32, tag="zib")
                mvloc = pv.tile([16, 2, 128], F32, tag="mvloc")
                bs16 = pv.tile([16, 2, 128], F32, tag="bs16")

                with tc.tile_pool(name="pxv", bufs=1) as pxv:
                    xb = []
                    for b in range(BPC):
                        t = pxv.tile([128, DC, L], BF16, tag=f"xbt{b}",
                                     name=f"xb_{b}")
                        xb.append(t)
                    mvf = pxv.tile([1, BPC * L], F32, tag="mvf")

                    # ========= phase 1: mean_value (flipped space) =========
                    with (
                        tc.tile_pool(name="ph1", bufs=1) as ph1,
                        tc.tile_pool(name="ph1b", bufs=4) as ph1b,
                        tc.tile_pool(name="ph1w", bufs=3) as ph1w,
                        tc.tile_pool(name="ppm1", bufs=1, space="PSUM") as ppm1,
                    ):
                        wm_s = ph1.tile([128, DC, D], BF16, tag="wqk")
                        nc.sync.dma_start(wm_s[:], wm_d.ap())
                        # fine-grained x loads; bf16 convert (scalar) + scaled
                        # e4m3 convert (vector), both batches
                        for b in range(BPC):
                            for dci in range(DC):
                                for tw in range(TW):
                                    xq = ph1b.tile([128, 512], F32, tag="xq")
                                    o = dci * L + 512 * tw
                                    nc.sync.dma_start(
                                        xq[:], x_dm.ap()[b, :, o : o + 512]
                                    )
                                    nc.scalar.activation(
                                        xb[b][:, dci, 512 * tw : 512 * tw + 512],
                                        xq[:], AF.Copy,
                                    )

                        mv_reg = []
                        for i in range(4):
                            t = ppm1.tile([1, 512], F32, tag=f"mv{i}",
                                          name=f"mv_{i}")
                            mv_reg.append(t)

                        for b in range(BPC):
                            # q' = (Wq Wk^T)^T x (biases provably drop out of
                            # the statistic: rank-1 terms are constant over
                            # tau, and topk+softmax are shift-invariant)
                            q_s = ph1.tile([128, DC, L], BF16, tag="q_s")
                            for dco in range(DC):
                                for dci in range(DC):
                                    for twi in range(TW):
                                        nc.tensor.matmul(
                                            ps4[twi][:],
                                            lhsT=wm_s[:, dci,
                                                      128 * dco : 128 * dco + 128],
                                            rhs=xb[b][:, dci,
                                                      512 * twi : 512 * twi + 512],
                                            start=(dci == 0),
                                            stop=(dci == DC - 1),
                                        )
                                for twi in range(TW):
                                    nc.scalar.activation(
                                        q_s[:, dco, 512 * twi : 512 * twi + 512],
                                        ps4[twi][:], AF.Copy,
                                    )

                            def _emit_mv(A, wa):
                                for cc in range(4):
                                    w0 = (512 * cc + 128 * A) % L
                                    nc.tensor.matmul(
                                        mv_reg[cc][0:1, :],
                                        lhsT=ones_bf[:],
                                        rhs=wa[:, w0 : w0 + 512],
                                        start=(A == 0), stop=(A == TM - 1),
                                    )

                            pend = []
                            for A in range(TM):
                                bufA = dr3.tile([128, 4224], BF16, tag="bufA")
                                for dci in range(DC):
                                    for tB in range(TW):
                                        nc.tensor.matmul(
                                            ps4[tB][:],
                                            lhsT=q_s[:, dci,
                                                     128 * A : 128 * A + 128],
                                            rhs=xb[b][:, dci,
                                                     512 * tB : 512 * tB + 512],
                                            start=(dci == 0),
                                            stop=(dci == DC - 1),
                                        )
                                for tB in range(TW):
                                    c_sb = ph1b.tile([128, 512], BF16, tag="c_sb")
                                    nc.scalar.activation(c_sb[:], ps4[tB][:],
                                                         AF.Copy)
                                    for cp, eng in ((0, nc.sync), (1, nc.scalar)):
                                        dst = bass.AP(
                                            bufA[:].tensor,
                                            127 + 512 * tB + 2048 * cp,
                                            [[4223, 128], [1, 512]],
                                        )
                                        eng.dma_start(dst, c_sb[:])
                                wa = ph1w.tile([128, 2560], BF16, tag="wa")
                                nc.sync.dma_start(
                                    wa[:],
                                    bass.AP(bufA[:].tensor, 128,
                                            [[4224, 128], [1, 2560]]),
                                )
                                pend.append((A, wa))
                                if len(pend) > 2:
                                    _emit_mv(*pend.pop(0))
                            for a_w in pend:
                                _emit_mv(*a_w)
                            for cc in range(4):
                                nc.scalar.activation(
                                    mvf[0:1,
                                        L * b + 512 * cc : L * b + 512 * cc + 512],
                                    mv_reg[cc][0:1, :], AF.Copy,
                                )
                            # issue this batch's AllReduce immediately: batch
                            # 0's collective hides under batch 1's phase 1.
                            cci = dr.tile([1, L], F32, tag=f"cci{b}")
                            cc_o = dr.tile([1, L], F32, tag=f"cco{b}")
                            nc.sync.dma_start(cci[:], mvf[0:1, L * b : L * b + L])
                            nc.gpsimd.collective_compute(
                                "AllReduce", AluOpType.add,
                                replica_groups=[list(range(n_group))],
                                ins=[cci[:].opt()], outs=[cc_o[:].opt()],
                            )
                            cco.append(cc_o)
                            # local + summed statistic back in [16,128]
                            # layout for the masked-exp path (off critical
                            # path; 16 contiguous packets each)
                            nc.gpsimd.dma_start(
                                mvloc[:, b, :],
                                bass.AP(cci[:].tensor, cci[:].offset,
                                        [[128, 16], [1, 128]]),
                            )
                            nc.gpsimd.dma_start(
                                bs16[:, b, :],
                                bass.AP(cc_o[:].tensor, cc_o[:].offset,
                                        [[128, 16], [1, 128]]),
                            )

                    # ========= phase 2: topk + softmax (both batches) ========
                    with (
                        tc.tile_pool(name="ph2", bufs=1) as ph2,
                        tc.tile_pool(name="ph2w", bufs=1) as ph2w,
                    ):
                        wv_s = ph2w.tile([128, DC, D], FP8, tag="wv_s")
                        nc.sync.dma_start(wv_s[:], wv_d.ap())

                        def _vproj(b):
                            # bf16 x against fp8 Wvo; psum = SW * v'
                            for tm in range(TM):
                                pt = ps4[tm % 4]
                                for dci in range(DC):
                                    nc.tensor.matmul(
                                        pt[:],
                                        lhsT=xb[b][:, dci,
                                                   128 * tm : 128 * tm + 128],
                                        rhs=wv_s[:, dci, :],
                                        start=(dci == 0),
                                        stop=(dci == DC - 1),
                                    )
                                # reversed chunk order for the DoubleRow agg
                                nc.scalar.activation(
                                    v8[b][:, TM - 1 - tm, :], pt[:], AF.Copy,
                                    scale=SV / SW,
                                )

                        _vproj(0)  # hides the second collective
                        # top-22 threshold via gpsimd kth_largest: the lerped
                        # quantile at rank 21.5 lies strictly between the 22nd
                        # and 23rd largest values -> tie-free >= mask.
                        bsA = ph2.tile([128, 16], F32, tag="bsA")
                        bsB = ph2.tile([128, 16], F32, tag="bsB")
                        nc.sync.dma_start(
                            bsA[:], bass.AP(cco[0][:].tensor, 0, [[16, 128], [1, 16]])
                        )
                        nc.sync.dma_start(
                            bsB[:], bass.AP(cco[1][:].tensor, 0, [[16, 128], [1, 16]])
                        )
                        bs128 = ph2.tile([128, 16], F32, tag="bs128")
                        nc.vector.tensor_add(out=bs128[:], in0=bsA[:], in1=bsB[:])
                        kth = ph2.tile([1, 2], F32, tag="kth")
                        nc.gpsimd.kth_largest(
                            kth[:], bs128[:], 16, 24,
                            quantile=1.0 - (TOPK - 0.5) / (L - 1),
                        )
                        _vproj(1)  # hides kth_largest
                        # masked exp (unnormalized) on [16,128] per batch;
                        # normalization deferred to the agg PSUM drain.
                        thr16 = ph2.tile([16, 1], F32, tag="thr16")
                        nc.gpsimd.partition_broadcast(thr16[:], kth[0:1, 0:1],
                                                      channels=16)
                        bsum16 = ph2.tile([16, 128], F32, tag="bsum16")
                        nc.vector.tensor_add(out=bsum16[:], in0=bs16[:, 0, :],
                                             in1=bs16[:, 1, :])
                        mask16 = ph2.tile([16, 128], F32, tag="mask16")
                        nc.vector.tensor_scalar(
                            out=mask16[:], in0=bsum16[:], scalar1=thr16[:, 0:1],
                            scalar2=None, op0=AluOpType.is_ge,
                        )
                        # stt = stat*mask + (mask-1)*1e9: selected entries keep
                        # the exact statistic (no 1e9 roundtrip -- f32 at 1e9
                        # has quantum 64, which would wipe out the values).
                        neg916 = ph2.tile([16, 128], F32, tag="neg916")
                        nc.vector.tensor_scalar(
                            out=neg916[:], in0=mask16[:], scalar1=1.0,
                            scalar2=1.0e9,
                            op0=AluOpType.subtract, op1=AluOpType.mult,
                        )
                        stt = ph2.tile([16, 2 * 128], F32, tag="stt")
                        for b in range(BPC):
                            sl = stt[:, 128 * b : 128 * b + 128]
                            nc.vector.tensor_mul(
                                out=sl, in0=mvloc[:, b, :], in1=mask16[:],
                            )
                            nc.vector.tensor_add(out=sl, in0=sl, in1=neg916[:])
                        nc.scalar.activation(g16[:], stt[:], AF.Exp)
                        # periodic replication B[q] = g_f[q mod L]: 16-packet
                        # row write + 3-way step-0-source DRAM->DRAM blast
                        # (the row-step-2047 gbuf read needs 129 copies).
                        for b, eng in ((1, nc.sync), (0, nc.scalar)):
                            # two g copies up front so the blast replicates
                            # 2L units (half the packets)
                            hb0 = dr.tile([1, 2 * L], FP8, tag=f"hb0{b}")
                            for rep in range(2):
                                eng.dma_start(
                                    bass.AP(hb0[:].tensor,
                                            hb0[:].offset + rep * L,
                                            [[128, 16], [1, 128]]),
                                    g16[:, 128 * b : 128 * b + 128],
                                )
                            hbb = dr.tile([1, 130 * L], FP8, tag=f"hb{b}")
                            hb[b] = hbb
                            for (r0, r1), e2 in zip(
                                ((0, 22), (22, 44), (44, 65)),
                                (nc.sync, nc.scalar, nc.gpsimd),
                            ):
                                e2.dma_start(
                                    bass.AP(hbb[:].tensor,
                                            hbb[:].offset + 2 * L * r0,
                                            [[2 * L, r1 - r0], [1, 2 * L]]),
                                    bass.AP(hb0[:].tensor, hb0[:].offset,
                                            [[0, r1 - r0], [1, 2 * L]]),
                                )
                        # softmax normalizers 1/Z per batch (off critical path)
                        nc.tensor.matmul(ps4[0][0:1, 0:256], lhsT=ones16[:],
                                         rhs=g16[:], start=True, stop=True)
                        zrow = ph2.tile([1, 256], F32, tag="zrow")
                        nc.scalar.activation(zrow[:], ps4[0][0:1, 0:256],
                                             AF.Copy)
                        z2 = ph2.tile([1, 2], F32, tag="z2")
                        ztmp = ph2.tile([1, 256], F32, tag="ztmp")
                        for b in range(BPC):
                            nc.scalar.activation(
                                ztmp[0:1, 128 * b : 128 * b + 128],
                                zrow[0:1, 128 * b : 128 * b + 128], AF.Copy,
                                accum_out=z2[0:1, b : b + 1],
                            )
                        nc.vector.reciprocal(out=z2[:], in_=z2[:])
                        # agg psum carries sum g~ * (SV v'); fold 1/SV here
                        nc.vector.tensor_scalar(
                            out=z2[:], in0=z2[:], scalar1=1.0 / SV,
                            scalar2=None, op0=AluOpType.mult,
                        )
                        nc.gpsimd.partition_broadcast(zib[:], z2[0:1, :])

                # == phases 3-4, batch order b1 then b0 (decomp overlaps PE) ==
                with (
                    tc.tile_pool(name="p3", bufs=1) as p3,
                    tc.tile_pool(name="p3r", bufs=2) as p3r,
                    tc.tile_pool(name="p3s", bufs=1) as p3s,
                ):
                    gbufs = {}
                    # b1 (needed first) on all 3 queues; b0 on scalar/gpsimd
                    # only so the sync queue stays free for the xr loads that
                    # pace the b1 acx drains.
                    for b, split in (
                        (1, (((0, 43), nc.sync), ((43, 86), nc.scalar),
                             ((86, 128), nc.gpsimd))),
                        (0, (((0, 64), nc.scalar), ((64, 128), nc.gpsimd))),
                    ):
                        gbuf = p3.tile([128, 3968], FP8, tag=f"gbuf{b}")
                        gbufs[b] = gbuf
                        for (i0, i1), eng in split:
                            eng.dma_start(
                                gbuf[i0:i1, :],
                                bass.AP(hb[b][:].tensor,
                                        hb[b][:].offset + 127 + 2047 * i0,
                                        [[2047, i1 - i0], [1, 3968]]),
                            )
                    for b in (1, 0):
                        gbuf = gbufs[b]

                        def gpair(Bp, nw):
                            # [128, 2, 512] gbuf k-pair: v8 chunk j=2Bp is
                            # true time chunk TM-1-2Bp -> base column
                            # 512nw + 256Bp, +128 for the second k-subtile
                            gs = gbuf[:]
                            return bass.AP(
                                gs.tensor, gs.offset + 512 * nw + 256 * Bp,
                                [list(gs.ap[0]), [128, 2], [1, 512]],
                            )

                        # ac = sum_k g_k roll(v') + bop + x, drained straight
                        # to acx (Wo is folded into v' = x Wv Wo on the host);
                        # per-batch tiles so b0's drains don't WAR-stall on
                        # b1's decompose reads
                        acx = p3.tile([128, DC, L], F32, tag=f"acx{b}")
                        for dm in range(DC):
                            for Bp in range(TM // 2):
                                for nw in range(TW):
                                    nc.tensor.matmul(
                                        ps4[nw][:],
                                        lhsT=v8[b][:, 2 * Bp : 2 * Bp + 2,
                                                   128 * dm : 128 * dm + 128],
                                        rhs=gpair(Bp, nw),
                                        start=(Bp == 0),
                                        stop=(Bp == TM // 2 - 1),
                                        perf_mode=DR,
                                    )
                            for nw in range(TW):
                                xr = p3r.tile([128, 512], F32, tag="xr")
                                nc.sync.dma_start(
                                    xr[:],
                                    x_dm.ap()[b, :,
                                              dm * L + 512 * nw :
                                              dm * L + 512 * nw + 512],
                                )
                                act = p3r.tile([128, 512], F32, tag="act")
                                nc.scalar.activation(
                                    act[:], ps4[nw][:], AF.Identity,
                                    scale=zib[:, b : b + 1],
                                    bias=bop_c[:, dm : dm + 1],
                                )
                                nc.vector.tensor_add(
                                    out=acx[:, dm, 512 * nw : 512 * nw + 512],
                                    in0=act[:], in1=xr[:],
                                )
                            # this channel chunk of acx is complete: its
                            # decomposition scan can overlap the next chunks'
                            # matmuls (and the following conv stages)
                            _decompose_chunk(nc, p3s, acx, seab[b], dm)

            # ======= phases 5-7, interleaved across the two batches =======
            with (
                tc.tile_pool(name="pcv", bufs=1) as pcv,
                tc.tile_pool(name="p5w1", bufs=2) as p5w1,
                tc.tile_pool(name="p5w2", bufs=2) as p5w2,
                tc.tile_pool(name="p6s", bufs=2) as p6s,
                tc.tile_pool(name="p7s", bufs=1) as p7s,
                tc.tile_pool(name="p7b", bufs=1) as p7b,
                tc.tile_pool(name="p7d", bufs=2, space="DRAM") as p7d,
                tc.tile_pool(name="ppc2", bufs=1, space="PSUM") as ppc2,
            ):
                pc2 = []
                for i in range(4):
                    t = ppc2.tile([128, 512], F32, tag=f"pc2_{i}", name=f"pc2_{i}")
                    pc2.append(t)
                h1 = pcv.tile([128, CH, L + 2], BF16, tag="h1")
                acc2 = pcv.tile([128, DC, L], BF16, tag="acc2")
                ysb = pcv.tile([128, DC, L], F32, tag="ysb")
                sea2 = pcv.tile([128, DC, L + 2], F32, tag="sea2")

                def conv1_half(b, half):
                    for co8 in range(CH):
                        co = CH * half + co8
                        w1t = p5w1.tile([128, DC * 3 * 128], BF16, tag="w1t")
                        nc.sync.dma_start(w1t[:], w1_d.ap()[co])
                        first = True
                        for dci in range(DC):
                            for tap in range(3):
                                ki = (3 * dci + tap) * 128
                                for nw in range(TW):
                                    nc.tensor.matmul(
                                        ps4[nw][:],
                                        lhsT=w1t[:, ki : ki + 128],
                                        rhs=seab[b][:, dci,
                                             512 * nw + tap : 512 * nw + tap + 512],
                                        start=first,
                                        stop=(dci == DC - 1 and tap == 2),
                                    )
                                first = False
                        for nw in range(TW):
                            nc.scalar.activation(
                                h1[:, co8, 1 + 512 * nw : 513 + 512 * nw],
                                ps4[nw][:], AF.Prelu, alpha=SLOPE,
                            )
                        nc.vector.tensor_copy(out=h1[:, co8, 0:1],
                                              in_=h1[:, co8, 1:2])
                        nc.vector.tensor_copy(out=h1[:, co8, L + 1 : L + 2],
                                              in_=h1[:, co8, L : L + 1])

                def conv2_pass(b, half):
                    for co in range(DC):
                        w2t = p5w2.tile([128, CH * 3 * 128], BF16, tag="w2t")
                        src = w2_d.ap()[co]
                        ofs = half * CH * 3 * 128
                        nc.sync.dma_start(
                            w2t[:],
                            bass.AP(src.tensor, src.offset + ofs,
                                    [[CFC * 3 * 128, 128], [1, CH * 3 * 128]]),
                        )
                        first = True
                        for ci8 in range(CH):
                            for tap in range(3):
                                ki = (3 * ci8 + tap) * 128
                                for nw in range(TW):
                                    nc.tensor.matmul(
                                        pc2[nw][:],
                                        lhsT=w2t[:, ki : ki + 128],
                                        rhs=h1[:, ci8,
                                             512 * nw + tap : 512 * nw + tap + 512],
                                        start=first,
                                        stop=(ci8 == CH - 1 and tap == 2),
                                    )
                                first = False
                        if half == 0:
                            for nw in range(TW):
                                nc.scalar.activation(
                                    acc2[:, co, 512 * nw : 512 * nw + 512],
                                    pc2[nw][:], AF.Copy,
                                )
                        else:
                            for nw in range(TW):
                                h2t = p6s.tile([128, 512], F32, tag="h2t")
                                nc.vector.tensor_add(
                                    out=h2t[:], in0=pc2[nw][:],
                                    in1=acc2[:, co, 512 * nw : 512 * nw + 512],
                                )
                                h2r = p6s.tile([128, 512], F32, tag="h2r")
                                nc.scalar.activation(h2r[:], h2t[:], AF.Prelu,
                                                     alpha=SLOPE)
                                nc.vector.tensor_add(
                                    out=ysb[:, co, 512 * nw : 512 * nw + 512],
                                    in0=h2r[:],
                                    in1=seab[b][:, co,
                                                1 + 512 * nw : 513 + 512 * nw],
                                )
                            # decompose2 of this channel chunk can start now
                            _decompose_chunk(nc, p7s, ysb, sea2, co)

                def layernorm(b):
                    # windowed pipeline; stats borrow conv2's PSUM banks
                    for twi in range(TW):
                        st_s = pc2[2 * (twi % 2)][0:1, :]
                        st_q = pc2[2 * (twi % 2) + 1][0:1, :]
                        for dci in range(DC):
                            sqt = p6s.tile([128, 512], F32, tag="sqt")
                            nc.scalar.activation(
                                sqt[:],
                                sea2[:, dci, 1 + 512 * twi : 513 + 512 * twi],
                                AF.Square,
                            )
                            nc.tensor.matmul(
                                st_s,
                                lhsT=ones_mv[:],
                                rhs=sea2[:, dci, 1 + 512 * twi : 513 + 512 * twi],
                                start=(dci == 0), stop=(dci == DC - 1),
                            )
                            nc.tensor.matmul(
                                st_q,
                                lhsT=ones_mv[:],
                                rhs=sqt[:],
                                start=(dci == 0), stop=(dci == DC - 1),
                            )
                        mu = p7b.tile([1, 512], F32, tag="mu")
                        rs = p7b.tile([1, 512], F32, tag="rs")
                        nc.scalar.activation(mu[:], st_s, AF.Copy)
                        nc.vector.tensor_mul(out=rs[:], in0=mu[:], in1=mu[:])
                        nc.vector.tensor_sub(out=rs[:], in0=st_q, in1=rs[:])
                        nc.vector.tensor_scalar_add(rs[:], rs[:], EPS)
                        nc.vector.reciprocal(out=rs[:], in_=rs[:])
                        nc.scalar.activation(rs[:], rs[:], AF.Sqrt)
                        mub = p7b.tile([128, 512], F32, tag="mub")
                        rsb = p7b.tile([128, 512], F32, tag="rsb")
                        nc.gpsimd.partition_broadcast(mub[:], mu[:])
                        nc.gpsimd.partition_broadcast(rsb[:], rs[:])
                        for dci in range(DC):
                            ve = nc.vector
                            og = p6s.tile([128, 512], F32, tag="og")
                            ve.tensor_sub(
                                out=og[:],
                                in0=sea2[:, dci, 1 + 512 * twi : 513 + 512 * twi],
                                in1=mub[:],
                            )
                            ve.scalar_tensor_tensor(
                                out=og[:], in0=og[:],
                                scalar=lng_c[:, dci : dci + 1], in1=rsb[:],
                                op0=AluOpType.mult, op1=AluOpType.mult,
                            )
                            nc.scalar.activation(
                                og[:], og[:], AF.Identity,
                                bias=lnb_c[:, dci : dci + 1],
                            )
                            nc.scalar.dma_start(
                                out_dm.ap()[b, :, dci,
                                            512 * twi : 512 * twi + 512],
                                og[:],
                            )

                # schedule: conv(b1) fully; LN(b1) sits between batch-0 conv
                # stages so its vector/DMA chains hide under PE work.
                conv1_half(1, 0)
                conv2_pass(1, 0)
                conv1_half(1, 1)
                conv2_pass(1, 1)      # finalize emits decompose2(b1) per chunk
                conv1_half(0, 0)
                layernorm(1)          # stats borrow pc2 (free here)
                conv2_pass(0, 0)
                conv1_half(0, 1)
                conv2_pass(0, 1)      # finalize emits decompose2(b0) per chunk
                layernorm(0)


# ---------------------------------------------------------------------------
# host side
# ---------------------------------------------------------------------------
_CACHE = {}


def _get_nc(n_group: int):
    key = n_group
    if key not in _CACHE:
        nc = bacc.Bacc("TRN2", target_bir_lowering=False, debug=False,
                       num_devices=n_group)
        build(nc, n_group)
        nc.compile()
        _CACHE[key] = nc
    return _CACHE[key]


def stage_inputs(inputs, ncores=NCORES):
    x = np.asarray(inputs["x"], np.float32)
    Wq = np.asarray(inputs["Wq"], np.float32)
    Wk = np.asarray(inputs["Wk"], np.float32)
    Wv = np.asarray(inputs["Wv"], np.float32)
    Wo = np.asarray(inputs["Wo"], np.float32)
    bq = np.asarray(inputs["bq"], np.float32)
    bk = np.asarray(inputs["bk"], np.float32)
    bv = np.asarray(inputs["bv"], np.float32)
    bo = np.asarray(inputs["bo"], np.float32)
    w1 = np.asarray(inputs["conv1_w"], np.float32)
    w2 = np.asarray(inputs["conv2_w"], np.float32)
    lng = np.asarray(inputs["ln_g"], np.float32)
    lnb = np.asarray(inputs["ln_b"], np.float32)

    bop = bo + bv @ Wo
    # merged QK projection: the statistic only needs x^T (Wq Wk^T) x
    # (biases drop: they shift the statistic by a constant, and both topk
    # and softmax are shift-invariant). Wo folds into the V projection.
    Wm = Wq @ Wk.T
    Wvo = Wv @ Wo
    col = lambda v: np.ascontiguousarray(v.reshape(DC, 128).T)
    # projection weights partition-major: W[dci*128+p, n] -> [p, dci*D + n]
    wmaj = lambda W: np.ascontiguousarray(
        W.reshape(DC, 128, D).transpose(1, 0, 2).reshape(128, DC * D)
    ).astype(BF16_NP)
    # same, scaled into the e4m3 normal range
    wmaj8 = lambda W: np.ascontiguousarray(
        np.clip(W * SW, -240, 240)
        .reshape(DC, 128, D).transpose(1, 0, 2).reshape(128, DC * D)
    ).astype(FP8_NP)
    # conv1 [3, D, CF] -> [CFC, 128(ci-part), DC*3*128(co)]
    w1h = np.ascontiguousarray(
        w1.reshape(3, DC, 128, CFC, 128).transpose(3, 2, 1, 0, 4)
    ).reshape(CFC, 128, DC * 3 * 128).astype(BF16_NP)
    # conv2 [3, CF, D] -> [DC, 128(ci-part), CFC*3*128(co)]
    w2h = np.ascontiguousarray(
        w2.reshape(3, CFC, 128, DC, 128).transpose(3, 2, 1, 0, 4)
    ).reshape(DC, 128, CFC * 3 * 128).astype(BF16_NP)

    shared = {
        "wm": wmaj(Wm), "wv": wmaj8(Wvo),
        "bop_t": col(bop),
        "w1h": w1h, "w2h": w2h, "lng_t": col(lng), "lnb_t": col(lnb),
    }
    bpc = B // ncores
    in_maps = []
    for c in range(ncores):
        m = dict(shared)
        # x [bpc, L, D] -> [bpc, 128, DC*L]  (d-major per partition)
        xc = x[bpc * c : bpc * (c + 1)]
        xc = np.ascontiguousarray(
            xc.reshape(bpc, L, DC, 128).transpose(0, 3, 2, 1)
        ).reshape(bpc, 128, DC * L)
        m["x_dm"] = xc
        in_maps.append(m)
    return in_maps


def unstage_output(res, ncores=NCORES):
    out = np.empty((B, L, D), np.float32)
    bpc = B // ncores
    for c in range(ncores):
        o = np.asarray(res.results[c]["out_dm"])  # [bpc, 128, DC, L]
        for i in range(bpc):
            # full[t, dci*128+p] = o[i][p, dci, t]
            out[bpc * c + i] = o[i].transpose(2, 1, 0).reshape(L, D)
    return out


def kernel(**inputs):
    nc = _get_nc(NCORES)
    in_maps = stage_inputs(inputs)
    res = bass_utils.run_bass_kernel_spmd(nc, in_maps, core_ids=list(range(NCORES)))
    return unstage_output(res)



# revision 60
# speedup vs baseline: 1.2233x; 1.0374x over previous
"""Trainium2 Bass kernel for nn_Encoder_78889959293176 (Autoformer-style encoder layer).

Strategy: data-parallel over batch (16 batches -> 8 cores x 2).
All heavy compute on the TensorEngine in a d-major ([channel, time]) layout:
  - the Q/K projections are merged on the host into M = Wq Wk^T (the
    statistic only needs x^T M x; biases shift it by a constant, and both
    topk and softmax are shift-invariant), Wo is folded into the V
    projection (v' = x Wv Wo), so only two projections remain
  - autocorrelation statistic mean_value via q'^T x tiles + a 2-copy
    diagonal "shear" DMA into DRAM + ones-matmul partition reduction
    (flipped-tau space)
  - AllReduce(8 cores) of the batch-summed statistic, on-device top-22
    threshold via gpsimd kth_largest, masked exp in a [16,128] layout
    (unnormalized; 1/Z folds into the agg PSUM drain scale)
  - the rolls-weighted aggregation as an fp8e4 DoubleRow circulant matmul
    of v' (e4m3, time chunks reversed) against a Toeplitz band buffer
    read row-step-2047 from a 129-copy periodic DRAM buffer built by one
    16-packet row write + a 3-way step-0-source DRAM->DRAM blast
  - series decomposition via tensor_tensor_scan cumsum, convs as bf16
    matmuls (fp8 convs/statistic provably break the rel-err budget /
    top-k selection), layernorm stats via ones-matmuls.

Pipeline layout (single pool scopes, no per-batch barriers):
  - per-batch statistic AllReduces issue right after each batch's phase 1;
    batch 1's collective + kth_largest hide under the two V projections.
  - phases 3..7 interleave the two batches so the vector-engine chains
    (decompositions, layernorm) of one batch always overlap TensorEngine
    conv work of the other batch; layernorm stats borrow conv2's PSUM banks.
  - conv2 runs in two channel-half passes (partials parked in bf16 acc2) so
    h1 only holds 8 of 16 hidden chunks.
  - all DRAM operands are host-staged partition-major: every DMA line is
    contiguous per partition.
"""

import numpy as np

import concourse.bass as bass
import concourse.bacc as bacc
import concourse.mybir as mybir
import concourse.tile as tile
from concourse import bass_utils
from concourse import library_config
from concourse.alu_op_type import AluOpType

try:
    import ml_dtypes

    BF16_NP = ml_dtypes.bfloat16
    FP8_NP = ml_dtypes.float8_e4m3
except Exception:  # pragma: no cover
    BF16_NP = np.float32
    FP8_NP = np.float32

F32 = mybir.dt.float32
BF16 = mybir.dt.bfloat16
FP8 = mybir.dt.float8e4
DR = mybir.MatmulPerfMode.DoubleRow
AF = mybir.ActivationFunctionType

# fp8 e4m3 scale factors (folded back out via matmul-drain scales / the
# mean-value reduction constant)
SX = 4.0    # x
SW = 64.0   # wm = Wq Wk^T and wvo = Wv Wo weight matrices
SQ = 16.0   # q' = x (Wq Wk^T)
SV = 16.0   # v' = x (Wv Wo)

B, L, D = 16, 2048, 512
CF = 2048  # conv hidden
TOPK = 22
KER = 25
EPS = 1e-5
SLOPE = 0.01
NCORES = 8
BPC = B // NCORES  # batches per core
DC = D // 128  # 4 d-chunks
CFC = CF // 128  # 16 conv-hidden chunks
CH = CFC // 2  # conv-hidden chunks per half
TW = L // 512  # 4 time windows of 512
TM = L // 128  # 16 time chunks of 128
NEG = -1.0e30


def build(nc: bass.Bass, n_group: int):
    x_dm = nc.dram_tensor("x_dm", [BPC, 128, DC * L], F32, kind="ExternalInput")
    wm_d = nc.dram_tensor("wm", [128, DC * D], BF16, kind="ExternalInput")
    wv_d = nc.dram_tensor("wv", [128, DC * D], FP8, kind="ExternalInput")
    bop_d = nc.dram_tensor("bop_t", [128, DC], F32, kind="ExternalInput")
    w1_d = nc.dram_tensor("w1h", [CFC, 128, DC * 3 * 128], BF16, kind="ExternalInput")
    w2_d = nc.dram_tensor("w2h", [DC, 128, CFC * 3 * 128], BF16, kind="ExternalInput")
    lng_d = nc.dram_tensor("lng_t", [128, DC], F32, kind="ExternalInput")
    lnb_d = nc.dram_tensor("lnb_t", [128, DC], F32, kind="ExternalInput")
    out_dm = nc.dram_tensor("out_dm", [BPC, 128, DC, L], F32, kind="ExternalOutput")

    with tile.TileContext(nc) as tc:
        _body(nc, tc, n_group, x_dm, wm_d, wv_d, bop_d,
              w1_d, w2_d, lng_d, lnb_d, out_dm)
    return nc


def _decompose_chunk(nc, scan_pool, src, dst, dci):
    """dst[:, dci, 1:L+1] = src[:, dci] - movavg_KER(src[:, dci])."""
    half = (KER - 1) // 2
    pad = scan_pool.tile([128, L + KER], F32, tag="scan_pad")
    cs = scan_pool.tile([128, L + KER], F32, tag="scan_cs")
    nc.vector.memset(pad[:, 0:1], 0.0)
    nc.vector.tensor_copy(
        out=pad[:, 1 : 1 + half],
        in_=src[:, dci, 0:1].to_broadcast([128, half]),
    )
    nc.scalar.activation(pad[:, 1 + half : 1 + half + L], src[:, dci, :], AF.Copy)
    nc.vector.tensor_copy(
        out=pad[:, 1 + half + L :],
        in_=src[:, dci, L - 1 : L].to_broadcast([128, half]),
    )
    nc.vector.tensor_tensor_scan(
        out=cs[:], data0=pad[:], data1=pad[:], initial=0.0,
        op0=AluOpType.add, op1=AluOpType.bypass,
    )
    d1 = pad[:, 0:L]  # cumsum done; reuse pad for the boxcar difference
    nc.vector.tensor_sub(out=d1, in0=cs[:, KER:], in1=cs[:, 0:L])
    nc.vector.scalar_tensor_tensor(
        out=dst[:, dci, 1 : L + 1], in0=d1, scalar=-1.0 / KER,
        in1=src[:, dci, :], op0=AluOpType.mult, op1=AluOpType.add,
    )
    nc.vector.tensor_copy(out=dst[:, dci, 0:1], in_=dst[:, dci, 1:2])
    nc.vector.tensor_copy(
        out=dst[:, dci, L + 1 : L + 2], in_=dst[:, dci, L : L + 1]
    )


def _decompose(nc, scan_pool, src, dst):
    for dci in range(DC):
        _decompose_chunk(nc, scan_pool, src, dst, dci)


def _body(nc, tc, n_group, x_dm, wm_d, wv_d, bop_d,
          w1_d, w2_d, lng_d, lnb_d, out_dm):
    with (
        tc.tile_pool(name="p0", bufs=1) as p0,
        tc.tile_pool(name="pp", bufs=1, space="PSUM") as pp,
        tc.tile_pool(name="dr", bufs=1, space="DRAM") as dr,
        tc.tile_pool(name="dr3", bufs=4, space="DRAM") as dr3,
    ):
        nc.gpsimd.load_library(library_config.attn)
        # ----- persistent constants -----
        ones_mv = p0.tile([128, 1], F32, tag="ones_mv")
        nc.vector.memset(ones_mv[:], 1.0 / D)
        ones_bf = p0.tile([128, 1], BF16, tag="ones_bf")
        nc.vector.memset(ones_bf[:], 1.0 / D)
        ones16 = p0.tile([16, 1], BF16, tag="ones16")
        nc.vector.memset(ones16[:], 1.0)
        bop_c = p0.tile([128, DC], F32, tag="bop_c")
        lng_c = p0.tile([128, DC], F32, tag="lng_c")
        lnb_c = p0.tile([128, DC], F32, tag="lnb_c")
        nc.sync.dma_start(bop_c[:], bop_d[:, :])
        nc.sync.dma_start(lng_c[:], lng_d[:, :])
        nc.sync.dma_start(lnb_c[:], lnb_d[:, :])

        # 4 rotating PSUM accumulators shared by all phases
        ps4 = []
        for i in range(4):
            t = pp.tile([128, 512], F32, tag=f"ps{i}", name=f"ps_{i}")
            ps4.append(t)

        hb = {}
        seab = []
        cco = []

        with tc.tile_pool(name="psea", bufs=1) as psea:
            for b in range(BPC):
                t = psea.tile([128, DC, L + 2], BF16, tag=f"seab{b}",
                              name=f"seab_{b}")
                seab.append(t)

            with tc.tile_pool(name="pv", bufs=1) as pv:
                # v' = x (Wv Wo) in e4m3, time chunks stored REVERSED
                # (chunk j = true time chunk TM-1-j) so the DoubleRow agg
                # pairs read gbuf with a positive +128 k-substride.
                v8 = []
                for b in range(BPC):
                    t = pv.tile([128, TM, D], FP8, tag=f"v8_{b}", name=f"v8_{b}")
                    v8.append(t)
                # unnormalized masked-exp weights in [16,128] layout
                # (row r holds stat indices 128r..128r+127, so the hb0 write
                # is 16 contiguous 128B packets); softmax 1/Z folded into
                # the agg PSUM drain via zib.
                g16 = pv.tile([16, 2 * 128], FP8, tag="g16")
                zib = pv.tile([128, 2], F32, tag="zib")
                mvloc = pv.tile([16, 2, 128], F32, tag="mvloc")
                bs16 = pv.tile([16, 2, 128], F32, tag="bs16")

                with tc.tile_pool(name="pxv", bufs=1) as pxv:
                    xb = []
                    for b in range(BPC):
                        t = pxv.tile([128, DC, L], BF16, tag=f"xbt{b}",
                                     name=f"xb_{b}")
                        xb.append(t)
                    mvf = pxv.tile([1, BPC * L], F32, tag="mvf")

                    # ========= phase 1: mean_value (flipped space) =========
                    with (
                        tc.tile_pool(name="ph1", bufs=1) as ph1,
                        tc.tile_pool(name="ph1b", bufs=4) as ph1b,
                        tc.tile_pool(name="ph1w", bufs=3) as ph1w,
                        tc.tile_pool(name="ppm1", bufs=1, space="PSUM") as ppm1,
                    ):
                        wm_s = ph1.tile([128, DC, D], BF16, tag="wqk")
                        nc.sync.dma_start(wm_s[:], wm_d.ap())
                        # x loads in [128, L] chunks (one 4KB packet per
                        # partition row) + bf16 converts, both batches
                        for b in range(BPC):
                            for dci in range(DC):
                                xq = ph1b.tile([128, L], F32, tag="xq")
                                o = dci * L
                                nc.sync.dma_start(
                                    xq[:], x_dm.ap()[b, :, o : o + L]
                                )
                                nc.scalar.activation(
                                    xb[b][:, dci, :], xq[:], AF.Copy,
                                )

                        mv_reg = []
                        for i in range(4):
                            t = ppm1.tile([1, 512], F32, tag=f"mv{i}",
                                          name=f"mv_{i}")
                            mv_reg.append(t)

                        for b in range(BPC):
                            # q' = (Wq Wk^T)^T x (biases provably drop out of
                            # the statistic: rank-1 terms are constant over
                            # tau, and topk+softmax are shift-invariant)
                            q_s = ph1.tile([128, DC, L], BF16, tag="q_s")
                            for dco in range(DC):
                                for dci in range(DC):
                                    for twi in range(TW):
                                        nc.tensor.matmul(
                                            ps4[twi][:],
                                            lhsT=wm_s[:, dci,
                                                      128 * dco : 128 * dco + 128],
                                            rhs=xb[b][:, dci,
                                                      512 * twi : 512 * twi + 512],
                                            start=(dci == 0),
                                            stop=(dci == DC - 1),
                                        )
                                for twi in range(TW):
                                    nc.scalar.activation(
                                        q_s[:, dco, 512 * twi : 512 * twi + 512],
                                        ps4[twi][:], AF.Copy,
                                    )

                            def _emit_mv(A, wa):
                                for cc in range(4):
                                    w0 = (512 * cc + 128 * A) % L
                                    nc.tensor.matmul(
                                        mv_reg[cc][0:1, :],
                                        lhsT=ones_bf[:],
                                        rhs=wa[:, w0 : w0 + 512],
                                        start=(A == 0), stop=(A == TM - 1),
                                    )

                            pend = []
                            for A in range(TM):
                                bufA = dr3.tile([128, 2688], BF16, tag="bufA")
                                for dci in range(DC):
                                    for tB in range(TW):
                                        nc.tensor.matmul(
                                            ps4[tB][:],
                                            lhsT=q_s[:, dci,
                                                     128 * A : 128 * A + 128],
                                            rhs=xb[b][:, dci,
                                                     512 * tB : 512 * tB + 512],
                                            start=(dci == 0),
                                            stop=(dci == DC - 1),
                                        )
                                for tB in range(TW):
                                    c_sb = ph1b.tile([128, 512], BF16, tag="c_sb")
                                    nc.scalar.activation(c_sb[:], ps4[tB][:],
                                                         AF.Copy)
                                    # the wrap copy (cp=1) is only ever read
                                    # up to column 2559, which tile tB=0's
                                    # second copy covers for every row: skip
                                    # the other three (37% fewer packets)
                                    cps = (0, 1) if tB == 0 else (0,)
                                    for cp in cps:
                                        eng = nc.sync if (tB + cp) % 2 == 0 \
                                            else nc.scalar
                                        dst = bass.AP(
                                            bufA[:].tensor,
                                            127 + 512 * tB + 2048 * cp,
                                            [[2687, 128], [1, 512]],
                                        )
                                        eng.dma_start(dst, c_sb[:])
                                wa = ph1w.tile([128, 2432], BF16, tag="wa")
                                nc.sync.dma_start(
                                    wa[:],
                                    bass.AP(bufA[:].tensor, 128,
                                            [[2688, 128], [1, 2432]]),
                                )
                                pend.append((A, wa))
                                if len(pend) > 2:
                                    _emit_mv(*pend.pop(0))
                            for a_w in pend:
                                _emit_mv(*a_w)
                            for cc in range(4):
                                nc.scalar.activation(
                                    mvf[0:1,
                                        L * b + 512 * cc : L * b + 512 * cc + 512],
                                    mv_reg[cc][0:1, :], AF.Copy,
                                )
                            # issue this batch's AllReduce immediately: batch
                            # 0's collective hides under batch 1's phase 1.
                            cci = dr.tile([1, L], F32, tag=f"cci{b}")
                            cc_o = dr.tile([1, L], F32, tag=f"cco{b}")
                            nc.sync.dma_start(cci[:], mvf[0:1, L * b : L * b + L])
                            nc.gpsimd.collective_compute(
                                "AllReduce", AluOpType.add,
                                replica_groups=[list(range(n_group))],
                                ins=[cci[:].opt()], outs=[cc_o[:].opt()],
                            )
                            cco.append(cc_o)
                            # local + summed statistic back in [16,128]
                            # layout for the masked-exp path (off critical
                            # path; 16 contiguous packets each)
                            nc.gpsimd.dma_start(
                                mvloc[:, b, :],
                                bass.AP(cci[:].tensor, cci[:].offset,
                                        [[128, 16], [1, 128]]),
                            )
                            nc.gpsimd.dma_start(
                                bs16[:, b, :],
                                bass.AP(cc_o[:].tensor, cc_o[:].offset,
                                        [[128, 16], [1, 128]]),
                            )

                    # ========= phase 2: topk + softmax (both batches) ========
                    with (
                        tc.tile_pool(name="ph2", bufs=1) as ph2,
                        tc.tile_pool(name="ph2w", bufs=1) as ph2w,
                    ):
                        wv_s = ph2w.tile([128, DC, D], FP8, tag="wv_s")
                        nc.sync.dma_start(wv_s[:], wv_d.ap())

                        def _vproj(b):
                            # bf16 x against fp8 Wvo; psum = SW * v'
                            for tm in range(TM):
                                pt = ps4[tm % 4]
                                for dci in range(DC):
                                    nc.tensor.matmul(
                                        pt[:],
                                        lhsT=xb[b][:, dci,
                                                   128 * tm : 128 * tm + 128],
                                        rhs=wv_s[:, dci, :],
                                        start=(dci == 0),
                                        stop=(dci == DC - 1),
                                    )
                                # reversed chunk order for the DoubleRow agg
                                nc.scalar.activation(
                                    v8[b][:, TM - 1 - tm, :], pt[:], AF.Copy,
                                    scale=SV / SW,
                                )

                        _vproj(0)  # hides the second collective
                        # top-22 threshold via gpsimd kth_largest: the lerped
                        # quantile at rank 21.5 lies strictly between the 22nd
                        # and 23rd largest values -> tie-free >= mask.
                        bsA = ph2.tile([128, 16], F32, tag="bsA")
                        bsB = ph2.tile([128, 16], F32, tag="bsB")
                        nc.sync.dma_start(
                            bsA[:], bass.AP(cco[0][:].tensor, 0, [[16, 128], [1, 16]])
                        )
                        nc.sync.dma_start(
                            bsB[:], bass.AP(cco[1][:].tensor, 0, [[16, 128], [1, 16]])
                        )
                        bs128 = ph2.tile([128, 16], F32, tag="bs128")
                        nc.vector.tensor_add(out=bs128[:], in0=bsA[:], in1=bsB[:])
                        kth = ph2.tile([1, 2], F32, tag="kth")
                        nc.gpsimd.kth_largest(
                            kth[:], bs128[:], 16, 24,
                            quantile=1.0 - (TOPK - 0.5) / (L - 1),
                        )
                        _vproj(1)  # hides kth_largest
                        # masked exp (unnormalized) on [16,128] per batch;
                        # normalization deferred to the agg PSUM drain.
                        thr16 = ph2.tile([16, 1], F32, tag="thr16")
                        nc.gpsimd.partition_broadcast(thr16[:], kth[0:1, 0:1],
                                                      channels=16)
                        bsum16 = ph2.tile([16, 128], F32, tag="bsum16")
                        nc.vector.tensor_add(out=bsum16[:], in0=bs16[:, 0, :],
                                             in1=bs16[:, 1, :])
                        mask16 = ph2.tile([16, 128], F32, tag="mask16")
                        nc.vector.tensor_scalar(
                            out=mask16[:], in0=bsum16[:], scalar1=thr16[:, 0:1],
                            scalar2=None, op0=AluOpType.is_ge,
                        )
                        # stt = stat*mask + (mask-1)*1e9: selected entries keep
                        # the exact statistic (no 1e9 roundtrip -- f32 at 1e9
                        # has quantum 64, which would wipe out the values).
                        neg916 = ph2.tile([16, 128], F32, tag="neg916")
                        nc.vector.tensor_scalar(
                            out=neg916[:], in0=mask16[:], scalar1=1.0,
                            scalar2=1.0e9,
                            op0=AluOpType.subtract, op1=AluOpType.mult,
                        )
                        stt = ph2.tile([16, 2 * 128], F32, tag="stt")
                        for b in range(BPC):
                            sl = stt[:, 128 * b : 128 * b + 128]
                            nc.vector.tensor_mul(
                                out=sl, in0=mvloc[:, b, :], in1=mask16[:],
                            )
                            nc.vector.tensor_add(out=sl, in0=sl, in1=neg916[:])
                        nc.scalar.activation(g16[:], stt[:], AF.Exp)
                        # periodic replication B[q] = g_f[q mod L]: 16-packet
                        # row write + 3-way step-0-source DRAM->DRAM blast
                        # (the row-step-2047 gbuf read needs 129 copies).
                        for b, eng in ((1, nc.sync), (0, nc.scalar)):
                            # two g copies up front so the blast replicates
                            # 2L units (half the packets)
                            hb0 = dr.tile([1, 2 * L], FP8, tag=f"hb0{b}")
                            for rep in range(2):
                                eng.dma_start(
                                    bass.AP(hb0[:].tensor,
                                            hb0[:].offset + rep * L,
                                            [[128, 16], [1, 128]]),
                                    g16[:, 128 * b : 128 * b + 128],
                                )
                            hbb = dr.tile([1, 130 * L], FP8, tag=f"hb{b}")
                            hb[b] = hbb
                            for (r0, r1), e2 in zip(
                                ((0, 22), (22, 44), (44, 65)),
                                (nc.sync, nc.scalar, nc.gpsimd),
                            ):
                                e2.dma_start(
                                    bass.AP(hbb[:].tensor,
                                            hbb[:].offset + 2 * L * r0,
                                            [[2 * L, r1 - r0], [1, 2 * L]]),
                                    bass.AP(hb0[:].tensor, hb0[:].offset,
                                            [[0, r1 - r0], [1, 2 * L]]),
                                )
                        # softmax normalizers 1/Z per batch (off critical path)
                        nc.tensor.matmul(ps4[0][0:1, 0:256], lhsT=ones16[:],
                                         rhs=g16[:], start=True, stop=True)
                        zrow = ph2.tile([1, 256], F32, tag="zrow")
                        nc.scalar.activation(zrow[:], ps4[0][0:1, 0:256],
                                             AF.Copy)
                        z2 = ph2.tile([1, 2], F32, tag="z2")
                        ztmp = ph2.tile([1, 256], F32, tag="ztmp")
                        for b in range(BPC):
                            nc.scalar.activation(
                                ztmp[0:1, 128 * b : 128 * b + 128],
                                zrow[0:1, 128 * b : 128 * b + 128], AF.Copy,
                                accum_out=z2[0:1, b : b + 1],
                            )
                        nc.vector.reciprocal(out=z2[:], in_=z2[:])
                        # agg psum carries sum g~ * (SV v'); fold 1/SV here
                        nc.vector.tensor_scalar(
                            out=z2[:], in0=z2[:], scalar1=1.0 / SV,
                            scalar2=None, op0=AluOpType.mult,
                        )
                        nc.gpsimd.partition_broadcast(zib[:], z2[0:1, :])

                # == phases 3-4, batch order b1 then b0 (decomp overlaps PE) ==
                with (
                    tc.tile_pool(name="p3", bufs=1) as p3,
                    tc.tile_pool(name="p3r", bufs=2) as p3r,
                    tc.tile_pool(name="p3s", bufs=1) as p3s,
                ):
                    gbufs = {}
                    # b1 (needed first) on all 3 queues; b0 on scalar/gpsimd
                    # only so the sync queue stays free for the xr loads that
                    # pace the b1 acx drains.
                    for b, split in (
                        (1, (((0, 43), nc.sync), ((43, 86), nc.scalar),
                             ((86, 128), nc.gpsimd))),
                        (0, (((0, 64), nc.scalar), ((64, 128), nc.gpsimd))),
                    ):
                        gbuf = p3.tile([128, 3968], FP8, tag=f"gbuf{b}")
                        gbufs[b] = gbuf
                        for (i0, i1), eng in split:
                            eng.dma_start(
                                gbuf[i0:i1, :],
                                bass.AP(hb[b][:].tensor,
                                        hb[b][:].offset + 127 + 2047 * i0,
                                        [[2047, i1 - i0], [1, 3968]]),
                            )
                    for b in (1, 0):
                        gbuf = gbufs[b]

                        def gpair(Bp, nw):
                            # [128, 2, 512] gbuf k-pair: v8 chunk j=2Bp is
                            # true time chunk TM-1-2Bp -> base column
                            # 512nw + 256Bp, +128 for the second k-subtile
                            gs = gbuf[:]
                            return bass.AP(
                                gs.tensor, gs.offset + 512 * nw + 256 * Bp,
                                [list(gs.ap[0]), [128, 2], [1, 512]],
                            )

                        # ac = sum_k g_k roll(v') + bop + x, drained straight
                        # to acx (Wo is folded into v' = x Wv Wo on the host);
                        # per-batch tiles so b0's drains don't WAR-stall on
                        # b1's decompose reads
                        acx = p3.tile([128, DC, L], F32, tag=f"acx{b}")
                        for dm in range(DC):
                            for Bp in range(TM // 2):
                                for nw in range(TW):
                                    nc.tensor.matmul(
                                        ps4[nw][:],
                                        lhsT=v8[b][:, 2 * Bp : 2 * Bp + 2,
                                                   128 * dm : 128 * dm + 128],
                                        rhs=gpair(Bp, nw),
                                        start=(Bp == 0),
                                        stop=(Bp == TM // 2 - 1),
                                        perf_mode=DR,
                                    )
                            for nw in range(TW):
                                xr = p3r.tile([128, 512], F32, tag="xr")
                                nc.sync.dma_start(
                                    xr[:],
                                    x_dm.ap()[b, :,
                                              dm * L + 512 * nw :
                                              dm * L + 512 * nw + 512],
                                )
                                act = p3r.tile([128, 512], F32, tag="act")
                                nc.scalar.activation(
                                    act[:], ps4[nw][:], AF.Identity,
                                    scale=zib[:, b : b + 1],
                                    bias=bop_c[:, dm : dm + 1],
                                )
                                nc.vector.tensor_add(
                                    out=acx[:, dm, 512 * nw : 512 * nw + 512],
                                    in0=act[:], in1=xr[:],
                                )
                            # this channel chunk of acx is complete: its
                            # decomposition scan can overlap the next chunks'
                            # matmuls (and the following conv stages)
                            _decompose_chunk(nc, p3s, acx, seab[b], dm)

            # ======= phases 5-7, interleaved across the two batches =======
            with (
                tc.tile_pool(name="pcv", bufs=1) as pcv,
                tc.tile_pool(name="p5w1", bufs=2) as p5w1,
                tc.tile_pool(name="p5w2", bufs=2) as p5w2,
                tc.tile_pool(name="p6s", bufs=2) as p6s,
                tc.tile_pool(name="p7s", bufs=1) as p7s,
                tc.tile_pool(name="p7b", bufs=1) as p7b,
                tc.tile_pool(name="p7d", bufs=2, space="DRAM") as p7d,
                tc.tile_pool(name="ppc2", bufs=1, space="PSUM") as ppc2,
            ):
                pc2 = []
                for i in range(4):
                    t = ppc2.tile([128, 512], F32, tag=f"pc2_{i}", name=f"pc2_{i}")
                    pc2.append(t)
                h1 = pcv.tile([128, CH, L + 2], BF16, tag="h1")
                acc2 = pcv.tile([128, DC, L], BF16, tag="acc2")
                ysb = pcv.tile([128, DC, L], F32, tag="ysb")
                sea2 = pcv.tile([128, DC, L + 2], F32, tag="sea2")

                def conv1_half(b, half):
                    for co8 in range(CH):
                        co = CH * half + co8
                        w1t = p5w1.tile([128, DC * 3 * 128], BF16, tag="w1t")
                        nc.sync.dma_start(w1t[:], w1_d.ap()[co])
                        first = True
                        for dci in range(DC):
                            for tap in range(3):
                                ki = (3 * dci + tap) * 128
                                for nw in range(TW):
                                    nc.tensor.matmul(
                                        ps4[nw][:],
                                        lhsT=w1t[:, ki : ki + 128],
                                        rhs=seab[b][:, dci,
                                             512 * nw + tap : 512 * nw + tap + 512],
                                        start=first,
                                        stop=(dci == DC - 1 and tap == 2),
                                    )
                                first = False
                        for nw in range(TW):
                            nc.scalar.activation(
                                h1[:, co8, 1 + 512 * nw : 513 + 512 * nw],
                                ps4[nw][:], AF.Prelu, alpha=SLOPE,
                            )
                        nc.vector.tensor_copy(out=h1[:, co8, 0:1],
                                              in_=h1[:, co8, 1:2])
                        nc.vector.tensor_copy(out=h1[:, co8, L + 1 : L + 2],
                                              in_=h1[:, co8, L : L + 1])

                def conv2_pass(b, half):
                    for co in range(DC):
                        w2t = p5w2.tile([128, CH * 3 * 128], BF16, tag="w2t")
                        src = w2_d.ap()[co]
                        ofs = half * CH * 3 * 128
                        nc.sync.dma_start(
                            w2t[:],
                            bass.AP(src.tensor, src.offset + ofs,
                                    [[CFC * 3 * 128, 128], [1, CH * 3 * 128]]),
                        )
                        first = True
                        for ci8 in range(CH):
                            for tap in range(3):
                                ki = (3 * ci8 + tap) * 128
                                for nw in range(TW):
                                    nc.tensor.matmul(
                                        pc2[nw][:],
                                        lhsT=w2t[:, ki : ki + 128],
                                        rhs=h1[:, ci8,
                                             512 * nw + tap : 512 * nw + tap + 512],
                                        start=first,
                                        stop=(ci8 == CH - 1 and tap == 2),
                                    )
                                first = False
                        if half == 0:
                            for nw in range(TW):
                                nc.scalar.activation(
                                    acc2[:, co, 512 * nw : 512 * nw + 512],
                                    pc2[nw][:], AF.Copy,
                                )
                        else:
                            for nw in range(TW):
                                h2t = p6s.tile([128, 512], F32, tag="h2t")
                                nc.vector.tensor_add(
                                    out=h2t[:], in0=pc2[nw][:],
                                    in1=acc2[:, co, 512 * nw : 512 * nw + 512],
                                )
                                h2r = p6s.tile([128, 512], F32, tag="h2r")
                                nc.scalar.activation(h2r[:], h2t[:], AF.Prelu,
                                                     alpha=SLOPE)
                                nc.vector.tensor_add(
                                    out=ysb[:, co, 512 * nw : 512 * nw + 512],
                                    in0=h2r[:],
                                    in1=seab[b][:, co,
                                                1 + 512 * nw : 513 + 512 * nw],
                                )
                            # decompose2 of this channel chunk can start now
                            _decompose_chunk(nc, p7s, ysb, sea2, co)

                def layernorm(b):
                    # windowed pipeline; stats borrow conv2's PSUM banks
                    for twi in range(TW):
                        st_s = pc2[2 * (twi % 2)][0:1, :]
                        st_q = pc2[2 * (twi % 2) + 1][0:1, :]
                        for dci in range(DC):
                            sqt = p6s.tile([128, 512], F32, tag="sqt")
                            nc.scalar.activation(
                                sqt[:],
                                sea2[:, dci, 1 + 512 * twi : 513 + 512 * twi],
                                AF.Square,
                            )
                            nc.tensor.matmul(
                                st_s,
                                lhsT=ones_mv[:],
                                rhs=sea2[:, dci, 1 + 512 * twi : 513 + 512 * twi],
                                start=(dci == 0), stop=(dci == DC - 1),
                            )
                            nc.tensor.matmul(
                                st_q,
                                lhsT=ones_mv[:],
                                rhs=sqt[:],
                                start=(dci == 0), stop=(dci == DC - 1),
                            )
                        mu = p7b.tile([1, 512], F32, tag="mu")
                        rs = p7b.tile([1, 512], F32, tag="rs")
                        nc.scalar.activation(mu[:], st_s, AF.Copy)
                        nc.vector.tensor_mul(out=rs[:], in0=mu[:], in1=mu[:])
                        nc.vector.tensor_sub(out=rs[:], in0=st_q, in1=rs[:])
                        nc.vector.tensor_scalar_add(rs[:], rs[:], EPS)
                        nc.vector.reciprocal(out=rs[:], in_=rs[:])
                        nc.scalar.activation(rs[:], rs[:], AF.Sqrt)
                        mub = p7b.tile([128, 512], F32, tag="mub")
                        rsb = p7b.tile([128, 512], F32, tag="rsb")
                        nc.gpsimd.partition_broadcast(mub[:], mu[:])
                        nc.gpsimd.partition_broadcast(rsb[:], rs[:])
                        for dci in range(DC):
                            ve = nc.vector
                            og = p6s.tile([128, 512], F32, tag="og")
                            ve.tensor_sub(
                                out=og[:],
                                in0=sea2[:, dci, 1 + 512 * twi : 513 + 512 * twi],
                                in1=mub[:],
                            )
                            ve.scalar_tensor_tensor(
                                out=og[:], in0=og[:],
                                scalar=lng_c[:, dci : dci + 1], in1=rsb[:],
                                op0=AluOpType.mult, op1=AluOpType.mult,
                            )
                            nc.scalar.activation(
                                og[:], og[:], AF.Identity,
                                bias=lnb_c[:, dci : dci + 1],
                            )
                            nc.scalar.dma_start(
                                out_dm.ap()[b, :, dci,
                                            512 * twi : 512 * twi + 512],
                                og[:],
                            )

                # schedule: conv(b1) fully; LN(b1) sits between batch-0 conv
                # stages so its vector/DMA chains hide under PE work.
                conv1_half(1, 0)
                conv2_pass(1, 0)
                conv1_half(1, 1)
                conv2_pass(1, 1)      # finalize emits decompose2(b1) per chunk
                conv1_half(0, 0)
                layernorm(1)          # stats borrow pc2 (free here)
                conv2_pass(0, 0)
                conv1_half(0, 1)
                conv2_pass(0, 1)      # finalize emits decompose2(b0) per chunk
                layernorm(0)


# ---------------------------------------------------------------------------
# host side
# ---------------------------------------------------------------------------
_CACHE = {}


def _get_nc(n_group: int):
    key = n_group
    if key not in _CACHE:
        nc = bacc.Bacc("TRN2", target_bir_lowering=False, debug=False,
                       num_devices=n_group)
        build(nc, n_group)
        nc.compile()
        _CACHE[key] = nc
    return _CACHE[key]


def stage_inputs(inputs, ncores=NCORES):
    x = np.asarray(inputs["x"], np.float32)
    Wq = np.asarray(inputs["Wq"], np.float32)
    Wk = np.asarray(inputs["Wk"], np.float32)
    Wv = np.asarray(inputs["Wv"], np.float32)
    Wo = np.asarray(inputs["Wo"], np.float32)
    bq = np.asarray(inputs["bq"], np.float32)
    bk = np.asarray(inputs["bk"], np.float32)
    bv = np.asarray(inputs["bv"], np.float32)
    bo = np.asarray(inputs["bo"], np.float32)
    w1 = np.asarray(inputs["conv1_w"], np.float32)
    w2 = np.asarray(inputs["conv2_w"], np.float32)
    lng = np.asarray(inputs["ln_g"], np.float32)
    lnb = np.asarray(inputs["ln_b"], np.float32)

    bop = bo + bv @ Wo
    # merged QK projection: the statistic only needs x^T (Wq Wk^T) x
    # (biases drop: they shift the statistic by a constant, and both topk
    # and softmax are shift-invariant). Wo folds into the V projection.
    Wm = Wq @ Wk.T
    Wvo = Wv @ Wo
    col = lambda v: np.ascontiguousarray(v.reshape(DC, 128).T)
    # projection weights partition-major: W[dci*128+p, n] -> [p, dci*D + n]
    wmaj = lambda W: np.ascontiguousarray(
        W.reshape(DC, 128, D).transpose(1, 0, 2).reshape(128, DC * D)
    ).astype(BF16_NP)
    # same, scaled into the e4m3 normal range
    wmaj8 = lambda W: np.ascontiguousarray(
        np.clip(W * SW, -240, 240)
        .reshape(DC, 128, D).transpose(1, 0, 2).reshape(128, DC * D)
    ).astype(FP8_NP)
    # conv1 [3, D, CF] -> [CFC, 128(ci-part), DC*3*128(co)]
    w1h = np.ascontiguousarray(
        w1.reshape(3, DC, 128, CFC, 128).transpose(3, 2, 1, 0, 4)
    ).reshape(CFC, 128, DC * 3 * 128).astype(BF16_NP)
    # conv2 [3, CF, D] -> [DC, 128(ci-part), CFC*3*128(co)]
    w2h = np.ascontiguousarray(
        w2.reshape(3, CFC, 128, DC, 128).transpose(3, 2, 1, 0, 4)
    ).reshape(DC, 128, CFC * 3 * 128).astype(BF16_NP)

    shared = {
        "wm": wmaj(Wm), "wv": wmaj8(Wvo),
        "bop_t": col(bop),
        "w1h": w1h, "w2h": w2h, "lng_t": col(lng), "lnb_t": col(lnb),
    }
    bpc = B // ncores
    in_maps = []
    for c in range(ncores):
        m = dict(shared)
        # x [bpc, L, D] -> [bpc, 128, DC*L]  (d-major per partition)
        xc = x[bpc * c : bpc * (c + 1)]
        xc = np.ascontiguousarray(
            xc.reshape(bpc, L, DC, 128).transpose(0, 3, 2, 1)
        ).reshape(bpc, 128, DC * L)
        m["x_dm"] = xc
        in_maps.append(m)
    return in_maps


def unstage_output(res, ncores=NCORES):
    out = np.empty((B, L, D), np.float32)
    bpc = B // ncores
    for c in range(ncores):
        o = np.asarray(res.results[c]["out_dm"])  # [bpc, 128, DC, L]
        for i in range(bpc):
            # full[t, dci*128+p] = o[i][p, dci, t]
            out[bpc * c + i] = o[i].transpose(2, 1, 0).reshape(L, D)
    return out


def kernel(**inputs):
    nc = _get_nc(NCORES)
    in_maps = stage_inputs(inputs)
    res = bass_utils.run_bass_kernel_spmd(nc, in_maps, core_ids=list(range(NCORES)))
    return unstage_output(res)

